# revision 16
# baseline (speedup 1.0000x reference)
"""Trainium2 Bass kernel for the masked set-transformer encoder (ISAB stack).

Strategy (pure data parallel, B=16 over 8 cores, 2 batch elements/core):
  * The compaction permutation commutes with the whole network: softmax over
    keys is permutation invariant, everything else is row-wise, and masked
    rows are exactly zero throughout.  So the host only computes the stable
    argsort *indices*; the device processes tokens in compacted order and the
    output is already compacted (zero tail appended on device).
  * Only NT = ceil(max_b nvalid_b / 128) tiles of 128 tokens are processed
    (~42 instead of 82 for random masks).  NT is a compile-time constant
    derived from the actual mask; the program is recompiled if it changes.
  * The one-hot input FF collapses to a [43,128] table matmul against a
    device-built X^T = [one_hot(c); t*mk; u*mk] (no gathers: one-hot rows are
    built with an is_equal against an iota column; invalid tokens get an
    out-of-range channel id so their X^T column is exactly zero).
  * Attention layouts keep softmax reductions on natural axes:
      MAB0 scores  S^T[tok,(h,q)] = Z @ G0,  G0 = fold(Wk, I@Wq+bq)  (host)
      MAB0 key masking is folded into the exp bias: exp(s*scale + (mk-1)*30)
      MAB0 num/den via lhsT=E^T_h, rhs=[Vh_h | 1], DVE-accumulated over chunks
      MAB1 scores  S1^T[k, tok] per head via lhsT=G1_h, rhs=Z^T (4-chunk tiles)
      MAB1 num/den via lhsT=E1^T_h, rhs=[Vh1_h | 1]
  * Z^T lives in SBUF as [128, 512] group tiles so MAB1 score matmuls stream
    512 tokens per instruction.
  * ACT does exp only; biases are all zero in practice (trace-time fallbacks
    emit extra ones-row matmuls / bias adds when they are not).
"""

import math

import numpy as np

import concourse.bacc as bacc
import concourse.bass as bass
import concourse.mybir as mybir
import concourse.tile as tile
from concourse.bass_utils import run_bass_kernel_spmd
from concourse.masks import make_identity

F32 = mybir.dt.float32
AF = mybir.ActivationFunctionType
OP = mybir.AluOpType

B, S, D = 16, 256, 41
L = S * D                      # 10496
LATENT, NREF, NLAYERS, NHEADS = 128, 128, 3, 4
DH = LATENT // NHEADS          # 32
SCALE = 1.0 / math.sqrt(LATENT)
NCORES = 8
BPC = B // NCORES              # 2
CMAX = L // 128                # 82
NEGBIG = -30.0                 # exp(-30) ~ 1e-13: masked-key contribution
GW = 4                         # chunks per Z^T group tile

# set by test harness to capture profiling info
TRACE = False
LAST_RESULT = None

_PROG_CACHE: dict = {}


def _build_program(NT: int, flags: dict, nlayers: int = NLAYERS):
    NTP = NT * 128
    NG = -(-NT // GW)          # number of Z^T group tiles
    nc = bacc.Bacc("TRN2")

    def gslice(c):
        """(group index, column slice within the group tile) for chunk c."""
        return c // GW, slice((c % GW) * 128, (c % GW) * 128 + 128)

    # ---------------- DRAM I/O ----------------
    d_cidx = nc.dram_tensor("cidx", [BPC, NTP], F32, kind="ExternalInput")
    d_tmk = nc.dram_tensor("tmk", [BPC, NTP], F32, kind="ExternalInput")
    d_umk = nc.dram_tensor("umk", [BPC, NTP], F32, kind="ExternalInput")
    d_mk = nc.dram_tensor("mk", [BPC, NTP], F32, kind="ExternalInput")
    d_wtab = nc.dram_tensor("wtab", [43, LATENT], F32, kind="ExternalInput")
    d_g0 = nc.dram_tensor("g0", [NLAYERS, LATENT, 512], F32, kind="ExternalInput")
    d_qh0 = nc.dram_tensor("qh0", [NLAYERS, NREF, LATENT], F32, kind="ExternalInput")
    WNAMES = ["wv0", "wo0", "wq1", "wq1t", "wk1", "wv1", "wo1"]
    d_w = {
        n: nc.dram_tensor(n, [NLAYERS, LATENT, LATENT], F32, kind="ExternalInput")
        for n in WNAMES
    }
    BNAMES = ["r0", "bv0", "bo0", "bq1", "bk1", "bv1", "bo1"]
    d_b = {
        n: nc.dram_tensor(n, [NLAYERS, 512 if n == "r0" else LATENT], F32,
                          kind="ExternalInput")
        for n in BNAMES
    }
    d_zout = nc.dram_tensor("zout", [BPC, L, LATENT], F32, kind="ExternalOutput")
    d_mkout = nc.dram_tensor("mkout", [BPC, L], F32, kind="ExternalOutput")

    with tile.TileContext(nc) as tc:
        with (
            tc.tile_pool(name="persist", bufs=1) as pp,
            tc.tile_pool(name="work", bufs=2) as wp,
            tc.tile_pool(name="stream", bufs=4) as sp,
            tc.tile_pool(name="ps_sc", bufs=2, space="PSUM") as ps_sc,
            tc.tile_pool(name="ps_n1", bufs=2, space="PSUM") as ps_n1,
            tc.tile_pool(name="ps_sm", bufs=2, space="PSUM") as ps_sm,
        ):
            # ---------------- constants & weights ----------------
            ident = pp.tile([128, 128], F32, name="ident")
            make_identity(nc, ident[:])

            iota_i = pp.tile([41, 1], mybir.dt.int32, name="iota_i")
            nc.gpsimd.iota(iota_i[:], [[1, 1]], channel_multiplier=1)
            iota_f = pp.tile([41, 1], F32, name="iota_f")
            nc.vector.tensor_copy(iota_f[:], iota_i[:])

            ones_row = pp.tile([1, 128], F32, name="ones_row")
            nc.vector.memset(ones_row[:], 1.0)
            zerot = pp.tile([128, 512], F32, name="zerot")
            nc.vector.memset(zerot[:], 0.0)

            wtab_s = pp.tile([43, LATENT], F32, name="wtab_s")
            nc.sync.dma_start(wtab_s[:], d_wtab[:, :])

            g0_s, qh0_s = [], []
            w_s = {n: [] for n in WNAMES}
            b_s = {n: [] for n in BNAMES}
            for l in range(NLAYERS):
                g = pp.tile([LATENT, 512], F32, name=f"g0s{l}", tag=f"g0s{l}")
                nc.sync.dma_start(g[:], d_g0[l, :, :])
                g0_s.append(g)
                q = pp.tile([NREF, LATENT], F32, name=f"qh0s{l}", tag=f"qh0s{l}")
                nc.sync.dma_start(q[:], d_qh0[l, :, :])
                qh0_s.append(q)
                for n in WNAMES:
                    if n == "wq1t":
                        # per-head [32,128] tiles (PE weights must start at
                        # partition 0/32/64, so a [96:128] slice is illegal)
                        hh_tiles = []
                        for h in range(NHEADS):
                            t = pp.tile([DH, LATENT], F32, name=f"wq1t{l}h{h}",
                                        tag=f"wq1t{l}h{h}")
                            nc.sync.dma_start(
                                t[:], d_w[n][l, h * DH:(h + 1) * DH, :])
                            hh_tiles.append(t)
                        w_s[n].append(hh_tiles)
                        continue
                    t = pp.tile([LATENT, LATENT], F32, name=f"{n}s{l}", tag=f"{n}s{l}")
                    nc.sync.dma_start(t[:], d_w[n][l, :, :])
                    w_s[n].append(t)
                for n in BNAMES:
                    if not flags[n]:
                        b_s[n].append(None)
                        continue
                    if n in ("bk1",):        # needed as a [128,1] column
                        t = pp.tile([LATENT, 1], F32, name=f"{n}s{l}", tag=f"{n}s{l}")
                        nc.sync.dma_start(
                            t[:], bass.AP(d_b[n], l * LATENT, [[1, LATENT], [1, 1]]))
                    elif n == "bq1":         # per-head column tiles [32,1]
                        t = []
                        for h in range(NHEADS):
                            th = pp.tile([DH, 1], F32, name=f"{n}c{l}h{h}",
                                         tag=f"{n}c{l}h{h}")
                            nc.sync.dma_start(
                                th[:], bass.AP(d_b[n], l * LATENT + h * DH,
                                               [[1, DH], [1, 1]]))
                            t.append(th)
                    else:
                        w = 512 if n == "r0" else LATENT
                        t = pp.tile([1, w], F32, name=f"{n}s{l}", tag=f"{n}s{l}")
                        nc.sync.dma_start(t[:], d_b[n][l:l + 1, :])
                    b_s[n].append(t)
                if flags["bq1"]:  # row form for the ones-matmul into Qh1
                    t = pp.tile([1, LATENT], F32, name=f"bq1rs{l}", tag=f"bq1rs{l}")
                    nc.sync.dma_start(t[:], d_b["bq1"][l:l + 1, :])
                    b_s.setdefault("bq1r", []).append(t)

            # ---------------- per-batch setup + Z0 ----------------
            mkp_s, mkneg_s, mkt_s, ZT = [], [], [], []
            for b in range(BPC):
                mkt = pp.tile([NT, 128], F32, name=f"mkt{b}", tag=f"mkt{b}")
                nc.sync.dma_start(mkt[:], bass.AP(d_mk, b * NTP, [[128, NT], [1, 128]]))
                mkt_s.append(mkt)
                mkpp = ps_sm.tile([128, NT], F32, name="mkpp", tag="sm")
                nc.tensor.transpose(mkpp[:], mkt[:], ident[0:NT, 0:NT])
                mkp = pp.tile([128, NT], F32, name=f"mkp{b}", tag=f"mkp{b}")
                nc.vector.tensor_copy(mkp[:], mkpp[:])
                mkp_s.append(mkp)
                mkneg = pp.tile([128, NT], F32, name=f"mkneg{b}", tag=f"mkneg{b}")
                nc.vector.tensor_scalar(
                    mkneg[:], mkp[:], -1.0, -NEGBIG, op0=OP.add, op1=OP.mult)
                mkneg_s.append(mkneg)

                xt = pp.tile([43, NTP], F32, name=f"xt{b}", tag=f"xt{b}")
                crow = pp.tile([1, NTP], F32, name=f"crow{b}", tag=f"crow{b}")
                nc.sync.dma_start(crow[:], d_cidx[b:b + 1, :])
                # replicate cidx row across 41 partitions via a K=1 matmul,
                # then one-hot it against the iota column
                for j in range(0, NTP, 512):
                    w = min(512, NTP - j)
                    cb = ps_sm.tile([41, 512], F32, name="cb", tag="sm")
                    nc.tensor.matmul(cb[:, :w], lhsT=ones_row[:, 0:41],
                                     rhs=crow[:, j:j + w], start=True, stop=True)
                    nc.vector.tensor_scalar(
                        xt[0:41, j:j + w], cb[:, :w], iota_f[:], None,
                        op0=OP.is_equal)
                nc.sync.dma_start(xt[41:42, :], d_tmk[b:b + 1, :])
                nc.sync.dma_start(xt[42:43, :], d_umk[b:b + 1, :])

                ztg = []
                for g in range(NG):
                    w = min(GW * 128, NTP - g * GW * 128)
                    zt = pp.tile([128, GW * 128], F32, name=f"zt{b}_{g}",
                                 tag=f"zt{b}_{g}")
                    ztg.append(zt)
                for c in range(NT):
                    g, js = gslice(c)
                    z0p = ps_sm.tile([128, 128], F32, name="z0p", tag="sm")
                    nc.tensor.matmul(
                        z0p[:], lhsT=wtab_s[:], rhs=xt[:, c * 128:(c + 1) * 128],
                        start=True, stop=True)
                    nc.vector.tensor_scalar_max(ztg[g][:, js], z0p[:], 0.0)
                ZT.append(ztg)

            # ---------------- layers ----------------
            for l in range(nlayers):
                for b in range(BPC):
                    ztg = ZT[b]
                    # ===== MAB0: induced points attend to data =====
                    num0 = wp.tile([128, 132], F32, name="num0", tag="num0")
                    nc.vector.memset(num0[:], 0.0)
                    for c in range(NT):
                        g, js = gslice(c)
                        s0 = ps_sc.tile([128, 512], F32, name="s0", tag="sc")
                        if flags["r0"]:
                            nc.tensor.matmul(s0[:], lhsT=ones_row[:],
                                             rhs=b_s["r0"][l][:],
                                             start=True, stop=False)
                        nc.tensor.matmul(s0[:], lhsT=ztg[g][:, js], rhs=g0_s[l][:],
                                         start=not flags["r0"], stop=True)
                        et = sp.tile([128, 512], F32, name="et", tag="et")
                        nc.scalar.activation(et[:], s0[:], AF.Exp,
                                             bias=mkneg_s[b][:, c:c + 1],
                                             scale=SCALE)
                        vh = ps_sm.tile([128, 128], F32, name="vh", tag="sm")
                        if flags["bv0"]:
                            nc.tensor.matmul(vh[:], lhsT=ones_row[:],
                                             rhs=b_s["bv0"][l][:],
                                             start=True, stop=False)
                        nc.tensor.matmul(vh[:], lhsT=ztg[g][:, js],
                                         rhs=w_s["wv0"][l][:],
                                         start=not flags["bv0"], stop=True)
                        vo = sp.tile([128, 132], F32, name="vo", tag="vo")
                        nc.vector.memset(vo[:], 1.0)
                        nc.vector.tensor_copy(
                            vo[:].rearrange("p (h x) -> p h x", x=33)[:, :, 0:32],
                            vh[:].rearrange("p (h x) -> p h x", x=32))
                        n0c = ps_n1.tile([128, 132], F32, name="n0c", tag="n0c")
                        for h in range(NHEADS):
                            nc.tensor.matmul(
                                n0c[:, h * 33:(h + 1) * 33],
                                lhsT=et[:, h * 128:(h + 1) * 128],
                                rhs=vo[:, h * 33:(h + 1) * 33],
                                start=True, stop=True)
                        nc.vector.tensor_add(num0[:], num0[:], n0c[:])
                    rd0 = sp.tile([128, 4], F32, name="rd0", tag="rd0")
                    nc.vector.reciprocal(
                        rd0[:].rearrange("p (h x) -> p h x", x=1),
                        num0[:].rearrange("p (h x) -> p h x", x=33)[:, :, 32:33])
                    o0 = wp.tile([128, 128], F32, name="o0", tag="o0")
                    nc.vector.tensor_tensor(
                        o0[:].rearrange("p (h x) -> p h x", x=32),
                        num0[:].rearrange("p (h x) -> p h x", x=33)[:, :, 0:32],
                        rd0[:].rearrange("p (h x) -> p h x", x=1).to_broadcast(
                            [128, 4, 32]),
                        op=OP.mult)
                    nc.vector.tensor_add(o0[:], o0[:], qh0_s[l][:])
                    o0tp = ps_sm.tile([128, 128], F32, name="o0tp", tag="sm")
                    nc.tensor.transpose(o0tp[:], o0[:], ident[:])
                    o0t = wp.tile([128, 128], F32, name="o0t", tag="o0t")
                    nc.vector.tensor_copy(o0t[:], o0tp[:])
                    fc0 = ps_sm.tile([128, 128], F32, name="fc0", tag="sm")
                    if flags["bo0"]:
                        nc.tensor.matmul(fc0[:], lhsT=ones_row[:],
                                         rhs=b_s["bo0"][l][:], start=True, stop=False)
                    nc.tensor.matmul(fc0[:], lhsT=o0t[:], rhs=w_s["wo0"][l][:],
                                     start=not flags["bo0"], stop=True)
                    hh = wp.tile([128, 128], F32, name="hh", tag="hh")
                    nc.vector.scalar_tensor_tensor(
                        hh[:], in0=fc0[:], scalar=0.0, in1=o0[:],
                        op0=OP.max, op1=OP.add)
                    htp = ps_sm.tile([128, 128], F32, name="htp", tag="sm")
                    nc.tensor.transpose(htp[:], hh[:], ident[:])
                    ht = wp.tile([128, 128], F32, name="ht", tag="ht")
                    nc.vector.tensor_copy(ht[:], htp[:])

                    # ===== MAB1 prep =====
                    kh1p = ps_sm.tile([128, 128], F32, name="kh1p", tag="sm")
                    nc.tensor.matmul(kh1p[:], lhsT=w_s["wk1"][l][:], rhs=ht[:],
                                     start=True, stop=True)
                    kh1 = wp.tile([128, 128], F32, name="kh1", tag="kh1")
                    if flags["bk1"]:
                        nc.vector.tensor_scalar_add(kh1[:], kh1p[:], b_s["bk1"][l][:])
                    else:
                        nc.vector.tensor_copy(kh1[:], kh1p[:])
                    kh1h = []
                    for h in range(NHEADS):
                        t = wp.tile([DH, LATENT], F32, name=f"kh1h{h}", tag=f"kh1h{h}")
                        nc.sync.dma_start(t[:], kh1[h * DH:(h + 1) * DH, :])
                        kh1h.append(t)
                    g1p = ps_sc.tile([128, 512], F32, name="g1p", tag="sc")
                    for h in range(NHEADS):
                        nc.tensor.matmul(g1p[:, h * 128:(h + 1) * 128],
                                         lhsT=w_s["wq1t"][l][h][:], rhs=kh1h[h][:],
                                         start=True, stop=True)
                    g1 = wp.tile([128, 512], F32, name="g1", tag="g1")
                    nc.vector.tensor_copy(g1[:], g1p[:])
                    vh1p = ps_sm.tile([128, 128], F32, name="vh1p", tag="sm")
                    if flags["bv1"]:
                        nc.tensor.matmul(vh1p[:], lhsT=ones_row[:],
                                         rhs=b_s["bv1"][l][:], start=True, stop=False)
                    nc.tensor.matmul(vh1p[:], lhsT=ht[:], rhs=w_s["wv1"][l][:],
                                     start=not flags["bv1"], stop=True)
                    vo1 = wp.tile([128, 132], F32, name="vo1", tag="vo1")
                    nc.vector.memset(vo1[:], 1.0)
                    nc.vector.tensor_copy(
                        vo1[:].rearrange("p (h x) -> p h x", x=33)[:, :, 0:32],
                        vh1p[:].rearrange("p (h x) -> p h x", x=32))
                    r1b = None
                    if flags["bq1"]:
                        r1bp = ps_sm.tile([128, 4], F32, name="r1bp", tag="sm")
                        for h in range(NHEADS):
                            nc.tensor.matmul(r1bp[:, h:h + 1], lhsT=kh1h[h][:],
                                             rhs=b_s["bq1"][l][h][:],
                                             start=True, stop=True)
                        r1b = wp.tile([128, 4], F32, name="r1b", tag="r1b")
                        nc.vector.tensor_scalar_mul(r1b[:], r1bp[:], SCALE)

                    # ===== MAB1 chunks: data attends to induced =====
                    for g in range(NG):
                        gw = min(GW * 128, NTP - g * GW * 128)
                        e1h = []
                        for h in range(NHEADS):
                            s1 = ps_sc.tile([128, GW * 128], F32, name="s1", tag="sc")
                            nc.tensor.matmul(s1[:, :gw],
                                             lhsT=g1[:, h * 128:(h + 1) * 128],
                                             rhs=ztg[g][:, :gw],
                                             start=True, stop=True)
                            e1 = sp.tile([128, GW * 128], F32, name="e1", tag="e1")
                            if flags["bq1"]:
                                nc.scalar.activation(e1[:, :gw], s1[:, :gw], AF.Exp,
                                                     bias=r1b[:, h:h + 1], scale=SCALE)
                            else:
                                nc.scalar.activation(e1[:, :gw], s1[:, :gw], AF.Exp,
                                                     scale=SCALE)
                            e1h.append(e1)
                        for j in range(gw // 128):
                            c = g * GW + j
                            js = slice(j * 128, (j + 1) * 128)
                            num1 = ps_n1.tile([128, 132], F32, name="num1", tag="num1")
                            for h in range(NHEADS):
                                nc.tensor.matmul(num1[:, h * 33:(h + 1) * 33],
                                                 lhsT=e1h[h][:, js],
                                                 rhs=vo1[:, h * 33:(h + 1) * 33],
                                                 start=True, stop=True)
                            qh1 = ps_sm.tile([128, 128], F32, name="qh1", tag="sm")
                            if flags["bq1"]:
                                nc.tensor.matmul(qh1[:], lhsT=ones_row[:],
                                                 rhs=b_s["bq1r"][l][:],
                                                 start=True, stop=False)
                            nc.tensor.matmul(qh1[:], lhsT=ztg[g][:, js],
                                             rhs=w_s["wq1"][l][:],
                                             start=not flags["bq1"], stop=True)
                            rd1 = sp.tile([128, 4], F32, name="rd1", tag="rd1")
                            nc.vector.reciprocal(
                                rd1[:].rearrange("p (h x) -> p h x", x=1),
                                num1[:].rearrange("p (h x) -> p h x", x=33)[:, :, 32:33])
                            o1 = sp.tile([128, 128], F32, name="o1", tag="o1")
                            nc.vector.tensor_tensor(
                                o1[:].rearrange("p (h x) -> p h x", x=32),
                                num1[:].rearrange("p (h x) -> p h x", x=33)[:, :, 0:32],
                                rd1[:].rearrange("p (h x) -> p h x", x=1).to_broadcast(
                                    [128, 4, 32]),
                                op=OP.mult)
                            nc.vector.tensor_add(o1[:], o1[:], qh1[:])
                            o1tp = ps_sm.tile([128, 128], F32, name="o1tp", tag="sm")
                            nc.tensor.transpose(o1tp[:], o1[:], ident[:])
                            o1t = sp.tile([128, 128], F32, name="o1t", tag="o1t")
                            nc.vector.tensor_copy(o1t[:], o1tp[:])
                            fc1 = ps_sm.tile([128, 128], F32, name="fc1", tag="sm")
                            if flags["bo1"]:
                                nc.tensor.matmul(fc1[:], lhsT=ones_row[:],
                                                 rhs=b_s["bo1"][l][:],
                                                 start=True, stop=False)
                            nc.tensor.matmul(fc1[:], lhsT=o1t[:], rhs=w_s["wo1"][l][:],
                                             start=not flags["bo1"], stop=True)
                            u = sp.tile([128, 128], F32, name="u", tag="u")
                            nc.vector.scalar_tensor_tensor(
                                u[:], in0=fc1[:], scalar=0.0, in1=o1[:],
                                op0=OP.max, op1=OP.add)
                            nc.vector.tensor_scalar_mul(u[:], u[:],
                                                        mkp_s[b][:, c:c + 1])
                            utp = ps_sm.tile([128, 128], F32, name="utp", tag="sm")
                            nc.tensor.transpose(utp[:], u[:], ident[:])
                            nc.vector.tensor_add(ztg[g][:, js], ztg[g][:, js], utp[:])

            # ---------------- output ----------------
            for b in range(BPC):
                for c in range(NT):
                    g, js = gslice(c)
                    zp = ps_sm.tile([128, 128], F32, name="zp", tag="sm")
                    nc.tensor.transpose(zp[:], ZT[b][g][:, js], ident[:])
                    zo = sp.tile([128, 128], F32, name="zo", tag="zo")
                    nc.vector.tensor_copy(zo[:], zp[:])
                    nc.sync.dma_start(
                        bass.AP(d_zout, (b * L + c * 128) * LATENT,
                                [[LATENT, 128], [1, LATENT]]),
                        zo[:])
                # zero tail rows [NTP, L): 2KB-contiguous descriptor runs
                r = NTP
                while r < L:
                    n = min(512, L - r)   # rows; n*128 elems; dst stays contiguous
                    nelem = n * LATENT
                    inner = nelem // 128
                    nc.sync.dma_start(
                        bass.AP(d_zout, (b * L + r) * LATENT,
                                [[inner, 128], [1, inner]]),
                        zerot[:, :inner])
                    r += n
                # mask output (contiguous 512B rows from the token-major tile)
                nc.sync.dma_start(
                    bass.AP(d_mkout, b * L, [[128, NT], [1, 128]]), mkt_s[b][:])
                if CMAX > NT:
                    nc.sync.dma_start(
                        bass.AP(d_mkout, b * L + NTP, [[128, CMAX - NT], [1, 128]]),
                        zerot[0:CMAX - NT, 0:128])
    nc.compile()
    return nc


def _prep(inputs):
    """Host-side prep: compaction indices + weight folding (all O(small))."""
    time_x = np.ascontiguousarray(np.asarray(inputs["time_x"], np.float32))
    value_x = np.ascontiguousarray(np.asarray(inputs["value_x"], np.float32))
    mask_x = np.asarray(inputs["mask_x"])
    Wi = np.asarray(inputs["Wi"], np.float32)
    bi = np.asarray(inputs["bi"], np.float32)
    I = np.asarray(inputs["I"], np.float32)
    Wq = np.asarray(inputs["Wq"], np.float32)
    bq = np.asarray(inputs["bq"], np.float32)
    Wk = np.asarray(inputs["Wk"], np.float32)
    bk = np.asarray(inputs["bk"], np.float32)
    Wv = np.asarray(inputs["Wv"], np.float32)
    bv = np.asarray(inputs["bv"], np.float32)
    Wo = np.asarray(inputs["Wo"], np.float32)
    bo = np.asarray(inputs["bo"], np.float32)

    mflat = mask_x.reshape(B, L)
    order = np.argsort(1 - mflat, axis=1, kind="stable")
    nvalid = int(mflat.sum(axis=1).max())
    NT = max(1, min(CMAX, -(-nvalid // 128)))
    NTP = NT * 128

    ordp = order[:, :NTP]
    mkc = np.take_along_axis(mflat, ordp, axis=1).astype(np.float32)
    tfull = np.broadcast_to(time_x[:, :, None], (B, S, D)).reshape(B, L)
    tmk = np.take_along_axis(tfull, ordp, axis=1) * mkc
    umk = np.take_along_axis(value_x.reshape(B, L), ordp, axis=1) * mkc
    cidx = np.where(mkc > 0, (ordp % D).astype(np.float32), 63.0).astype(np.float32)

    wtab = np.concatenate([Wi[:D] + bi[None, :], Wi[D:D + 2]], 0)  # [43,128]

    g0 = np.zeros((NLAYERS, LATENT, 512), np.float32)
    qh0 = np.zeros((NLAYERS, NREF, LATENT), np.float32)
    r0 = np.zeros((NLAYERS, 512), np.float32)
    for l in range(NLAYERS):
        Q = I[l] @ Wq[l, 0] + bq[l, 0]
        qh0[l] = Q
        for h in range(NHEADS):
            hs = slice(h * DH, (h + 1) * DH)
            g0[l][:, h * NREF:(h + 1) * NREF] = Wk[l, 0][:, hs] @ Q[:, hs].T
            r0[l][h * NREF:(h + 1) * NREF] = bk[l, 0][hs] @ Q[:, hs].T

    w = dict(
        wv0=Wv[:, 0], wo0=Wo[:, 0], wq1=Wq[:, 1],
        wq1t=np.ascontiguousarray(Wq[:, 1].transpose(0, 2, 1)),
        wk1=Wk[:, 1], wv1=Wv[:, 1], wo1=Wo[:, 1],
    )
    bvec = dict(r0=r0, bv0=bv[:, 0], bo0=bo[:, 0], bq1=bq[:, 1],
                bk1=bk[:, 1], bv1=bv[:, 1], bo1=bo[:, 1])
    flags = {n: bool(np.any(v != 0)) for n, v in bvec.items()}
    return dict(NT=NT, NTP=NTP, tmk=tmk, umk=umk, mkc=mkc, cidx=cidx,
                wtab=wtab, g0=g0, qh0=qh0, w=w, bvec=bvec, flags=flags)


def kernel(**inputs):
    global LAST_RESULT
    p = _prep(inputs)

    key = (p["NT"], tuple(sorted(p["flags"].items())))
    if key not in _PROG_CACHE:
        _PROG_CACHE[key] = _build_program(p["NT"], p["flags"])
    nc = _PROG_CACHE[key]

    shared = dict(wtab=np.ascontiguousarray(p["wtab"]),
                  g0=p["g0"], qh0=p["qh0"])
    for n, v in p["w"].items():
        shared[n] = np.ascontiguousarray(v)
    for n, v in p["bvec"].items():
        shared[n] = np.ascontiguousarray(v)

    in_maps = []
    for m in range(NCORES):
        sl = slice(m * BPC, (m + 1) * BPC)
        im = dict(shared)
        im["cidx"] = np.ascontiguousarray(p["cidx"][sl])
        im["tmk"] = np.ascontiguousarray(p["tmk"][sl])
        im["umk"] = np.ascontiguousarray(p["umk"][sl])
        im["mk"] = np.ascontiguousarray(p["mkc"][sl])
        in_maps.append(im)

    res = run_bass_kernel_spmd(nc, in_maps, core_ids=list(range(NCORES)),
                               trace=TRACE)
    LAST_RESULT = res

    Z = np.concatenate([r["zout"] for r in res.results], axis=0)
    mk = np.concatenate([r["mkout"] for r in res.results], axis=0)[..., None]
    return Z.reshape(B, L, LATENT), mk.reshape(B, L, 1)


# revision 20
# speedup vs baseline: 1.0793x; 1.0793x over previous
"""Trainium2 Bass kernel for the masked set-transformer encoder (ISAB stack).

Strategy (pure data parallel, B=16 over 8 cores, 2 batch elements/core):
  * The compaction permutation commutes with the whole network: softmax over
    keys is permutation invariant, everything else is row-wise, and masked
    rows are exactly zero throughout.  So the host only computes the stable
    argsort *indices*; the device processes tokens in compacted order and the
    output is already compacted (zero tail appended on device).
  * Only NT = ceil(max_b nvalid_b / 128) tiles of 128 tokens are processed
    (~42 instead of 82 for random masks).  NT is a compile-time constant
    derived from the actual mask; the program is recompiled if it changes.
  * The one-hot input FF collapses to a [43,128] table matmul against a
    device-built X^T = [one_hot(c); t*mk; u*mk] (no gathers: one-hot rows are
    built with an is_equal against an iota column; invalid tokens get an
    out-of-range channel id so their X^T column is exactly zero).
  * Attention layouts keep softmax reductions on natural axes:
      MAB0 scores  S^T[tok,(h,q)] = Z @ G0,  G0 = fold(Wk, I@Wq+bq)  (host)
      MAB0 key masking is folded into the exp bias: exp(s*scale + (mk-1)*30)
      MAB0 num/den via lhsT=E^T_h, rhs=[Vh_h | 1], DVE-accumulated over chunks
      MAB1 scores  S1^T[k, tok] per head via lhsT=G1_h, rhs=Z^T (4-chunk tiles)
      MAB1 num/den via lhsT=E1^T_h, rhs=[Vh1_h | 1]
  * Z^T lives in SBUF as [128, 512] group tiles so MAB1 score matmuls stream
    512 tokens per instruction.
  * ACT does exp only; biases are all zero in practice (trace-time fallbacks
    emit extra ones-row matmuls / bias adds when they are not).
"""

import math

import numpy as np

import concourse.bacc as bacc
import concourse.bass as bass
import concourse.mybir as mybir
import concourse.tile as tile
from concourse.bass_utils import run_bass_kernel_spmd
from concourse.masks import make_identity

F32 = mybir.dt.float32
AF = mybir.ActivationFunctionType
OP = mybir.AluOpType

B, S, D = 16, 256, 41
L = S * D                      # 10496
LATENT, NREF, NLAYERS, NHEADS = 128, 128, 3, 4
DH = LATENT // NHEADS          # 32
SCALE = 1.0 / math.sqrt(LATENT)
NCORES = 8
BPC = B // NCORES              # 2
CMAX = L // 128                # 82
NEGBIG = -30.0                 # exp(-30) ~ 1e-13: masked-key contribution
GW = 4                         # chunks per Z^T group tile

# set by test harness to capture profiling info
TRACE = False
LAST_RESULT = None

_PROG_CACHE: dict = {}


def _build_program(NT: int, flags: dict, nlayers: int = NLAYERS):
    NTP = NT * 128
    NG = -(-NT // GW)          # number of Z^T group tiles
    nc = bacc.Bacc("TRN2")

    def gslice(c):
        """(group index, column slice within the group tile) for chunk c."""
        return c // GW, slice((c % GW) * 128, (c % GW) * 128 + 128)

    # ---------------- DRAM I/O ----------------
    d_cidx = nc.dram_tensor("cidx", [BPC, NTP], F32, kind="ExternalInput")
    d_tmk = nc.dram_tensor("tmk", [BPC, NTP], F32, kind="ExternalInput")
    d_umk = nc.dram_tensor("umk", [BPC, NTP], F32, kind="ExternalInput")
    d_mk = nc.dram_tensor("mk", [BPC, NTP], F32, kind="ExternalInput")
    d_wtab = nc.dram_tensor("wtab", [43, LATENT], F32, kind="ExternalInput")
    d_g0 = nc.dram_tensor("g0", [NLAYERS, LATENT, 512], F32, kind="ExternalInput")
    d_qh0 = nc.dram_tensor("qh0", [NLAYERS, NREF, LATENT], F32, kind="ExternalInput")
    WNAMES = ["wv0", "wo0", "wq1", "wq1t", "wk1", "wv1", "wo1"]
    d_w = {
        n: nc.dram_tensor(n, [NLAYERS, LATENT, LATENT], F32, kind="ExternalInput")
        for n in WNAMES
    }
    BNAMES = ["r0", "bv0", "bo0", "bq1", "bk1", "bv1", "bo1"]
    d_b = {
        n: nc.dram_tensor(n, [NLAYERS, 512 if n == "r0" else LATENT], F32,
                          kind="ExternalInput")
        for n in BNAMES
    }
    d_zout = nc.dram_tensor("zout", [BPC, L, LATENT], F32, kind="ExternalOutput")
    d_mkout = nc.dram_tensor("mkout", [BPC, L], F32, kind="ExternalOutput")

    with tile.TileContext(nc) as tc:
        with (
            tc.tile_pool(name="persist", bufs=1) as pp,
            tc.tile_pool(name="work", bufs=2) as wp,
            tc.tile_pool(name="stream", bufs=4) as sp,
            tc.tile_pool(name="ps_sc", bufs=2, space="PSUM") as ps_sc,
            tc.tile_pool(name="ps_n1", bufs=2, space="PSUM") as ps_n1,
            tc.tile_pool(name="ps_sm", bufs=2, space="PSUM") as ps_sm,
        ):
            # ---------------- constants & weights ----------------
            ident = pp.tile([128, 128], F32, name="ident")
            make_identity(nc, ident[:])

            iota_i = pp.tile([41, 1], mybir.dt.int32, name="iota_i")
            nc.gpsimd.iota(iota_i[:], [[1, 1]], channel_multiplier=1)
            iota_f = pp.tile([41, 1], F32, name="iota_f")
            nc.vector.tensor_copy(iota_f[:], iota_i[:])

            ones_row = pp.tile([1, 128], F32, name="ones_row")
            nc.vector.memset(ones_row[:], 1.0)
            zerot = pp.tile([128, 512], F32, name="zerot")
            nc.vector.memset(zerot[:], 0.0)

            wtab_s = pp.tile([43, LATENT], F32, name="wtab_s")
            nc.sync.dma_start(wtab_s[:], d_wtab[:, :])

            g0_s, qh0_s = [], []
            w_s = {n: [] for n in WNAMES}
            b_s = {n: [] for n in BNAMES}
            for l in range(NLAYERS):
                g = pp.tile([LATENT, 512], F32, name=f"g0s{l}", tag=f"g0s{l}")
                nc.sync.dma_start(g[:], d_g0[l, :, :])
                g0_s.append(g)
                q = pp.tile([NREF, LATENT], F32, name=f"qh0s{l}", tag=f"qh0s{l}")
                nc.sync.dma_start(q[:], d_qh0[l, :, :])
                qh0_s.append(q)
                for n in WNAMES:
                    if n == "wq1t":
                        # per-head [32,128] tiles (PE weights must start at
                        # partition 0/32/64, so a [96:128] slice is illegal)
                        hh_tiles = []
                        for h in range(NHEADS):
                            t = pp.tile([DH, LATENT], F32, name=f"wq1t{l}h{h}",
                                        tag=f"wq1t{l}h{h}")
                            nc.sync.dma_start(
                                t[:], d_w[n][l, h * DH:(h + 1) * DH, :])
                            hh_tiles.append(t)
                        w_s[n].append(hh_tiles)
                        continue
                    t = pp.tile([LATENT, LATENT], F32, name=f"{n}s{l}", tag=f"{n}s{l}")
                    nc.sync.dma_start(t[:], d_w[n][l, :, :])
                    w_s[n].append(t)
                for n in BNAMES:
                    if not flags[n]:
                        b_s[n].append(None)
                        continue
                    if n in ("bk1",):        # needed as a [128,1] column
                        t = pp.tile([LATENT, 1], F32, name=f"{n}s{l}", tag=f"{n}s{l}")
                        nc.sync.dma_start(
                            t[:], bass.AP(d_b[n], l * LATENT, [[1, LATENT], [1, 1]]))
                    elif n == "bq1":         # per-head column tiles [32,1]
                        t = []
                        for h in range(NHEADS):
                            th = pp.tile([DH, 1], F32, name=f"{n}c{l}h{h}",
                                         tag=f"{n}c{l}h{h}")
                            nc.sync.dma_start(
                                th[:], bass.AP(d_b[n], l * LATENT + h * DH,
                                               [[1, DH], [1, 1]]))
                            t.append(th)
                    else:
                        w = 512 if n == "r0" else LATENT
                        t = pp.tile([1, w], F32, name=f"{n}s{l}", tag=f"{n}s{l}")
                        nc.sync.dma_start(t[:], d_b[n][l:l + 1, :])
                    b_s[n].append(t)
                if flags["bq1"]:  # row form for the ones-matmul into Qh1
                    t = pp.tile([1, LATENT], F32, name=f"bq1rs{l}", tag=f"bq1rs{l}")
                    nc.sync.dma_start(t[:], d_b["bq1"][l:l + 1, :])
                    b_s.setdefault("bq1r", []).append(t)

            # ---------------- per-batch setup + Z0 ----------------
            mkp_s, mkneg_s, mkt_s, ZT = [], [], [], []
            for b in range(BPC):
                mkt = pp.tile([NT, 128], F32, name=f"mkt{b}", tag=f"mkt{b}")
                nc.sync.dma_start(mkt[:], bass.AP(d_mk, b * NTP, [[128, NT], [1, 128]]))
                mkt_s.append(mkt)
                mkpp = ps_sm.tile([128, NT], F32, name="mkpp", tag="sm")
                nc.tensor.transpose(mkpp[:], mkt[:], ident[0:NT, 0:NT])
                mkp = pp.tile([128, NT], F32, name=f"mkp{b}", tag=f"mkp{b}")
                nc.vector.tensor_copy(mkp[:], mkpp[:])
                mkp_s.append(mkp)
                mkneg = pp.tile([128, NT], F32, name=f"mkneg{b}", tag=f"mkneg{b}")
                nc.vector.tensor_scalar(
                    mkneg[:], mkp[:], -1.0, -NEGBIG, op0=OP.add, op1=OP.mult)
                mkneg_s.append(mkneg)

                xt = pp.tile([43, NTP], F32, name=f"xt{b}", tag="xt")
                crow = pp.tile([1, NTP], F32, name=f"crow{b}", tag="crow")
                nc.sync.dma_start(crow[:], d_cidx[b:b + 1, :])
                # replicate cidx row across 41 partitions via a K=1 matmul,
                # then one-hot it against the iota column
                for j in range(0, NTP, 512):
                    w = min(512, NTP - j)
                    cb = ps_sm.tile([41, 512], F32, name="cb", tag="sm")
                    nc.tensor.matmul(cb[:, :w], lhsT=ones_row[:, 0:41],
                                     rhs=crow[:, j:j + w], start=True, stop=True)
                    nc.vector.tensor_scalar(
                        xt[0:41, j:j + w], cb[:, :w], iota_f[:], None,
                        op0=OP.is_equal)
                nc.sync.dma_start(xt[41:42, :], d_tmk[b:b + 1, :])
                nc.sync.dma_start(xt[42:43, :], d_umk[b:b + 1, :])

                ztg = []
                for g in range(NG):
                    w = min(GW * 128, NTP - g * GW * 128)
                    zt = pp.tile([128, GW * 128], F32, name=f"zt{b}_{g}",
                                 tag=f"zt{b}_{g}")
                    ztg.append(zt)
                for c in range(NT):
                    g, js = gslice(c)
                    z0p = ps_sm.tile([128, 128], F32, name="z0p", tag="sm")
                    nc.tensor.matmul(
                        z0p[:], lhsT=wtab_s[:], rhs=xt[:, c * 128:(c + 1) * 128],
                        start=True, stop=True)
                    nc.vector.tensor_scalar_max(ztg[g][:, js], z0p[:], 0.0)
                ZT.append(ztg)

            # ---------------- layers ----------------
            for l in range(nlayers):
                num0_b, rd0_b, o0_b, hh_b, ht_b = {}, {}, {}, {}, {}
                kh1_b, g1_b, vo1_b, r1b_b = {}, {}, {}, {}
                for b in range(BPC):
                    num0 = wp.tile([128, 132], F32, name="num0", tag=f"num0{b}")
                    nc.vector.memset(num0[:], 0.0)
                    num0_b[b] = num0
                # ===== MAB0: induced points attend to data =====
                for c in range(NT):
                    for b in range(BPC):
                        ztg = ZT[b]
                        num0 = num0_b[b]
                        g, js = gslice(c)
                        s0 = ps_sc.tile([128, 512], F32, name="s0", tag="sc")
                        if flags["r0"]:
                            nc.tensor.matmul(s0[:], lhsT=ones_row[:],
                                             rhs=b_s["r0"][l][:],
                                             start=True, stop=False)
                        nc.tensor.matmul(s0[:], lhsT=ztg[g][:, js], rhs=g0_s[l][:],
                                         start=not flags["r0"], stop=True)
                        et = sp.tile([128, 512], F32, name="et", tag="et")
                        nc.scalar.activation(et[:], s0[:], AF.Exp,
                                             bias=mkneg_s[b][:, c:c + 1],
                                             scale=SCALE)
                        vh = ps_sm.tile([128, 128], F32, name="vh", tag="sm")
                        if flags["bv0"]:
                            nc.tensor.matmul(vh[:], lhsT=ones_row[:],
                                             rhs=b_s["bv0"][l][:],
                                             start=True, stop=False)
                        nc.tensor.matmul(vh[:], lhsT=ztg[g][:, js],
                                         rhs=w_s["wv0"][l][:],
                                         start=not flags["bv0"], stop=True)
                        vo = sp.tile([128, 132], F32, name="vo", tag="vo")
                        nc.vector.memset(vo[:], 1.0)
                        nc.vector.tensor_copy(
                            vo[:].rearrange("p (h x) -> p h x", x=33)[:, :, 0:32],
                            vh[:].rearrange("p (h x) -> p h x", x=32))
                        n0c = ps_n1.tile([128, 132], F32, name="n0c", tag="n0c")
                        for h in range(NHEADS):
                            nc.tensor.matmul(
                                n0c[:, h * 33:(h + 1) * 33],
                                lhsT=et[:, h * 128:(h + 1) * 128],
                                rhs=vo[:, h * 33:(h + 1) * 33],
                                start=True, stop=True)
                        nc.vector.tensor_add(num0[:], num0[:], n0c[:])
                # ===== MAB0 tail + MAB1 prep (per batch) =====
                for b in range(BPC):
                    ztg = ZT[b]
                    num0 = num0_b[b]
                    rd0 = sp.tile([128, 4], F32, name="rd0", tag="rd0")
                    nc.vector.reciprocal(
                        rd0[:].rearrange("p (h x) -> p h x", x=1),
                        num0[:].rearrange("p (h x) -> p h x", x=33)[:, :, 32:33])
                    o0 = wp.tile([128, 128], F32, name="o0", tag=f"o0{b}")
                    nc.vector.tensor_tensor(
                        o0[:].rearrange("p (h x) -> p h x", x=32),
                        num0[:].rearrange("p (h x) -> p h x", x=33)[:, :, 0:32],
                        rd0[:].rearrange("p (h x) -> p h x", x=1).to_broadcast(
                            [128, 4, 32]),
                        op=OP.mult)
                    nc.vector.tensor_add(o0[:], o0[:], qh0_s[l][:])
                    o0tp = ps_sm.tile([128, 128], F32, name="o0tp", tag="sm")
                    nc.tensor.transpose(o0tp[:], o0[:], ident[:])
                    o0t = wp.tile([128, 128], F32, name="o0t", tag=f"o0t{b}")
                    nc.vector.tensor_copy(o0t[:], o0tp[:])
                    fc0 = ps_sm.tile([128, 128], F32, name="fc0", tag="sm")
                    if flags["bo0"]:
                        nc.tensor.matmul(fc0[:], lhsT=ones_row[:],
                                         rhs=b_s["bo0"][l][:], start=True, stop=False)
                    nc.tensor.matmul(fc0[:], lhsT=o0t[:], rhs=w_s["wo0"][l][:],
                                     start=not flags["bo0"], stop=True)
                    hh = wp.tile([128, 128], F32, name="hh", tag=f"hh{b}")
                    nc.vector.scalar_tensor_tensor(
                        hh[:], in0=fc0[:], scalar=0.0, in1=o0[:],
                        op0=OP.max, op1=OP.add)
                    htp = ps_sm.tile([128, 128], F32, name="htp", tag="sm")
                    nc.tensor.transpose(htp[:], hh[:], ident[:])
                    ht = wp.tile([128, 128], F32, name="ht", tag=f"ht{b}")
                    nc.vector.tensor_copy(ht[:], htp[:])

                    # ===== MAB1 prep =====
                    kh1p = ps_sm.tile([128, 128], F32, name="kh1p", tag="sm")
                    nc.tensor.matmul(kh1p[:], lhsT=w_s["wk1"][l][:], rhs=ht[:],
                                     start=True, stop=True)
                    kh1 = wp.tile([128, 128], F32, name="kh1", tag=f"kh1{b}")
                    if flags["bk1"]:
                        nc.vector.tensor_scalar_add(kh1[:], kh1p[:], b_s["bk1"][l][:])
                    else:
                        nc.vector.tensor_copy(kh1[:], kh1p[:])
                    kh1h = []
                    for h in range(NHEADS):
                        t = wp.tile([DH, LATENT], F32, name=f"kh1h{h}", tag=f"kh1h{h}")
                        nc.sync.dma_start(t[:], kh1[h * DH:(h + 1) * DH, :])
                        kh1h.append(t)
                    g1p = ps_sc.tile([128, 512], F32, name="g1p", tag="sc")
                    for h in range(NHEADS):
                        nc.tensor.matmul(g1p[:, h * 128:(h + 1) * 128],
                                         lhsT=w_s["wq1t"][l][h][:], rhs=kh1h[h][:],
                                         start=True, stop=True)
                    g1 = wp.tile([128, 512], F32, name="g1", tag=f"g1{b}")
                    nc.vector.tensor_copy(g1[:], g1p[:])
                    vh1p = ps_sm.tile([128, 128], F32, name="vh1p", tag="sm")
                    if flags["bv1"]:
                        nc.tensor.matmul(vh1p[:], lhsT=ones_row[:],
                                         rhs=b_s["bv1"][l][:], start=True, stop=False)
                    nc.tensor.matmul(vh1p[:], lhsT=ht[:], rhs=w_s["wv1"][l][:],
                                     start=not flags["bv1"], stop=True)
                    vo1 = wp.tile([128, 132], F32, name="vo1", tag=f"vo1{b}")
                    nc.vector.memset(vo1[:], 1.0)
                    nc.vector.tensor_copy(
                        vo1[:].rearrange("p (h x) -> p h x", x=33)[:, :, 0:32],
                        vh1p[:].rearrange("p (h x) -> p h x", x=32))
                    r1b = None
                    if flags["bq1"]:
                        r1bp = ps_sm.tile([128, 4], F32, name="r1bp", tag="sm")
                        for h in range(NHEADS):
                            nc.tensor.matmul(r1bp[:, h:h + 1], lhsT=kh1h[h][:],
                                             rhs=b_s["bq1"][l][h][:],
                                             start=True, stop=True)
                        r1b = wp.tile([128, 4], F32, name="r1b", tag=f"r1b{b}")
                        nc.vector.tensor_scalar_mul(r1b[:], r1bp[:], SCALE)
                    g1_b[b], vo1_b[b], r1b_b[b] = g1, vo1, r1b

                # ===== MAB1 chunks: data attends to induced (b-interleaved) ==
                for g in range(NG):
                    gw = min(GW * 128, NTP - g * GW * 128)
                    for b in range(BPC):
                        ztg = ZT[b]
                        g1, vo1, r1b = g1_b[b], vo1_b[b], r1b_b[b]
                        e1h = []
                        for h in range(NHEADS):
                            s1 = ps_sc.tile([128, GW * 128], F32, name="s1", tag="sc")
                            nc.tensor.matmul(s1[:, :gw],
                                             lhsT=g1[:, h * 128:(h + 1) * 128],
                                             rhs=ztg[g][:, :gw],
                                             start=True, stop=True)
                            e1 = sp.tile([128, GW * 128], F32, name="e1", tag="e1")
                            if flags["bq1"]:
                                nc.scalar.activation(e1[:, :gw], s1[:, :gw], AF.Exp,
                                                     bias=r1b[:, h:h + 1], scale=SCALE)
                            else:
                                nc.scalar.activation(e1[:, :gw], s1[:, :gw], AF.Exp,
                                                     scale=SCALE)
                            e1h.append(e1)
                        for j in range(gw // 128):
                            c = g * GW + j
                            js = slice(j * 128, (j + 1) * 128)
                            num1 = ps_n1.tile([128, 132], F32, name="num1", tag="num1")
                            for h in range(NHEADS):
                                nc.tensor.matmul(num1[:, h * 33:(h + 1) * 33],
                                                 lhsT=e1h[h][:, js],
                                                 rhs=vo1[:, h * 33:(h + 1) * 33],
                                                 start=True, stop=True)
                            qh1 = ps_sm.tile([128, 128], F32, name="qh1", tag="sm")
                            if flags["bq1"]:
                                nc.tensor.matmul(qh1[:], lhsT=ones_row[:],
                                                 rhs=b_s["bq1r"][l][:],
                                                 start=True, stop=False)
                            nc.tensor.matmul(qh1[:], lhsT=ztg[g][:, js],
                                             rhs=w_s["wq1"][l][:],
                                             start=not flags["bq1"], stop=True)
                            rd1 = sp.tile([128, 4], F32, name="rd1", tag="rd1")
                            nc.vector.reciprocal(
                                rd1[:].rearrange("p (h x) -> p h x", x=1),
                                num1[:].rearrange("p (h x) -> p h x", x=33)[:, :, 32:33])
                            o1 = sp.tile([128, 128], F32, name="o1", tag="o1")
                            nc.vector.tensor_tensor(
                                o1[:].rearrange("p (h x) -> p h x", x=32),
                                num1[:].rearrange("p (h x) -> p h x", x=33)[:, :, 0:32],
                                rd1[:].rearrange("p (h x) -> p h x", x=1).to_broadcast(
                                    [128, 4, 32]),
                                op=OP.mult)
                            nc.vector.tensor_add(o1[:], o1[:], qh1[:])
                            o1tp = ps_sm.tile([128, 128], F32, name="o1tp", tag="sm")
                            nc.tensor.transpose(o1tp[:], o1[:], ident[:])
                            o1t = sp.tile([128, 128], F32, name="o1t", tag="o1t")
                            nc.vector.tensor_copy(o1t[:], o1tp[:])
                            fc1 = ps_sm.tile([128, 128], F32, name="fc1", tag="sm")
                            if flags["bo1"]:
                                nc.tensor.matmul(fc1[:], lhsT=ones_row[:],
                                                 rhs=b_s["bo1"][l][:],
                                                 start=True, stop=False)
                            nc.tensor.matmul(fc1[:], lhsT=o1t[:], rhs=w_s["wo1"][l][:],
                                             start=not flags["bo1"], stop=True)
                            u = sp.tile([128, 128], F32, name="u", tag="u")
                            nc.vector.scalar_tensor_tensor(
                                u[:], in0=fc1[:], scalar=0.0, in1=o1[:],
                                op0=OP.max, op1=OP.add)
                            nc.vector.tensor_scalar_mul(u[:], u[:],
                                                        mkp_s[b][:, c:c + 1])
                            utp = ps_sm.tile([128, 128], F32, name="utp", tag="sm")
                            nc.tensor.transpose(utp[:], u[:], ident[:])
                            nc.vector.tensor_add(ztg[g][:, js], ztg[g][:, js], utp[:])

            # ---------------- output ----------------
            for c in range(NT):
                for b in range(BPC):
                    g, js = gslice(c)
                    zp = ps_sm.tile([128, 128], F32, name="zp", tag="sm")
                    nc.tensor.transpose(zp[:], ZT[b][g][:, js], ident[:])
                    zo = sp.tile([128, 128], F32, name="zo", tag="zo")
                    nc.vector.tensor_copy(zo[:], zp[:])
                    nc.sync.dma_start(
                        bass.AP(d_zout, (b * L + c * 128) * LATENT,
                                [[LATENT, 128], [1, LATENT]]),
                        zo[:])
            for b in range(BPC):
                # zero tail rows [NTP, L): 2KB-contiguous descriptor runs
                r = NTP
                while r < L:
                    n = min(512, L - r)   # rows; n*128 elems; dst stays contiguous
                    nelem = n * LATENT
                    inner = nelem // 128
                    nc.sync.dma_start(
                        bass.AP(d_zout, (b * L + r) * LATENT,
                                [[inner, 128], [1, inner]]),
                        zerot[:, :inner])
                    r += n
                # mask output (contiguous 512B rows from the token-major tile)
                nc.sync.dma_start(
                    bass.AP(d_mkout, b * L, [[128, NT], [1, 128]]), mkt_s[b][:])
                if CMAX > NT:
                    nc.sync.dma_start(
                        bass.AP(d_mkout, b * L + NTP, [[128, CMAX - NT], [1, 128]]),
                        zerot[0:CMAX - NT, 0:128])
    nc.compile()
    return nc


def _prep(inputs):
    """Host-side prep: compaction indices + weight folding (all O(small))."""
    time_x = np.ascontiguousarray(np.asarray(inputs["time_x"], np.float32))
    value_x = np.ascontiguousarray(np.asarray(inputs["value_x"], np.float32))
    mask_x = np.asarray(inputs["mask_x"])
    Wi = np.asarray(inputs["Wi"], np.float32)
    bi = np.asarray(inputs["bi"], np.float32)
    I = np.asarray(inputs["I"], np.float32)
    Wq = np.asarray(inputs["Wq"], np.float32)
    bq = np.asarray(inputs["bq"], np.float32)
    Wk = np.asarray(inputs["Wk"], np.float32)
    bk = np.asarray(inputs["bk"], np.float32)
    Wv = np.asarray(inputs["Wv"], np.float32)
    bv = np.asarray(inputs["bv"], np.float32)
    Wo = np.asarray(inputs["Wo"], np.float32)
    bo = np.asarray(inputs["bo"], np.float32)

    mflat = mask_x.reshape(B, L)
    order = np.argsort(1 - mflat, axis=1, kind="stable")
    nvalid = int(mflat.sum(axis=1).max())
    NT = max(1, min(CMAX, -(-nvalid // 128)))
    NTP = NT * 128

    ordp = order[:, :NTP]
    mkc = np.take_along_axis(mflat, ordp, axis=1).astype(np.float32)
    tfull = np.broadcast_to(time_x[:, :, None], (B, S, D)).reshape(B, L)
    tmk = np.take_along_axis(tfull, ordp, axis=1) * mkc
    umk = np.take_along_axis(value_x.reshape(B, L), ordp, axis=1) * mkc
    cidx = np.where(mkc > 0, (ordp % D).astype(np.float32), 63.0).astype(np.float32)

    wtab = np.concatenate([Wi[:D] + bi[None, :], Wi[D:D + 2]], 0)  # [43,128]

    g0 = np.zeros((NLAYERS, LATENT, 512), np.float32)
    qh0 = np.zeros((NLAYERS, NREF, LATENT), np.float32)
    r0 = np.zeros((NLAYERS, 512), np.float32)
    for l in range(NLAYERS):
        Q = I[l] @ Wq[l, 0] + bq[l, 0]
        qh0[l] = Q
        for h in range(NHEADS):
            hs = slice(h * DH, (h + 1) * DH)
            g0[l][:, h * NREF:(h + 1) * NREF] = Wk[l, 0][:, hs] @ Q[:, hs].T
            r0[l][h * NREF:(h + 1) * NREF] = bk[l, 0][hs] @ Q[:, hs].T

    w = dict(
        wv0=Wv[:, 0], wo0=Wo[:, 0], wq1=Wq[:, 1],
        wq1t=np.ascontiguousarray(Wq[:, 1].transpose(0, 2, 1)),
        wk1=Wk[:, 1], wv1=Wv[:, 1], wo1=Wo[:, 1],
    )
    bvec = dict(r0=r0, bv0=bv[:, 0], bo0=bo[:, 0], bq1=bq[:, 1],
                bk1=bk[:, 1], bv1=bv[:, 1], bo1=bo[:, 1])
    flags = {n: bool(np.any(v != 0)) for n, v in bvec.items()}
    return dict(NT=NT, NTP=NTP, tmk=tmk, umk=umk, mkc=mkc, cidx=cidx,
                wtab=wtab, g0=g0, qh0=qh0, w=w, bvec=bvec, flags=flags)


def kernel(**inputs):
    global LAST_RESULT
    p = _prep(inputs)

    key = (p["NT"], tuple(sorted(p["flags"].items())))
    if key not in _PROG_CACHE:
        _PROG_CACHE[key] = _build_program(p["NT"], p["flags"])
    nc = _PROG_CACHE[key]

    shared = dict(wtab=np.ascontiguousarray(p["wtab"]),
                  g0=p["g0"], qh0=p["qh0"])
    for n, v in p["w"].items():
        shared[n] = np.ascontiguousarray(v)
    for n, v in p["bvec"].items():
        shared[n] = np.ascontiguousarray(v)

    in_maps = []
    for m in range(NCORES):
        sl = slice(m * BPC, (m + 1) * BPC)
        im = dict(shared)
        im["cidx"] = np.ascontiguousarray(p["cidx"][sl])
        im["tmk"] = np.ascontiguousarray(p["tmk"][sl])
        im["umk"] = np.ascontiguousarray(p["umk"][sl])
        im["mk"] = np.ascontiguousarray(p["mkc"][sl])
        in_maps.append(im)

    res = run_bass_kernel_spmd(nc, in_maps, core_ids=list(range(NCORES)),
                               trace=TRACE)
    LAST_RESULT = res

    Z = np.concatenate([r["zout"] for r in res.results], axis=0)
    mk = np.concatenate([r["mkout"] for r in res.results], axis=0)[..., None]
    return Z.reshape(B, L, LATENT), mk.reshape(B, L, 1)


# revision 22
# speedup vs baseline: 1.7069x; 1.5815x over previous
"""Trainium2 Bass kernel for the masked set-transformer encoder (ISAB stack).

Strategy (pure data parallel, B=16 over 8 cores, 2 batch elements/core):
  * The compaction permutation commutes with the whole network: softmax over
    keys is permutation invariant, everything else is row-wise, and masked
    rows are exactly zero throughout.  So the host only computes the stable
    argsort *indices*; the device processes tokens in compacted order and the
    output is already compacted (zero tail appended on device).
  * Only NT = ceil(max_b nvalid_b / 128) tiles of 128 tokens are processed
    (~42 instead of 82 for random masks).  NT is a compile-time constant
    derived from the actual mask; the program is recompiled if it changes.
  * The one-hot input FF collapses to a [43,128] table matmul against a
    device-built X^T = [one_hot(c); t*mk; u*mk] (no gathers: one-hot rows are
    built with an is_equal against an iota column; invalid tokens get an
    out-of-range channel id so their X^T column is exactly zero).
  * Attention layouts keep softmax reductions on natural axes:
      MAB0 scores  S^T[tok,(h,q)] = Z @ G0,  G0 = fold(Wk, I@Wq+bq)  (host)
      MAB0 key masking is folded into the exp bias: exp(s*scale + (mk-1)*30)
      MAB0 num/den via lhsT=E^T_h, rhs=[Vh_h | 1], DVE-accumulated over chunks
      MAB1 scores  S1^T[k, tok] per head via lhsT=G1_h, rhs=Z^T (4-chunk tiles)
      MAB1 num/den via lhsT=E1^T_h, rhs=[Vh1_h | 1]
  * Z^T lives in SBUF as [128, 512] group tiles so MAB1 score matmuls stream
    512 tokens per instruction.
  * ACT does exp only; biases are all zero in practice (trace-time fallbacks
    emit extra ones-row matmuls / bias adds when they are not).
"""

import math

import numpy as np

import concourse.bacc as bacc
import concourse.bass as bass
import concourse.mybir as mybir
import concourse.tile as tile
from concourse.bass_utils import run_bass_kernel_spmd
from concourse.masks import make_identity

F32 = mybir.dt.float32
BF16 = mybir.dt.bfloat16
AF = mybir.ActivationFunctionType
OP = mybir.AluOpType

B, S, D = 16, 256, 41
L = S * D                      # 10496
LATENT, NREF, NLAYERS, NHEADS = 128, 128, 3, 4
DH = LATENT // NHEADS          # 32
SCALE = 1.0 / math.sqrt(LATENT)
NCORES = 8
BPC = B // NCORES              # 2
CMAX = L // 128                # 82
NEGBIG = -30.0                 # exp(-30) ~ 1e-13: masked-key contribution
GW = 4                         # chunks per Z^T group tile

# set by test harness to capture profiling info
TRACE = False
LAST_RESULT = None

_PROG_CACHE: dict = {}


def _build_program(NT: int, flags: dict, nlayers: int = NLAYERS):
    NTP = NT * 128
    NG = -(-NT // GW)          # number of Z^T group tiles
    nc = bacc.Bacc("TRN2")

    def gslice(c):
        """(group index, column slice within the group tile) for chunk c."""
        return c // GW, slice((c % GW) * 128, (c % GW) * 128 + 128)

    # ---------------- DRAM I/O ----------------
    d_cidx = nc.dram_tensor("cidx", [BPC, NTP], F32, kind="ExternalInput")
    d_tmk = nc.dram_tensor("tmk", [BPC, NTP], F32, kind="ExternalInput")
    d_umk = nc.dram_tensor("umk", [BPC, NTP], F32, kind="ExternalInput")
    d_mk = nc.dram_tensor("mk", [BPC, NTP], F32, kind="ExternalInput")
    d_wtab = nc.dram_tensor("wtab", [43, LATENT], F32, kind="ExternalInput")
    d_g0 = nc.dram_tensor("g0", [NLAYERS, LATENT, 512], BF16, kind="ExternalInput")
    d_qh0 = nc.dram_tensor("qh0", [NLAYERS, NREF, LATENT], F32, kind="ExternalInput")
    WNAMES = ["wv0", "wo0", "wq1", "wq1t", "wk1", "wv1", "wo1"]
    d_w = {
        n: nc.dram_tensor(n, [NLAYERS, LATENT, LATENT], BF16, kind="ExternalInput")
        for n in WNAMES
    }
    BNAMES = ["r0", "bv0", "bo0", "bq1", "bk1", "bv1", "bo1"]
    d_b = {
        n: nc.dram_tensor(n, [NLAYERS, 512 if n == "r0" else LATENT], F32,
                          kind="ExternalInput")
        for n in BNAMES
    }
    d_zout = nc.dram_tensor("zout", [BPC, L, LATENT], F32, kind="ExternalOutput")
    d_mkout = nc.dram_tensor("mkout", [BPC, L], F32, kind="ExternalOutput")

    with tile.TileContext(nc) as tc:
        with (
            tc.tile_pool(name="persist", bufs=1) as pp,
            tc.tile_pool(name="work", bufs=2) as wp,
            tc.tile_pool(name="stream", bufs=4) as sp,
            tc.tile_pool(name="ps_sc", bufs=2, space="PSUM") as ps_sc,
            tc.tile_pool(name="ps_n1", bufs=2, space="PSUM") as ps_n1,
            tc.tile_pool(name="ps_sm", bufs=2, space="PSUM") as ps_sm,
        ):
            # ---------------- constants & weights ----------------
            ident = pp.tile([128, 128], F32, name="ident")
            make_identity(nc, ident[:])

            iota_i = pp.tile([41, 1], mybir.dt.int32, name="iota_i")
            nc.gpsimd.iota(iota_i[:], [[1, 1]], channel_multiplier=1)
            iota_f = pp.tile([41, 1], F32, name="iota_f")
            nc.vector.tensor_copy(iota_f[:], iota_i[:])

            ones_row = pp.tile([1, 128], F32, name="ones_row")
            nc.vector.memset(ones_row[:], 1.0)
            zerot = pp.tile([128, 512], F32, name="zerot")
            nc.vector.memset(zerot[:], 0.0)

            wtab_s = pp.tile([43, LATENT], F32, name="wtab_s")
            nc.sync.dma_start(wtab_s[:], d_wtab[:, :])

            g0_s, qh0_s = [], []
            w_s = {n: [] for n in WNAMES}
            b_s = {n: [] for n in BNAMES}
            for l in range(NLAYERS):
                g = pp.tile([LATENT, 512], BF16, name=f"g0s{l}", tag=f"g0s{l}")
                nc.sync.dma_start(g[:], d_g0[l, :, :])
                g0_s.append(g)
                q = pp.tile([NREF, LATENT], F32, name=f"qh0s{l}", tag=f"qh0s{l}")
                nc.sync.dma_start(q[:], d_qh0[l, :, :])
                qh0_s.append(q)
                for n in WNAMES:
                    if n == "wq1t":
                        # per-head [32,128] tiles (PE weights must start at
                        # partition 0/32/64, so a [96:128] slice is illegal)
                        hh_tiles = []
                        for h in range(NHEADS):
                            t = pp.tile([DH, LATENT], BF16, name=f"wq1t{l}h{h}",
                                        tag=f"wq1t{l}h{h}")
                            nc.sync.dma_start(
                                t[:], d_w[n][l, h * DH:(h + 1) * DH, :])
                            hh_tiles.append(t)
                        w_s[n].append(hh_tiles)
                        continue
                    t = pp.tile([LATENT, LATENT], BF16, name=f"{n}s{l}", tag=f"{n}s{l}")
                    nc.sync.dma_start(t[:], d_w[n][l, :, :])
                    w_s[n].append(t)
                for n in BNAMES:
                    if not flags[n]:
                        b_s[n].append(None)
                        continue
                    if n in ("bk1",):        # needed as a [128,1] column
                        t = pp.tile([LATENT, 1], F32, name=f"{n}s{l}", tag=f"{n}s{l}")
                        nc.sync.dma_start(
                            t[:], bass.AP(d_b[n], l * LATENT, [[1, LATENT], [1, 1]]))
                    elif n == "bq1":         # per-head column tiles [32,1]
                        t = []
                        for h in range(NHEADS):
                            th = pp.tile([DH, 1], F32, name=f"{n}c{l}h{h}",
                                         tag=f"{n}c{l}h{h}")
                            nc.sync.dma_start(
                                th[:], bass.AP(d_b[n], l * LATENT + h * DH,
                                               [[1, DH], [1, 1]]))
                            t.append(th)
                    else:
                        w = 512 if n == "r0" else LATENT
                        t = pp.tile([1, w], F32, name=f"{n}s{l}", tag=f"{n}s{l}")
                        nc.sync.dma_start(t[:], d_b[n][l:l + 1, :])
                    b_s[n].append(t)
                if flags["bq1"]:  # row form for the ones-matmul into Qh1
                    t = pp.tile([1, LATENT], F32, name=f"bq1rs{l}", tag=f"bq1rs{l}")
                    nc.sync.dma_start(t[:], d_b["bq1"][l:l + 1, :])
                    b_s.setdefault("bq1r", []).append(t)

            # ---------------- per-batch setup + Z0 ----------------
            mkp_s, mkneg_s, mkt_s, ZT, ZTB = [], [], [], [], []
            for b in range(BPC):
                mkt = pp.tile([NT, 128], F32, name=f"mkt{b}", tag=f"mkt{b}")
                nc.sync.dma_start(mkt[:], bass.AP(d_mk, b * NTP, [[128, NT], [1, 128]]))
                mkt_s.append(mkt)
                mkpp = ps_sm.tile([128, NT], F32, name="mkpp", tag="sm")
                nc.tensor.transpose(mkpp[:], mkt[:], ident[0:NT, 0:NT])
                mkp = pp.tile([128, NT], F32, name=f"mkp{b}", tag=f"mkp{b}")
                nc.vector.tensor_copy(mkp[:], mkpp[:])
                mkp_s.append(mkp)
                mkneg = pp.tile([128, NT], F32, name=f"mkneg{b}", tag=f"mkneg{b}")
                nc.vector.tensor_scalar(
                    mkneg[:], mkp[:], -1.0, -NEGBIG, op0=OP.add, op1=OP.mult)
                mkneg_s.append(mkneg)

                xt = pp.tile([43, NTP], F32, name=f"xt{b}", tag="xt")
                crow = pp.tile([1, NTP], F32, name=f"crow{b}", tag="crow")
                nc.sync.dma_start(crow[:], d_cidx[b:b + 1, :])
                # replicate cidx row across 41 partitions via a K=1 matmul,
                # then one-hot it against the iota column
                for j in range(0, NTP, 512):
                    w = min(512, NTP - j)
                    cb = ps_sm.tile([41, 512], F32, name="cb", tag="sm")
                    nc.tensor.matmul(cb[:, :w], lhsT=ones_row[:, 0:41],
                                     rhs=crow[:, j:j + w], start=True, stop=True)
                    nc.vector.tensor_scalar(
                        xt[0:41, j:j + w], cb[:, :w], iota_f[:], None,
                        op0=OP.is_equal)
                nc.sync.dma_start(xt[41:42, :], d_tmk[b:b + 1, :])
                nc.sync.dma_start(xt[42:43, :], d_umk[b:b + 1, :])

                ztg, ztbg = [], []
                for g in range(NG):
                    zt = pp.tile([128, GW * 128], F32, name=f"zt{b}_{g}",
                                 tag=f"zt{b}_{g}")
                    ztg.append(zt)
                    ztb = pp.tile([128, GW * 128], BF16, name=f"ztb{b}_{g}",
                                  tag=f"ztb{b}_{g}")
                    ztbg.append(ztb)
                for c in range(NT):
                    g, js = gslice(c)
                    z0p = ps_sm.tile([128, 128], F32, name="z0p", tag="sm")
                    nc.tensor.matmul(
                        z0p[:], lhsT=wtab_s[:], rhs=xt[:, c * 128:(c + 1) * 128],
                        start=True, stop=True)
                    nc.vector.tensor_scalar_max(ztg[g][:, js], z0p[:], 0.0)
                    nc.vector.tensor_copy(ztbg[g][:, js], ztg[g][:, js])
                ZT.append(ztg)
                ZTB.append(ztbg)

            # ---------------- layers ----------------
            for l in range(nlayers):
                num0_b, rd0_b, o0_b, hh_b, ht_b = {}, {}, {}, {}, {}
                kh1_b, g1_b, vo1_b, r1b_b = {}, {}, {}, {}
                for b in range(BPC):
                    num0 = wp.tile([128, 132], F32, name="num0", tag=f"num0{b}")
                    nc.vector.memset(num0[:], 0.0)
                    num0_b[b] = num0
                # ===== MAB0: induced points attend to data =====
                for c in range(NT):
                    for b in range(BPC):
                        ztg, ztbg = ZT[b], ZTB[b]
                        num0 = num0_b[b]
                        g, js = gslice(c)
                        s0 = ps_sc.tile([128, 512], F32, name="s0", tag="sc")
                        if flags["r0"]:
                            nc.tensor.matmul(s0[:], lhsT=ones_row[:],
                                             rhs=b_s["r0"][l][:],
                                             start=True, stop=False)
                        nc.tensor.matmul(s0[:], lhsT=ztbg[g][:, js], rhs=g0_s[l][:],
                                         start=not flags["r0"], stop=True)
                        et = sp.tile([128, 512], BF16, name="et", tag="et")
                        nc.scalar.activation(et[:], s0[:], AF.Exp,
                                             bias=mkneg_s[b][:, c:c + 1],
                                             scale=SCALE)
                        vh = ps_sm.tile([128, 128], F32, name="vh", tag="sm")
                        if flags["bv0"]:
                            nc.tensor.matmul(vh[:], lhsT=ones_row[:],
                                             rhs=b_s["bv0"][l][:],
                                             start=True, stop=False)
                        nc.tensor.matmul(vh[:], lhsT=ztbg[g][:, js],
                                         rhs=w_s["wv0"][l][:],
                                         start=not flags["bv0"], stop=True)
                        vo = sp.tile([128, 132], BF16, name="vo", tag="vo")
                        nc.vector.memset(vo[:], 1.0)
                        nc.vector.tensor_copy(
                            vo[:].rearrange("p (h x) -> p h x", x=33)[:, :, 0:32],
                            vh[:].rearrange("p (h x) -> p h x", x=32))
                        n0c = ps_n1.tile([128, 132], F32, name="n0c", tag="n0c")
                        for h in range(NHEADS):
                            nc.tensor.matmul(
                                n0c[:, h * 33:(h + 1) * 33],
                                lhsT=et[:, h * 128:(h + 1) * 128],
                                rhs=vo[:, h * 33:(h + 1) * 33],
                                start=True, stop=True)
                        nc.vector.tensor_add(num0[:], num0[:], n0c[:])
                # ===== MAB0 tail + MAB1 prep (per batch) =====
                for b in range(BPC):
                    ztg = ZT[b]
                    num0 = num0_b[b]
                    rd0 = sp.tile([128, 4], F32, name="rd0", tag="rd0")
                    nc.vector.reciprocal(
                        rd0[:].rearrange("p (h x) -> p h x", x=1),
                        num0[:].rearrange("p (h x) -> p h x", x=33)[:, :, 32:33])
                    o0 = wp.tile([128, 128], F32, name="o0", tag=f"o0{b}")
                    nc.vector.tensor_tensor(
                        o0[:].rearrange("p (h x) -> p h x", x=32),
                        num0[:].rearrange("p (h x) -> p h x", x=33)[:, :, 0:32],
                        rd0[:].rearrange("p (h x) -> p h x", x=1).to_broadcast(
                            [128, 4, 32]),
                        op=OP.mult)
                    nc.vector.tensor_add(o0[:], o0[:], qh0_s[l][:])
                    o0tp = ps_sm.tile([128, 128], F32, name="o0tp", tag="sm")
                    nc.tensor.transpose(o0tp[:], o0[:], ident[:])
                    o0t = wp.tile([128, 128], BF16, name="o0t", tag=f"o0t{b}")
                    nc.vector.tensor_copy(o0t[:], o0tp[:])
                    fc0 = ps_sm.tile([128, 128], F32, name="fc0", tag="sm")
                    if flags["bo0"]:
                        nc.tensor.matmul(fc0[:], lhsT=ones_row[:],
                                         rhs=b_s["bo0"][l][:], start=True, stop=False)
                    nc.tensor.matmul(fc0[:], lhsT=o0t[:], rhs=w_s["wo0"][l][:],
                                     start=not flags["bo0"], stop=True)
                    hh = wp.tile([128, 128], F32, name="hh", tag=f"hh{b}")
                    nc.vector.scalar_tensor_tensor(
                        hh[:], in0=fc0[:], scalar=0.0, in1=o0[:],
                        op0=OP.max, op1=OP.add)
                    htp = ps_sm.tile([128, 128], F32, name="htp", tag="sm")
                    nc.tensor.transpose(htp[:], hh[:], ident[:])
                    ht = wp.tile([128, 128], BF16, name="ht", tag=f"ht{b}")
                    nc.vector.tensor_copy(ht[:], htp[:])

                    # ===== MAB1 prep =====
                    kh1p = ps_sm.tile([128, 128], F32, name="kh1p", tag="sm")
                    nc.tensor.matmul(kh1p[:], lhsT=w_s["wk1"][l][:], rhs=ht[:],
                                     start=True, stop=True)
                    kh1 = wp.tile([128, 128], BF16, name="kh1", tag=f"kh1{b}")
                    if flags["bk1"]:
                        nc.vector.tensor_scalar_add(kh1[:], kh1p[:], b_s["bk1"][l][:])
                    else:
                        nc.vector.tensor_copy(kh1[:], kh1p[:])
                    kh1h = []
                    for h in range(NHEADS):
                        t = wp.tile([DH, LATENT], BF16, name=f"kh1h{h}", tag=f"kh1h{h}")
                        nc.sync.dma_start(t[:], kh1[h * DH:(h + 1) * DH, :])
                        kh1h.append(t)
                    g1p = ps_sc.tile([128, 512], F32, name="g1p", tag="sc")
                    for h in range(NHEADS):
                        nc.tensor.matmul(g1p[:, h * 128:(h + 1) * 128],
                                         lhsT=w_s["wq1t"][l][h][:], rhs=kh1h[h][:],
                                         start=True, stop=True)
                    g1 = wp.tile([128, 512], BF16, name="g1", tag=f"g1{b}")
                    nc.vector.tensor_copy(g1[:], g1p[:])
                    vh1p = ps_sm.tile([128, 128], F32, name="vh1p", tag="sm")
                    if flags["bv1"]:
                        nc.tensor.matmul(vh1p[:], lhsT=ones_row[:],
                                         rhs=b_s["bv1"][l][:], start=True, stop=False)
                    nc.tensor.matmul(vh1p[:], lhsT=ht[:], rhs=w_s["wv1"][l][:],
                                     start=not flags["bv1"], stop=True)
                    vo1 = wp.tile([128, 132], BF16, name="vo1", tag=f"vo1{b}")
                    nc.vector.memset(vo1[:], 1.0)
                    nc.vector.tensor_copy(
                        vo1[:].rearrange("p (h x) -> p h x", x=33)[:, :, 0:32],
                        vh1p[:].rearrange("p (h x) -> p h x", x=32))
                    r1b = None
                    if flags["bq1"]:
                        r1bp = ps_sm.tile([128, 4], F32, name="r1bp", tag="sm")
                        for h in range(NHEADS):
                            nc.tensor.matmul(r1bp[:, h:h + 1], lhsT=kh1h[h][:],
                                             rhs=b_s["bq1"][l][h][:],
                                             start=True, stop=True)
                        r1b = wp.tile([128, 4], F32, name="r1b", tag=f"r1b{b}")
                        nc.vector.tensor_scalar_mul(r1b[:], r1bp[:], SCALE)
                    g1_b[b], vo1_b[b], r1b_b[b] = g1, vo1, r1b

                # ===== MAB1 chunks: data attends to induced (b-interleaved) ==
                for g in range(NG):
                    gw = min(GW * 128, NTP - g * GW * 128)
                    for b in range(BPC):
                        ztg, ztbg = ZT[b], ZTB[b]
                        g1, vo1, r1b = g1_b[b], vo1_b[b], r1b_b[b]
                        e1h = []
                        for h in range(NHEADS):
                            s1 = ps_sc.tile([128, GW * 128], F32, name="s1", tag="sc")
                            nc.tensor.matmul(s1[:, :gw],
                                             lhsT=g1[:, h * 128:(h + 1) * 128],
                                             rhs=ztbg[g][:, :gw],
                                             start=True, stop=True)
                            e1 = sp.tile([128, GW * 128], BF16, name="e1", tag="e1")
                            if flags["bq1"]:
                                nc.scalar.activation(e1[:, :gw], s1[:, :gw], AF.Exp,
                                                     bias=r1b[:, h:h + 1], scale=SCALE)
                            else:
                                nc.scalar.activation(e1[:, :gw], s1[:, :gw], AF.Exp,
                                                     scale=SCALE)
                            e1h.append(e1)
                        for j in range(gw // 128):
                            c = g * GW + j
                            js = slice(j * 128, (j + 1) * 128)
                            num1 = ps_n1.tile([128, 132], F32, name="num1", tag="num1")
                            for h in range(NHEADS):
                                nc.tensor.matmul(num1[:, h * 33:(h + 1) * 33],
                                                 lhsT=e1h[h][:, js],
                                                 rhs=vo1[:, h * 33:(h + 1) * 33],
                                                 start=True, stop=True)
                            qh1 = ps_sm.tile([128, 128], F32, name="qh1", tag="sm")
                            if flags["bq1"]:
                                nc.tensor.matmul(qh1[:], lhsT=ones_row[:],
                                                 rhs=b_s["bq1r"][l][:],
                                                 start=True, stop=False)
                            nc.tensor.matmul(qh1[:], lhsT=ztbg[g][:, js],
                                             rhs=w_s["wq1"][l][:],
                                             start=not flags["bq1"], stop=True)
                            rd1 = sp.tile([128, 4], F32, name="rd1", tag="rd1")
                            nc.vector.reciprocal(
                                rd1[:].rearrange("p (h x) -> p h x", x=1),
                                num1[:].rearrange("p (h x) -> p h x", x=33)[:, :, 32:33])
                            o1 = sp.tile([128, 128], F32, name="o1", tag="o1")
                            nc.vector.tensor_tensor(
                                o1[:].rearrange("p (h x) -> p h x", x=32),
                                num1[:].rearrange("p (h x) -> p h x", x=33)[:, :, 0:32],
                                rd1[:].rearrange("p (h x) -> p h x", x=1).to_broadcast(
                                    [128, 4, 32]),
                                op=OP.mult)
                            nc.vector.tensor_add(o1[:], o1[:], qh1[:])
                            o1tp = ps_sm.tile([128, 128], F32, name="o1tp", tag="sm")
                            nc.tensor.transpose(o1tp[:], o1[:], ident[:])
                            o1t = sp.tile([128, 128], BF16, name="o1t", tag="o1t")
                            nc.vector.tensor_copy(o1t[:], o1tp[:])
                            fc1 = ps_sm.tile([128, 128], F32, name="fc1", tag="sm")
                            if flags["bo1"]:
                                nc.tensor.matmul(fc1[:], lhsT=ones_row[:],
                                                 rhs=b_s["bo1"][l][:],
                                                 start=True, stop=False)
                            nc.tensor.matmul(fc1[:], lhsT=o1t[:], rhs=w_s["wo1"][l][:],
                                             start=not flags["bo1"], stop=True)
                            u = sp.tile([128, 128], F32, name="u", tag="u")
                            nc.vector.scalar_tensor_tensor(
                                u[:], in0=fc1[:], scalar=0.0, in1=o1[:],
                                op0=OP.max, op1=OP.add)
                            nc.vector.tensor_scalar_mul(u[:], u[:],
                                                        mkp_s[b][:, c:c + 1])
                            utp = ps_sm.tile([128, 128], F32, name="utp", tag="sm")
                            nc.tensor.transpose(utp[:], u[:], ident[:])
                            nc.vector.tensor_add(ztg[g][:, js], ztg[g][:, js], utp[:])
                            nc.vector.tensor_copy(ztbg[g][:, js], ztg[g][:, js])

            # ---------------- output ----------------
            for c in range(NT):
                for b in range(BPC):
                    g, js = gslice(c)
                    zp = ps_sm.tile([128, 128], F32, name="zp", tag="sm")
                    nc.tensor.transpose(zp[:], ZT[b][g][:, js], ident[:])
                    zo = sp.tile([128, 128], F32, name="zo", tag="zo")
                    nc.vector.tensor_copy(zo[:], zp[:])
                    nc.sync.dma_start(
                        bass.AP(d_zout, (b * L + c * 128) * LATENT,
                                [[LATENT, 128], [1, LATENT]]),
                        zo[:])
            for b in range(BPC):
                # zero tail rows [NTP, L): 2KB-contiguous descriptor runs
                r = NTP
                while r < L:
                    n = min(512, L - r)   # rows; n*128 elems; dst stays contiguous
                    nelem = n * LATENT
                    inner = nelem // 128
                    nc.sync.dma_start(
                        bass.AP(d_zout, (b * L + r) * LATENT,
                                [[inner, 128], [1, inner]]),
                        zerot[:, :inner])
                    r += n
                # mask output (contiguous 512B rows from the token-major tile)
                nc.sync.dma_start(
                    bass.AP(d_mkout, b * L, [[128, NT], [1, 128]]), mkt_s[b][:])
                if CMAX > NT:
                    nc.sync.dma_start(
                        bass.AP(d_mkout, b * L + NTP, [[128, CMAX - NT], [1, 128]]),
                        zerot[0:CMAX - NT, 0:128])
    nc.compile()
    return nc


def _prep(inputs):
    """Host-side prep: compaction indices + weight folding (all O(small))."""
    time_x = np.ascontiguousarray(np.asarray(inputs["time_x"], np.float32))
    value_x = np.ascontiguousarray(np.asarray(inputs["value_x"], np.float32))
    mask_x = np.asarray(inputs["mask_x"])
    Wi = np.asarray(inputs["Wi"], np.float32)
    bi = np.asarray(inputs["bi"], np.float32)
    I = np.asarray(inputs["I"], np.float32)
    Wq = np.asarray(inputs["Wq"], np.float32)
    bq = np.asarray(inputs["bq"], np.float32)
    Wk = np.asarray(inputs["Wk"], np.float32)
    bk = np.asarray(inputs["bk"], np.float32)
    Wv = np.asarray(inputs["Wv"], np.float32)
    bv = np.asarray(inputs["bv"], np.float32)
    Wo = np.asarray(inputs["Wo"], np.float32)
    bo = np.asarray(inputs["bo"], np.float32)

    mflat = mask_x.reshape(B, L)
    order = np.argsort(1 - mflat, axis=1, kind="stable")
    nvalid = int(mflat.sum(axis=1).max())
    NT = max(1, min(CMAX, -(-nvalid // 128)))
    NTP = NT * 128

    ordp = order[:, :NTP]
    mkc = np.take_along_axis(mflat, ordp, axis=1).astype(np.float32)
    tfull = np.broadcast_to(time_x[:, :, None], (B, S, D)).reshape(B, L)
    tmk = np.take_along_axis(tfull, ordp, axis=1) * mkc
    umk = np.take_along_axis(value_x.reshape(B, L), ordp, axis=1) * mkc
    cidx = np.where(mkc > 0, (ordp % D).astype(np.float32), 63.0).astype(np.float32)

    wtab = np.concatenate([Wi[:D] + bi[None, :], Wi[D:D + 2]], 0)  # [43,128]

    g0 = np.zeros((NLAYERS, LATENT, 512), np.float32)
    qh0 = np.zeros((NLAYERS, NREF, LATENT), np.float32)
    r0 = np.zeros((NLAYERS, 512), np.float32)
    for l in range(NLAYERS):
        Q = I[l] @ Wq[l, 0] + bq[l, 0]
        qh0[l] = Q
        for h in range(NHEADS):
            hs = slice(h * DH, (h + 1) * DH)
            g0[l][:, h * NREF:(h + 1) * NREF] = Wk[l, 0][:, hs] @ Q[:, hs].T
            r0[l][h * NREF:(h + 1) * NREF] = bk[l, 0][hs] @ Q[:, hs].T

    w = dict(
        wv0=Wv[:, 0], wo0=Wo[:, 0], wq1=Wq[:, 1],
        wq1t=np.ascontiguousarray(Wq[:, 1].transpose(0, 2, 1)),
        wk1=Wk[:, 1], wv1=Wv[:, 1], wo1=Wo[:, 1],
    )
    bvec = dict(r0=r0, bv0=bv[:, 0], bo0=bo[:, 0], bq1=bq[:, 1],
                bk1=bk[:, 1], bv1=bv[:, 1], bo1=bo[:, 1])
    flags = {n: bool(np.any(v != 0)) for n, v in bvec.items()}
    return dict(NT=NT, NTP=NTP, tmk=tmk, umk=umk, mkc=mkc, cidx=cidx,
                wtab=wtab, g0=g0, qh0=qh0, w=w, bvec=bvec, flags=flags)


def kernel(**inputs):
    global LAST_RESULT
    p = _prep(inputs)

    key = (p["NT"], tuple(sorted(p["flags"].items())))
    if key not in _PROG_CACHE:
        _PROG_CACHE[key] = _build_program(p["NT"], p["flags"])
    nc = _PROG_CACHE[key]

    import ml_dtypes
    bf16 = ml_dtypes.bfloat16
    shared = dict(wtab=np.ascontiguousarray(p["wtab"]),
                  g0=np.ascontiguousarray(p["g0"].astype(bf16)), qh0=p["qh0"])
    for n, v in p["w"].items():
        shared[n] = np.ascontiguousarray(v.astype(bf16))
    for n, v in p["bvec"].items():
        shared[n] = np.ascontiguousarray(v)

    in_maps = []
    for m in range(NCORES):
        sl = slice(m * BPC, (m + 1) * BPC)
        im = dict(shared)
        im["cidx"] = np.ascontiguousarray(p["cidx"][sl])
        im["tmk"] = np.ascontiguousarray(p["tmk"][sl])
        im["umk"] = np.ascontiguousarray(p["umk"][sl])
        im["mk"] = np.ascontiguousarray(p["mkc"][sl])
        in_maps.append(im)

    res = run_bass_kernel_spmd(nc, in_maps, core_ids=list(range(NCORES)),
                               trace=TRACE)
    LAST_RESULT = res

    Z = np.concatenate([r["zout"] for r in res.results], axis=0)
    mk = np.concatenate([r["mkout"] for r in res.results], axis=0)[..., None]
    return Z.reshape(B, L, LATENT), mk.reshape(B, L, 1)


# revision 23
# speedup vs baseline: 1.7319x; 1.0146x over previous
"""Trainium2 Bass kernel for the masked set-transformer encoder (ISAB stack).

Strategy (pure data parallel, B=16 over 8 cores, 2 batch elements/core):
  * The compaction permutation commutes with the whole network: softmax over
    keys is permutation invariant, everything else is row-wise, and masked
    rows are exactly zero throughout.  So the host only computes the stable
    argsort *indices*; the device processes tokens in compacted order and the
    output is already compacted (zero tail appended on device).
  * Only NT = ceil(max_b nvalid_b / 128) tiles of 128 tokens are processed
    (~42 instead of 82 for random masks).  NT is a compile-time constant
    derived from the actual mask; the program is recompiled if it changes.
  * The one-hot input FF collapses to a [43,128] table matmul against a
    device-built X^T = [one_hot(c); t*mk; u*mk] (no gathers: one-hot rows are
    built with an is_equal against an iota column; invalid tokens get an
    out-of-range channel id so their X^T column is exactly zero).
  * Attention layouts keep softmax reductions on natural axes:
      MAB0 scores  S^T[tok,(h,q)] = Z @ G0,  G0 = fold(Wk, I@Wq+bq)  (host)
      MAB0 key masking is folded into the exp bias: exp(s*scale + (mk-1)*30)
      MAB0 num/den via lhsT=E^T_h, rhs=[Vh_h | 1], DVE-accumulated over chunks
      MAB1 scores  S1^T[k, tok] per head via lhsT=G1_h, rhs=Z^T (4-chunk tiles)
      MAB1 num/den via lhsT=E1^T_h, rhs=[Vh1_h | 1]
  * Z^T lives in SBUF as [128, 512] group tiles so MAB1 score matmuls stream
    512 tokens per instruction.
  * ACT does exp only; biases are all zero in practice (trace-time fallbacks
    emit extra ones-row matmuls / bias adds when they are not).
"""

import math

import numpy as np

import concourse.bacc as bacc
import concourse.bass as bass
import concourse.mybir as mybir
import concourse.tile as tile
from concourse.bass_utils import run_bass_kernel_spmd
from concourse.masks import make_identity

F32 = mybir.dt.float32
BF16 = mybir.dt.bfloat16
AF = mybir.ActivationFunctionType
OP = mybir.AluOpType

B, S, D = 16, 256, 41
L = S * D                      # 10496
LATENT, NREF, NLAYERS, NHEADS = 128, 128, 3, 4
DH = LATENT // NHEADS          # 32
SCALE = 1.0 / math.sqrt(LATENT)
NCORES = 8
BPC = B // NCORES              # 2
CMAX = L // 128                # 82
NEGBIG = -30.0                 # exp(-30) ~ 1e-13: masked-key contribution
GW = 4                         # chunks per Z^T group tile

# set by test harness to capture profiling info
TRACE = False
LAST_RESULT = None

_PROG_CACHE: dict = {}


def _build_program(NT: int, flags: dict, nlayers: int = NLAYERS):
    NTP = NT * 128
    NG = -(-NT // GW)          # number of Z^T group tiles
    nc = bacc.Bacc("TRN2")

    def gslice(c):
        """(group index, column slice within the group tile) for chunk c."""
        return c // GW, slice((c % GW) * 128, (c % GW) * 128 + 128)

    # ---------------- DRAM I/O ----------------
    d_cidx = nc.dram_tensor("cidx", [BPC, NTP], F32, kind="ExternalInput")
    d_tmk = nc.dram_tensor("tmk", [BPC, NTP], F32, kind="ExternalInput")
    d_umk = nc.dram_tensor("umk", [BPC, NTP], F32, kind="ExternalInput")
    d_mk = nc.dram_tensor("mk", [BPC, NTP], F32, kind="ExternalInput")
    d_wtab = nc.dram_tensor("wtab", [43, LATENT], F32, kind="ExternalInput")
    d_g0 = nc.dram_tensor("g0", [NLAYERS, LATENT, 512], BF16, kind="ExternalInput")
    d_qh0 = nc.dram_tensor("qh0", [NLAYERS, NREF, LATENT], F32, kind="ExternalInput")
    WNAMES = ["wv0", "wo0", "wq1", "wq1t", "wk1", "wv1", "wo1"]
    d_w = {
        n: nc.dram_tensor(n, [NLAYERS, LATENT, LATENT], BF16, kind="ExternalInput")
        for n in WNAMES
    }
    BNAMES = ["r0", "bv0", "bo0", "bq1", "bk1", "bv1", "bo1"]
    d_b = {
        n: nc.dram_tensor(n, [NLAYERS, 512 if n == "r0" else LATENT], F32,
                          kind="ExternalInput")
        for n in BNAMES
    }
    d_zout = nc.dram_tensor("zout", [BPC, L, LATENT], F32, kind="ExternalOutput")
    d_mkout = nc.dram_tensor("mkout", [BPC, L], F32, kind="ExternalOutput")

    with tile.TileContext(nc) as tc:
        with (
            tc.tile_pool(name="persist", bufs=1) as pp,
            tc.tile_pool(name="work", bufs=2) as wp,
            tc.tile_pool(name="stream", bufs=6) as sp,
            tc.tile_pool(name="ps_sc", bufs=2, space="PSUM") as ps_sc,
            tc.tile_pool(name="ps_n1", bufs=2, space="PSUM") as ps_n1,
            tc.tile_pool(name="ps_sm", bufs=4, space="PSUM") as ps_sm,
        ):
            # ---------------- constants & weights ----------------
            ident = pp.tile([128, 128], F32, name="ident")
            make_identity(nc, ident[:])

            iota_i = pp.tile([41, 1], mybir.dt.int32, name="iota_i")
            nc.gpsimd.iota(iota_i[:], [[1, 1]], channel_multiplier=1)
            iota_f = pp.tile([41, 1], F32, name="iota_f")
            nc.vector.tensor_copy(iota_f[:], iota_i[:])

            ones_row = pp.tile([1, 128], F32, name="ones_row")
            nc.vector.memset(ones_row[:], 1.0)
            zerot = pp.tile([128, 512], F32, name="zerot")
            nc.vector.memset(zerot[:], 0.0)

            wtab_s = pp.tile([43, LATENT], F32, name="wtab_s")
            nc.sync.dma_start(wtab_s[:], d_wtab[:, :])

            g0_s, qh0_s = [], []
            w_s = {n: [] for n in WNAMES}
            b_s = {n: [] for n in BNAMES}
            for l in range(NLAYERS):
                g = pp.tile([LATENT, 512], BF16, name=f"g0s{l}", tag=f"g0s{l}")
                nc.sync.dma_start(g[:], d_g0[l, :, :])
                g0_s.append(g)
                q = pp.tile([NREF, LATENT], F32, name=f"qh0s{l}", tag=f"qh0s{l}")
                nc.sync.dma_start(q[:], d_qh0[l, :, :])
                qh0_s.append(q)
                for n in WNAMES:
                    if n == "wq1t":
                        # per-head [32,128] tiles (PE weights must start at
                        # partition 0/32/64, so a [96:128] slice is illegal)
                        hh_tiles = []
                        for h in range(NHEADS):
                            t = pp.tile([DH, LATENT], BF16, name=f"wq1t{l}h{h}",
                                        tag=f"wq1t{l}h{h}")
                            nc.sync.dma_start(
                                t[:], d_w[n][l, h * DH:(h + 1) * DH, :])
                            hh_tiles.append(t)
                        w_s[n].append(hh_tiles)
                        continue
                    t = pp.tile([LATENT, LATENT], BF16, name=f"{n}s{l}", tag=f"{n}s{l}")
                    nc.sync.dma_start(t[:], d_w[n][l, :, :])
                    w_s[n].append(t)
                for n in BNAMES:
                    if not flags[n]:
                        b_s[n].append(None)
                        continue
                    if n in ("bk1",):        # needed as a [128,1] column
                        t = pp.tile([LATENT, 1], F32, name=f"{n}s{l}", tag=f"{n}s{l}")
                        nc.sync.dma_start(
                            t[:], bass.AP(d_b[n], l * LATENT, [[1, LATENT], [1, 1]]))
                    elif n == "bq1":         # per-head column tiles [32,1]
                        t = []
                        for h in range(NHEADS):
                            th = pp.tile([DH, 1], F32, name=f"{n}c{l}h{h}",
                                         tag=f"{n}c{l}h{h}")
                            nc.sync.dma_start(
                                th[:], bass.AP(d_b[n], l * LATENT + h * DH,
                                               [[1, DH], [1, 1]]))
                            t.append(th)
                    else:
                        w = 512 if n == "r0" else LATENT
                        t = pp.tile([1, w], F32, name=f"{n}s{l}", tag=f"{n}s{l}")
                        nc.sync.dma_start(t[:], d_b[n][l:l + 1, :])
                    b_s[n].append(t)
                if flags["bq1"]:  # row form for the ones-matmul into Qh1
                    t = pp.tile([1, LATENT], F32, name=f"bq1rs{l}", tag=f"bq1rs{l}")
                    nc.sync.dma_start(t[:], d_b["bq1"][l:l + 1, :])
                    b_s.setdefault("bq1r", []).append(t)

            # ---------------- per-batch setup + Z0 ----------------
            mkp_s, mkneg_s, mkt_s, ZT, ZTB = [], [], [], [], []
            for b in range(BPC):
                mkt = pp.tile([NT, 128], F32, name=f"mkt{b}", tag=f"mkt{b}")
                nc.sync.dma_start(mkt[:], bass.AP(d_mk, b * NTP, [[128, NT], [1, 128]]))
                mkt_s.append(mkt)
                mkpp = ps_sm.tile([128, NT], F32, name="mkpp", tag="sm")
                nc.tensor.transpose(mkpp[:], mkt[:], ident[0:NT, 0:NT])
                mkp = pp.tile([128, NT], F32, name=f"mkp{b}", tag=f"mkp{b}")
                nc.vector.tensor_copy(mkp[:], mkpp[:])
                mkp_s.append(mkp)
                mkneg = pp.tile([128, NT], F32, name=f"mkneg{b}", tag=f"mkneg{b}")
                nc.vector.tensor_scalar(
                    mkneg[:], mkp[:], -1.0, -NEGBIG, op0=OP.add, op1=OP.mult)
                mkneg_s.append(mkneg)

                xt = pp.tile([43, NTP], F32, name=f"xt{b}", tag="xt")
                crow = pp.tile([1, NTP], F32, name=f"crow{b}", tag="crow")
                nc.sync.dma_start(crow[:], d_cidx[b:b + 1, :])
                # replicate cidx row across 41 partitions via a K=1 matmul,
                # then one-hot it against the iota column
                for j in range(0, NTP, 512):
                    w = min(512, NTP - j)
                    cb = ps_sm.tile([41, 512], F32, name="cb", tag="sm")
                    nc.tensor.matmul(cb[:, :w], lhsT=ones_row[:, 0:41],
                                     rhs=crow[:, j:j + w], start=True, stop=True)
                    nc.vector.tensor_scalar(
                        xt[0:41, j:j + w], cb[:, :w], iota_f[:], None,
                        op0=OP.is_equal)
                nc.sync.dma_start(xt[41:42, :], d_tmk[b:b + 1, :])
                nc.sync.dma_start(xt[42:43, :], d_umk[b:b + 1, :])

                ztg, ztbg = [], []
                for g in range(NG):
                    zt = pp.tile([128, GW * 128], F32, name=f"zt{b}_{g}",
                                 tag=f"zt{b}_{g}")
                    ztg.append(zt)
                    ztb = pp.tile([128, GW * 128], BF16, name=f"ztb{b}_{g}",
                                  tag=f"ztb{b}_{g}")
                    ztbg.append(ztb)
                for c in range(NT):
                    g, js = gslice(c)
                    z0p = ps_sm.tile([128, 128], F32, name="z0p", tag="sm")
                    nc.tensor.matmul(
                        z0p[:], lhsT=wtab_s[:], rhs=xt[:, c * 128:(c + 1) * 128],
                        start=True, stop=True)
                    nc.vector.tensor_scalar_max(ztg[g][:, js], z0p[:], 0.0)
                    nc.vector.tensor_copy(ztbg[g][:, js], ztg[g][:, js])
                ZT.append(ztg)
                ZTB.append(ztbg)

            # ---------------- layers ----------------
            for l in range(nlayers):
                num0_b, rd0_b, o0_b, hh_b, ht_b = {}, {}, {}, {}, {}
                kh1_b, g1_b, vo1_b, r1b_b = {}, {}, {}, {}
                for b in range(BPC):
                    num0 = wp.tile([128, 132], F32, name="num0", tag=f"num0{b}")
                    nc.vector.memset(num0[:], 0.0)
                    num0_b[b] = num0
                # ===== MAB0: induced points attend to data =====
                for c in range(NT):
                    for b in range(BPC):
                        ztg, ztbg = ZT[b], ZTB[b]
                        num0 = num0_b[b]
                        g, js = gslice(c)
                        s0 = ps_sc.tile([128, 512], F32, name="s0", tag="sc")
                        if flags["r0"]:
                            nc.tensor.matmul(s0[:], lhsT=ones_row[:],
                                             rhs=b_s["r0"][l][:],
                                             start=True, stop=False)
                        nc.tensor.matmul(s0[:], lhsT=ztbg[g][:, js], rhs=g0_s[l][:],
                                         start=not flags["r0"], stop=True)
                        et = sp.tile([128, 512], BF16, name="et", tag="et")
                        nc.scalar.activation(et[:], s0[:], AF.Exp,
                                             bias=mkneg_s[b][:, c:c + 1],
                                             scale=SCALE)
                        vh = ps_sm.tile([128, 128], F32, name="vh", tag="sm")
                        if flags["bv0"]:
                            nc.tensor.matmul(vh[:], lhsT=ones_row[:],
                                             rhs=b_s["bv0"][l][:],
                                             start=True, stop=False)
                        nc.tensor.matmul(vh[:], lhsT=ztbg[g][:, js],
                                         rhs=w_s["wv0"][l][:],
                                         start=not flags["bv0"], stop=True)
                        vo = sp.tile([128, 132], BF16, name="vo", tag="vo")
                        nc.vector.memset(vo[:], 1.0)
                        nc.vector.tensor_copy(
                            vo[:].rearrange("p (h x) -> p h x", x=33)[:, :, 0:32],
                            vh[:].rearrange("p (h x) -> p h x", x=32))
                        n0c = ps_n1.tile([128, 132], F32, name="n0c", tag="nacc")
                        for h in range(NHEADS):
                            nc.tensor.matmul(
                                n0c[:, h * 33:(h + 1) * 33],
                                lhsT=et[:, h * 128:(h + 1) * 128],
                                rhs=vo[:, h * 33:(h + 1) * 33],
                                start=True, stop=True)
                        nc.vector.tensor_add(num0[:], num0[:], n0c[:])
                # ===== MAB0 tail + MAB1 prep (per batch) =====
                for b in range(BPC):
                    ztg = ZT[b]
                    num0 = num0_b[b]
                    rd0 = sp.tile([128, 4], F32, name="rd0", tag="rd0")
                    nc.vector.reciprocal(
                        rd0[:].rearrange("p (h x) -> p h x", x=1),
                        num0[:].rearrange("p (h x) -> p h x", x=33)[:, :, 32:33])
                    o0 = wp.tile([128, 128], F32, name="o0", tag=f"o0{b}")
                    nc.vector.tensor_tensor(
                        o0[:].rearrange("p (h x) -> p h x", x=32),
                        num0[:].rearrange("p (h x) -> p h x", x=33)[:, :, 0:32],
                        rd0[:].rearrange("p (h x) -> p h x", x=1).to_broadcast(
                            [128, 4, 32]),
                        op=OP.mult)
                    nc.vector.tensor_add(o0[:], o0[:], qh0_s[l][:])
                    o0tp = ps_sm.tile([128, 128], F32, name="o0tp", tag="sm")
                    nc.tensor.transpose(o0tp[:], o0[:], ident[:])
                    o0t = wp.tile([128, 128], BF16, name="o0t", tag=f"o0t{b}")
                    nc.vector.tensor_copy(o0t[:], o0tp[:])
                    fc0 = ps_sm.tile([128, 128], F32, name="fc0", tag="sm")
                    if flags["bo0"]:
                        nc.tensor.matmul(fc0[:], lhsT=ones_row[:],
                                         rhs=b_s["bo0"][l][:], start=True, stop=False)
                    nc.tensor.matmul(fc0[:], lhsT=o0t[:], rhs=w_s["wo0"][l][:],
                                     start=not flags["bo0"], stop=True)
                    hh = wp.tile([128, 128], F32, name="hh", tag=f"hh{b}")
                    nc.vector.scalar_tensor_tensor(
                        hh[:], in0=fc0[:], scalar=0.0, in1=o0[:],
                        op0=OP.max, op1=OP.add)
                    htp = ps_sm.tile([128, 128], F32, name="htp", tag="sm")
                    nc.tensor.transpose(htp[:], hh[:], ident[:])
                    ht = wp.tile([128, 128], BF16, name="ht", tag=f"ht{b}")
                    nc.vector.tensor_copy(ht[:], htp[:])

                    # ===== MAB1 prep =====
                    kh1p = ps_sm.tile([128, 128], F32, name="kh1p", tag="sm")
                    nc.tensor.matmul(kh1p[:], lhsT=w_s["wk1"][l][:], rhs=ht[:],
                                     start=True, stop=True)
                    kh1 = wp.tile([128, 128], BF16, name="kh1", tag=f"kh1{b}")
                    if flags["bk1"]:
                        nc.vector.tensor_scalar_add(kh1[:], kh1p[:], b_s["bk1"][l][:])
                    else:
                        nc.vector.tensor_copy(kh1[:], kh1p[:])
                    kh1h = []
                    for h in range(NHEADS):
                        t = wp.tile([DH, LATENT], BF16, name=f"kh1h{h}", tag=f"kh1h{h}")
                        nc.sync.dma_start(t[:], kh1[h * DH:(h + 1) * DH, :])
                        kh1h.append(t)
                    g1p = ps_sc.tile([128, 512], F32, name="g1p", tag="sc")
                    for h in range(NHEADS):
                        nc.tensor.matmul(g1p[:, h * 128:(h + 1) * 128],
                                         lhsT=w_s["wq1t"][l][h][:], rhs=kh1h[h][:],
                                         start=True, stop=True)
                    g1 = wp.tile([128, 512], BF16, name="g1", tag=f"g1{b}")
                    nc.vector.tensor_copy(g1[:], g1p[:])
                    vh1p = ps_sm.tile([128, 128], F32, name="vh1p", tag="sm")
                    if flags["bv1"]:
                        nc.tensor.matmul(vh1p[:], lhsT=ones_row[:],
                                         rhs=b_s["bv1"][l][:], start=True, stop=False)
                    nc.tensor.matmul(vh1p[:], lhsT=ht[:], rhs=w_s["wv1"][l][:],
                                     start=not flags["bv1"], stop=True)
                    vo1 = wp.tile([128, 132], BF16, name="vo1", tag=f"vo1{b}")
                    nc.vector.memset(vo1[:], 1.0)
                    nc.vector.tensor_copy(
                        vo1[:].rearrange("p (h x) -> p h x", x=33)[:, :, 0:32],
                        vh1p[:].rearrange("p (h x) -> p h x", x=32))
                    r1b = None
                    if flags["bq1"]:
                        r1bp = ps_sm.tile([128, 4], F32, name="r1bp", tag="sm")
                        for h in range(NHEADS):
                            nc.tensor.matmul(r1bp[:, h:h + 1], lhsT=kh1h[h][:],
                                             rhs=b_s["bq1"][l][h][:],
                                             start=True, stop=True)
                        r1b = wp.tile([128, 4], F32, name="r1b", tag=f"r1b{b}")
                        nc.vector.tensor_scalar_mul(r1b[:], r1bp[:], SCALE)
                    g1_b[b], vo1_b[b], r1b_b[b] = g1, vo1, r1b

                # ===== MAB1 chunks: data attends to induced (b-interleaved) ==
                for g in range(NG):
                    gw = min(GW * 128, NTP - g * GW * 128)
                    for b in range(BPC):
                        ztg, ztbg = ZT[b], ZTB[b]
                        g1, vo1, r1b = g1_b[b], vo1_b[b], r1b_b[b]
                        e1h = []
                        for h in range(NHEADS):
                            s1 = ps_sc.tile([128, GW * 128], F32, name="s1", tag="sc")
                            nc.tensor.matmul(s1[:, :gw],
                                             lhsT=g1[:, h * 128:(h + 1) * 128],
                                             rhs=ztbg[g][:, :gw],
                                             start=True, stop=True)
                            e1 = sp.tile([128, GW * 128], BF16, name="e1", tag="e1")
                            if flags["bq1"]:
                                nc.scalar.activation(e1[:, :gw], s1[:, :gw], AF.Exp,
                                                     bias=r1b[:, h:h + 1], scale=SCALE)
                            else:
                                nc.scalar.activation(e1[:, :gw], s1[:, :gw], AF.Exp,
                                                     scale=SCALE)
                            e1h.append(e1)
                        for j in range(gw // 128):
                            c = g * GW + j
                            js = slice(j * 128, (j + 1) * 128)
                            num1 = ps_n1.tile([128, 132], F32, name="num1", tag="nacc")
                            for h in range(NHEADS):
                                nc.tensor.matmul(num1[:, h * 33:(h + 1) * 33],
                                                 lhsT=e1h[h][:, js],
                                                 rhs=vo1[:, h * 33:(h + 1) * 33],
                                                 start=True, stop=True)
                            qh1 = ps_sm.tile([128, 128], F32, name="qh1", tag="sm")
                            if flags["bq1"]:
                                nc.tensor.matmul(qh1[:], lhsT=ones_row[:],
                                                 rhs=b_s["bq1r"][l][:],
                                                 start=True, stop=False)
                            nc.tensor.matmul(qh1[:], lhsT=ztbg[g][:, js],
                                             rhs=w_s["wq1"][l][:],
                                             start=not flags["bq1"], stop=True)
                            rd1 = sp.tile([128, 4], F32, name="rd1", tag="rd1")
                            nc.vector.reciprocal(
                                rd1[:].rearrange("p (h x) -> p h x", x=1),
                                num1[:].rearrange("p (h x) -> p h x", x=33)[:, :, 32:33])
                            o1 = sp.tile([128, 128], F32, name="o1", tag="o1")
                            nc.vector.tensor_tensor(
                                o1[:].rearrange("p (h x) -> p h x", x=32),
                                num1[:].rearrange("p (h x) -> p h x", x=33)[:, :, 0:32],
                                rd1[:].rearrange("p (h x) -> p h x", x=1).to_broadcast(
                                    [128, 4, 32]),
                                op=OP.mult)
                            nc.vector.tensor_add(o1[:], o1[:], qh1[:])
                            o1tp = ps_sm.tile([128, 128], F32, name="o1tp", tag="sm")
                            nc.tensor.transpose(o1tp[:], o1[:], ident[:])
                            o1t = sp.tile([128, 128], BF16, name="o1t", tag="o1t")
                            nc.vector.tensor_copy(o1t[:], o1tp[:])
                            fc1 = ps_sm.tile([128, 128], F32, name="fc1", tag="sm")
                            if flags["bo1"]:
                                nc.tensor.matmul(fc1[:], lhsT=ones_row[:],
                                                 rhs=b_s["bo1"][l][:],
                                                 start=True, stop=False)
                            nc.tensor.matmul(fc1[:], lhsT=o1t[:], rhs=w_s["wo1"][l][:],
                                             start=not flags["bo1"], stop=True)
                            u = sp.tile([128, 128], F32, name="u", tag="u")
                            nc.vector.scalar_tensor_tensor(
                                u[:], in0=fc1[:], scalar=0.0, in1=o1[:],
                                op0=OP.max, op1=OP.add)
                            nc.vector.tensor_scalar_mul(u[:], u[:],
                                                        mkp_s[b][:, c:c + 1])
                            utp = ps_sm.tile([128, 128], F32, name="utp", tag="sm")
                            nc.tensor.transpose(utp[:], u[:], ident[:])
                            nc.vector.tensor_add(ztg[g][:, js], ztg[g][:, js], utp[:])
                            nc.vector.tensor_copy(ztbg[g][:, js], ztg[g][:, js])

            # ---------------- output ----------------
            for c in range(NT):
                for b in range(BPC):
                    g, js = gslice(c)
                    zp = ps_sm.tile([128, 128], F32, name="zp", tag="sm")
                    nc.tensor.transpose(zp[:], ZT[b][g][:, js], ident[:])
                    zo = sp.tile([128, 128], F32, name="zo", tag="zo")
                    nc.vector.tensor_copy(zo[:], zp[:])
                    nc.sync.dma_start(
                        bass.AP(d_zout, (b * L + c * 128) * LATENT,
                                [[LATENT, 128], [1, LATENT]]),
                        zo[:])
            for b in range(BPC):
                # zero tail rows [NTP, L): 2KB-contiguous descriptor runs
                r = NTP
                while r < L:
                    n = min(512, L - r)   # rows; n*128 elems; dst stays contiguous
                    nelem = n * LATENT
                    inner = nelem // 128
                    nc.sync.dma_start(
                        bass.AP(d_zout, (b * L + r) * LATENT,
                                [[inner, 128], [1, inner]]),
                        zerot[:, :inner])
                    r += n
                # mask output (contiguous 512B rows from the token-major tile)
                nc.sync.dma_start(
                    bass.AP(d_mkout, b * L, [[128, NT], [1, 128]]), mkt_s[b][:])
                if CMAX > NT:
                    nc.sync.dma_start(
                        bass.AP(d_mkout, b * L + NTP, [[128, CMAX - NT], [1, 128]]),
                        zerot[0:CMAX - NT, 0:128])
    nc.compile()
    return nc


def _prep(inputs):
    """Host-side prep: compaction indices + weight folding (all O(small))."""
    time_x = np.ascontiguousarray(np.asarray(inputs["time_x"], np.float32))
    value_x = np.ascontiguousarray(np.asarray(inputs["value_x"], np.float32))
    mask_x = np.asarray(inputs["mask_x"])
    Wi = np.asarray(inputs["Wi"], np.float32)
    bi = np.asarray(inputs["bi"], np.float32)
    I = np.asarray(inputs["I"], np.float32)
    Wq = np.asarray(inputs["Wq"], np.float32)
    bq = np.asarray(inputs["bq"], np.float32)
    Wk = np.asarray(inputs["Wk"], np.float32)
    bk = np.asarray(inputs["bk"], np.float32)
    Wv = np.asarray(inputs["Wv"], np.float32)
    bv = np.asarray(inputs["bv"], np.float32)
    Wo = np.asarray(inputs["Wo"], np.float32)
    bo = np.asarray(inputs["bo"], np.float32)

    mflat = mask_x.reshape(B, L)
    order = np.argsort(1 - mflat, axis=1, kind="stable")
    nvalid = int(mflat.sum(axis=1).max())
    NT = max(1, min(CMAX, -(-nvalid // 128)))
    NTP = NT * 128

    ordp = order[:, :NTP]
    mkc = np.take_along_axis(mflat, ordp, axis=1).astype(np.float32)
    tfull = np.broadcast_to(time_x[:, :, None], (B, S, D)).reshape(B, L)
    tmk = np.take_along_axis(tfull, ordp, axis=1) * mkc
    umk = np.take_along_axis(value_x.reshape(B, L), ordp, axis=1) * mkc
    cidx = np.where(mkc > 0, (ordp % D).astype(np.float32), 63.0).astype(np.float32)

    wtab = np.concatenate([Wi[:D] + bi[None, :], Wi[D:D + 2]], 0)  # [43,128]

    g0 = np.zeros((NLAYERS, LATENT, 512), np.float32)
    qh0 = np.zeros((NLAYERS, NREF, LATENT), np.float32)
    r0 = np.zeros((NLAYERS, 512), np.float32)
    for l in range(NLAYERS):
        Q = I[l] @ Wq[l, 0] + bq[l, 0]
        qh0[l] = Q
        for h in range(NHEADS):
            hs = slice(h * DH, (h + 1) * DH)
            g0[l][:, h * NREF:(h + 1) * NREF] = Wk[l, 0][:, hs] @ Q[:, hs].T
            r0[l][h * NREF:(h + 1) * NREF] = bk[l, 0][hs] @ Q[:, hs].T

    w = dict(
        wv0=Wv[:, 0], wo0=Wo[:, 0], wq1=Wq[:, 1],
        wq1t=np.ascontiguousarray(Wq[:, 1].transpose(0, 2, 1)),
        wk1=Wk[:, 1], wv1=Wv[:, 1], wo1=Wo[:, 1],
    )
    bvec = dict(r0=r0, bv0=bv[:, 0], bo0=bo[:, 0], bq1=bq[:, 1],
                bk1=bk[:, 1], bv1=bv[:, 1], bo1=bo[:, 1])
    flags = {n: bool(np.any(v != 0)) for n, v in bvec.items()}
    return dict(NT=NT, NTP=NTP, tmk=tmk, umk=umk, mkc=mkc, cidx=cidx,
                wtab=wtab, g0=g0, qh0=qh0, w=w, bvec=bvec, flags=flags)


def kernel(**inputs):
    global LAST_RESULT
    p = _prep(inputs)

    key = (p["NT"], tuple(sorted(p["flags"].items())))
    if key not in _PROG_CACHE:
        _PROG_CACHE[key] = _build_program(p["NT"], p["flags"])
    nc = _PROG_CACHE[key]

    import ml_dtypes
    bf16 = ml_dtypes.bfloat16
    shared = dict(wtab=np.ascontiguousarray(p["wtab"]),
                  g0=np.ascontiguousarray(p["g0"].astype(bf16)), qh0=p["qh0"])
    for n, v in p["w"].items():
        shared[n] = np.ascontiguousarray(v.astype(bf16))
    for n, v in p["bvec"].items():
        shared[n] = np.ascontiguousarray(v)

    in_maps = []
    for m in range(NCORES):
        sl = slice(m * BPC, (m + 1) * BPC)
        im = dict(shared)
        im["cidx"] = np.ascontiguousarray(p["cidx"][sl])
        im["tmk"] = np.ascontiguousarray(p["tmk"][sl])
        im["umk"] = np.ascontiguousarray(p["umk"][sl])
        im["mk"] = np.ascontiguousarray(p["mkc"][sl])
        in_maps.append(im)

    res = run_bass_kernel_spmd(nc, in_maps, core_ids=list(range(NCORES)),
                               trace=TRACE)
    LAST_RESULT = res

    Z = np.concatenate([r["zout"] for r in res.results], axis=0)
    mk = np.concatenate([r["mkout"] for r in res.results], axis=0)[..., None]
    return Z.reshape(B, L, LATENT), mk.reshape(B, L, 1)


# revision 24
# speedup vs baseline: 1.8746x; 1.0824x over previous
"""Trainium2 Bass kernel for the masked set-transformer encoder (ISAB stack).

Strategy (pure data parallel, B=16 over 8 cores, 2 batch elements/core):
  * The compaction permutation commutes with the whole network: softmax over
    keys is permutation invariant, everything else is row-wise, and masked
    rows are exactly zero throughout.  So the host only computes the stable
    argsort *indices*; the device processes tokens in compacted order and the
    output is already compacted (zero tail appended on device).
  * Only NT = ceil(max_b nvalid_b / 128) tiles of 128 tokens are processed
    (~42 instead of 82 for random masks).  NT is a compile-time constant
    derived from the actual mask; the program is recompiled if it changes.
  * The one-hot input FF collapses to a [43,128] table matmul against a
    device-built X^T = [one_hot(c); t*mk; u*mk] (no gathers: one-hot rows are
    built with an is_equal against an iota column; invalid tokens get an
    out-of-range channel id so their X^T column is exactly zero).
  * Attention layouts keep softmax reductions on natural axes:
      MAB0 scores  S^T[tok,(h,q)] = Z @ G0,  G0 = fold(Wk, I@Wq+bq)  (host)
      MAB0 key masking is folded into the exp bias: exp(s*scale + (mk-1)*30)
      MAB0 num/den via lhsT=E^T_h, rhs=[Vh_h | 1], DVE-accumulated over chunks
      MAB1 scores  S1^T[k, tok] per head via lhsT=G1_h, rhs=Z^T (4-chunk tiles)
      MAB1 num/den via lhsT=E1^T_h, rhs=[Vh1_h | 1]
  * Z^T lives in SBUF as [128, 512] group tiles so MAB1 score matmuls stream
    512 tokens per instruction.
  * ACT does exp only; biases are all zero in practice (trace-time fallbacks
    emit extra ones-row matmuls / bias adds when they are not).
"""

import math

import numpy as np

import concourse.bacc as bacc
import concourse.bass as bass
import concourse.mybir as mybir
import concourse.tile as tile
from concourse.bass_utils import run_bass_kernel_spmd
from concourse.masks import make_identity

F32 = mybir.dt.float32
BF16 = mybir.dt.bfloat16
AF = mybir.ActivationFunctionType
OP = mybir.AluOpType

B, S, D = 16, 256, 41
L = S * D                      # 10496
LATENT, NREF, NLAYERS, NHEADS = 128, 128, 3, 4
DH = LATENT // NHEADS          # 32
SCALE = 1.0 / math.sqrt(LATENT)
NCORES = 8
BPC = B // NCORES              # 2
CMAX = L // 128                # 82
NEGBIG = -30.0                 # exp(-30) ~ 1e-13: masked-key contribution
GW = 4                         # chunks per Z^T group tile

# set by test harness to capture profiling info
TRACE = False
LAST_RESULT = None

_PROG_CACHE: dict = {}


def _build_program(NT: int, flags: dict, nlayers: int = NLAYERS):
    NTP = NT * 128
    NG = -(-NT // GW)          # number of Z^T group tiles
    nc = bacc.Bacc("TRN2")

    def gslice(c):
        """(group index, column slice within the group tile) for chunk c."""
        return c // GW, slice((c % GW) * 128, (c % GW) * 128 + 128)

    # ---------------- DRAM I/O ----------------
    d_cidx = nc.dram_tensor("cidx", [BPC, NTP], F32, kind="ExternalInput")
    d_tmk = nc.dram_tensor("tmk", [BPC, NTP], F32, kind="ExternalInput")
    d_umk = nc.dram_tensor("umk", [BPC, NTP], F32, kind="ExternalInput")
    d_mk = nc.dram_tensor("mk", [BPC, NTP], F32, kind="ExternalInput")
    d_wtab = nc.dram_tensor("wtab", [43, LATENT], F32, kind="ExternalInput")
    d_g0 = nc.dram_tensor("g0", [NLAYERS, LATENT, 512], BF16, kind="ExternalInput")
    d_qh0 = nc.dram_tensor("qh0", [NLAYERS, NREF, LATENT], F32, kind="ExternalInput")
    WNAMES = ["wv0", "wo0", "wq1", "wq1t", "wk1", "wv1", "wo1"]
    d_w = {
        n: nc.dram_tensor(n, [NLAYERS, LATENT, LATENT], BF16, kind="ExternalInput")
        for n in WNAMES
    }
    BNAMES = ["r0", "bv0", "bo0", "bq1", "bk1", "bv1", "bo1"]
    d_b = {
        n: nc.dram_tensor(n, [NLAYERS, 512 if n == "r0" else LATENT], F32,
                          kind="ExternalInput")
        for n in BNAMES
    }
    d_zout = nc.dram_tensor("zout", [BPC, L, LATENT], F32, kind="ExternalOutput")
    d_mkout = nc.dram_tensor("mkout", [BPC, L], F32, kind="ExternalOutput")

    with tile.TileContext(nc) as tc:
        with (
            tc.tile_pool(name="persist", bufs=1) as pp,
            tc.tile_pool(name="work", bufs=2) as wp,
            tc.tile_pool(name="stream", bufs=6) as sp,
            tc.tile_pool(name="ps_sc", bufs=2, space="PSUM") as ps_sc,
            tc.tile_pool(name="ps_n1", bufs=2, space="PSUM") as ps_n1,
            tc.tile_pool(name="ps_sm", bufs=4, space="PSUM") as ps_sm,
        ):
            # ---------------- constants & weights ----------------
            ident = pp.tile([128, 128], F32, name="ident")
            make_identity(nc, ident[:])

            iota_i = pp.tile([41, 1], mybir.dt.int32, name="iota_i")
            nc.gpsimd.iota(iota_i[:], [[1, 1]], channel_multiplier=1)
            iota_f = pp.tile([41, 1], F32, name="iota_f")
            nc.vector.tensor_copy(iota_f[:], iota_i[:])

            ones_row = pp.tile([1, 128], F32, name="ones_row")
            nc.vector.memset(ones_row[:], 1.0)
            zerot = pp.tile([128, 512], F32, name="zerot")
            nc.vector.memset(zerot[:], 0.0)

            wtab_s = pp.tile([43, LATENT], F32, name="wtab_s")
            nc.sync.dma_start(wtab_s[:], d_wtab[:, :])

            g0_s, qh0_s = [], []
            w_s = {n: [] for n in WNAMES}
            b_s = {n: [] for n in BNAMES}
            for l in range(NLAYERS):
                g = pp.tile([LATENT, 512], BF16, name=f"g0s{l}", tag=f"g0s{l}")
                nc.sync.dma_start(g[:], d_g0[l, :, :])
                g0_s.append(g)
                q = pp.tile([NREF, LATENT], F32, name=f"qh0s{l}", tag=f"qh0s{l}")
                nc.sync.dma_start(q[:], d_qh0[l, :, :])
                qh0_s.append(q)
                for n in WNAMES:
                    if n == "wq1t":
                        # per-head [32,128] tiles (PE weights must start at
                        # partition 0/32/64, so a [96:128] slice is illegal)
                        hh_tiles = []
                        for h in range(NHEADS):
                            t = pp.tile([DH, LATENT], BF16, name=f"wq1t{l}h{h}",
                                        tag=f"wq1t{l}h{h}")
                            nc.sync.dma_start(
                                t[:], d_w[n][l, h * DH:(h + 1) * DH, :])
                            hh_tiles.append(t)
                        w_s[n].append(hh_tiles)
                        continue
                    t = pp.tile([LATENT, LATENT], BF16, name=f"{n}s{l}", tag=f"{n}s{l}")
                    nc.sync.dma_start(t[:], d_w[n][l, :, :])
                    w_s[n].append(t)
                for n in BNAMES:
                    if not flags[n]:
                        b_s[n].append(None)
                        continue
                    if n in ("bk1",):        # needed as a [128,1] column
                        t = pp.tile([LATENT, 1], F32, name=f"{n}s{l}", tag=f"{n}s{l}")
                        nc.sync.dma_start(
                            t[:], bass.AP(d_b[n], l * LATENT, [[1, LATENT], [1, 1]]))
                    elif n == "bq1":         # per-head column tiles [32,1]
                        t = []
                        for h in range(NHEADS):
                            th = pp.tile([DH, 1], F32, name=f"{n}c{l}h{h}",
                                         tag=f"{n}c{l}h{h}")
                            nc.sync.dma_start(
                                th[:], bass.AP(d_b[n], l * LATENT + h * DH,
                                               [[1, DH], [1, 1]]))
                            t.append(th)
                    else:
                        w = 512 if n == "r0" else LATENT
                        t = pp.tile([1, w], F32, name=f"{n}s{l}", tag=f"{n}s{l}")
                        nc.sync.dma_start(t[:], d_b[n][l:l + 1, :])
                    b_s[n].append(t)
                if flags["bq1"]:  # row form for the ones-matmul into Qh1
                    t = pp.tile([1, LATENT], F32, name=f"bq1rs{l}", tag=f"bq1rs{l}")
                    nc.sync.dma_start(t[:], d_b["bq1"][l:l + 1, :])
                    b_s.setdefault("bq1r", []).append(t)

            # ---------------- per-batch setup + Z0 ----------------
            mkp_s, mkneg_s, mkt_s, ZT, ZTB = [], [], [], [], []
            for b in range(BPC):
                mkt = pp.tile([NT, 128], F32, name=f"mkt{b}", tag=f"mkt{b}")
                nc.sync.dma_start(mkt[:], bass.AP(d_mk, b * NTP, [[128, NT], [1, 128]]))
                mkt_s.append(mkt)
                mkpp = ps_sm.tile([128, NT], F32, name="mkpp", tag="sm")
                nc.tensor.transpose(mkpp[:], mkt[:], ident[0:NT, 0:NT])
                mkp = pp.tile([128, NT], F32, name=f"mkp{b}", tag=f"mkp{b}")
                nc.vector.tensor_copy(mkp[:], mkpp[:])
                mkp_s.append(mkp)
                mkneg = pp.tile([128, NT], F32, name=f"mkneg{b}", tag=f"mkneg{b}")
                nc.vector.tensor_scalar(
                    mkneg[:], mkp[:], -1.0, -NEGBIG, op0=OP.add, op1=OP.mult)
                mkneg_s.append(mkneg)

                xt = pp.tile([43, NTP], F32, name=f"xt{b}", tag="xt")
                crow = pp.tile([1, NTP], F32, name=f"crow{b}", tag="crow")
                nc.sync.dma_start(crow[:], d_cidx[b:b + 1, :])
                # replicate cidx row across 41 partitions via a K=1 matmul,
                # then one-hot it against the iota column
                for j in range(0, NTP, 512):
                    w = min(512, NTP - j)
                    cb = ps_sm.tile([41, 512], F32, name="cb", tag="sm")
                    nc.tensor.matmul(cb[:, :w], lhsT=ones_row[:, 0:41],
                                     rhs=crow[:, j:j + w], start=True, stop=True)
                    nc.vector.tensor_scalar(
                        xt[0:41, j:j + w], cb[:, :w], iota_f[:], None,
                        op0=OP.is_equal)
                nc.sync.dma_start(xt[41:42, :], d_tmk[b:b + 1, :])
                nc.sync.dma_start(xt[42:43, :], d_umk[b:b + 1, :])

                ztg, ztbg = [], []
                for g in range(NG):
                    zt = pp.tile([128, GW * 128], F32, name=f"zt{b}_{g}",
                                 tag=f"zt{b}_{g}")
                    ztg.append(zt)
                    ztb = pp.tile([128, GW * 128], BF16, name=f"ztb{b}_{g}",
                                  tag=f"ztb{b}_{g}")
                    ztbg.append(ztb)
                for c in range(NT):
                    g, js = gslice(c)
                    z0p = ps_sm.tile([128, 128], F32, name="z0p", tag="sm")
                    nc.tensor.matmul(
                        z0p[:], lhsT=wtab_s[:], rhs=xt[:, c * 128:(c + 1) * 128],
                        start=True, stop=True)
                    nc.vector.tensor_scalar_max(ztg[g][:, js], z0p[:], 0.0)
                    nc.gpsimd.tensor_copy(ztbg[g][:, js], ztg[g][:, js])
                ZT.append(ztg)
                ZTB.append(ztbg)

            # ---------------- layers ----------------
            for l in range(nlayers):
                num0_b, rd0_b, o0_b, hh_b, ht_b = {}, {}, {}, {}, {}
                kh1_b, g1_b, vo1_b, r1b_b = {}, {}, {}, {}
                for b in range(BPC):
                    num0 = wp.tile([128, 132], F32, name="num0", tag=f"num0{b}")
                    nc.vector.memset(num0[:], 0.0)
                    num0_b[b] = num0
                # ===== MAB0: induced points attend to data =====
                for c in range(NT):
                    for b in range(BPC):
                        ztg, ztbg = ZT[b], ZTB[b]
                        num0 = num0_b[b]
                        g, js = gslice(c)
                        s0 = ps_sc.tile([128, 512], F32, name="s0", tag="sc")
                        if flags["r0"]:
                            nc.tensor.matmul(s0[:], lhsT=ones_row[:],
                                             rhs=b_s["r0"][l][:],
                                             start=True, stop=False)
                        nc.tensor.matmul(s0[:], lhsT=ztbg[g][:, js], rhs=g0_s[l][:],
                                         start=not flags["r0"], stop=True)
                        et = sp.tile([128, 512], BF16, name="et", tag="et")
                        nc.scalar.activation(et[:], s0[:], AF.Exp,
                                             bias=mkneg_s[b][:, c:c + 1],
                                             scale=SCALE)
                        vh = ps_sm.tile([128, 128], F32, name="vh", tag="sm")
                        if flags["bv0"]:
                            nc.tensor.matmul(vh[:], lhsT=ones_row[:],
                                             rhs=b_s["bv0"][l][:],
                                             start=True, stop=False)
                        nc.tensor.matmul(vh[:], lhsT=ztbg[g][:, js],
                                         rhs=w_s["wv0"][l][:],
                                         start=not flags["bv0"], stop=True)
                        vo = sp.tile([128, 132], BF16, name="vo", tag="vo")
                        nc.gpsimd.memset(vo[:], 1.0)
                        nc.vector.tensor_copy(
                            vo[:].rearrange("p (h x) -> p h x", x=33)[:, :, 0:32],
                            vh[:].rearrange("p (h x) -> p h x", x=32))
                        n0c = ps_n1.tile([128, 132], F32, name="n0c", tag="nacc")
                        for h in range(NHEADS):
                            nc.tensor.matmul(
                                n0c[:, h * 33:(h + 1) * 33],
                                lhsT=et[:, h * 128:(h + 1) * 128],
                                rhs=vo[:, h * 33:(h + 1) * 33],
                                start=True, stop=True)
                        nc.vector.tensor_add(num0[:], num0[:], n0c[:])
                # ===== MAB0 tail + MAB1 prep (per batch) =====
                for b in range(BPC):
                    ztg = ZT[b]
                    num0 = num0_b[b]
                    rd0 = sp.tile([128, 4], F32, name="rd0", tag="rd0")
                    nc.vector.reciprocal(
                        rd0[:].rearrange("p (h x) -> p h x", x=1),
                        num0[:].rearrange("p (h x) -> p h x", x=33)[:, :, 32:33])
                    o0 = wp.tile([128, 128], F32, name="o0", tag=f"o0{b}")
                    nc.vector.tensor_tensor(
                        o0[:].rearrange("p (h x) -> p h x", x=32),
                        num0[:].rearrange("p (h x) -> p h x", x=33)[:, :, 0:32],
                        rd0[:].rearrange("p (h x) -> p h x", x=1).to_broadcast(
                            [128, 4, 32]),
                        op=OP.mult)
                    nc.vector.tensor_add(o0[:], o0[:], qh0_s[l][:])
                    o0tp = ps_sm.tile([128, 128], F32, name="o0tp", tag="sm")
                    nc.tensor.transpose(o0tp[:], o0[:], ident[:])
                    o0t = wp.tile([128, 128], BF16, name="o0t", tag=f"o0t{b}")
                    nc.vector.tensor_copy(o0t[:], o0tp[:])
                    fc0 = ps_sm.tile([128, 128], F32, name="fc0", tag="sm")
                    if flags["bo0"]:
                        nc.tensor.matmul(fc0[:], lhsT=ones_row[:],
                                         rhs=b_s["bo0"][l][:], start=True, stop=False)
                    nc.tensor.matmul(fc0[:], lhsT=o0t[:], rhs=w_s["wo0"][l][:],
                                     start=not flags["bo0"], stop=True)
                    hh = wp.tile([128, 128], F32, name="hh", tag=f"hh{b}")
                    nc.vector.scalar_tensor_tensor(
                        hh[:], in0=fc0[:], scalar=0.0, in1=o0[:],
                        op0=OP.max, op1=OP.add)
                    htp = ps_sm.tile([128, 128], F32, name="htp", tag="sm")
                    nc.tensor.transpose(htp[:], hh[:], ident[:])
                    ht = wp.tile([128, 128], BF16, name="ht", tag=f"ht{b}")
                    nc.vector.tensor_copy(ht[:], htp[:])

                    # ===== MAB1 prep =====
                    kh1p = ps_sm.tile([128, 128], F32, name="kh1p", tag="sm")
                    nc.tensor.matmul(kh1p[:], lhsT=w_s["wk1"][l][:], rhs=ht[:],
                                     start=True, stop=True)
                    kh1 = wp.tile([128, 128], BF16, name="kh1", tag=f"kh1{b}")
                    if flags["bk1"]:
                        nc.vector.tensor_scalar_add(kh1[:], kh1p[:], b_s["bk1"][l][:])
                    else:
                        nc.vector.tensor_copy(kh1[:], kh1p[:])
                    kh1h = []
                    for h in range(NHEADS):
                        t = wp.tile([DH, LATENT], BF16, name=f"kh1h{h}", tag=f"kh1h{h}")
                        nc.sync.dma_start(t[:], kh1[h * DH:(h + 1) * DH, :])
                        kh1h.append(t)
                    g1p = ps_sc.tile([128, 512], F32, name="g1p", tag="sc")
                    for h in range(NHEADS):
                        nc.tensor.matmul(g1p[:, h * 128:(h + 1) * 128],
                                         lhsT=w_s["wq1t"][l][h][:], rhs=kh1h[h][:],
                                         start=True, stop=True)
                    g1 = wp.tile([128, 512], BF16, name="g1", tag=f"g1{b}")
                    nc.vector.tensor_copy(g1[:], g1p[:])
                    vh1p = ps_sm.tile([128, 128], F32, name="vh1p", tag="sm")
                    if flags["bv1"]:
                        nc.tensor.matmul(vh1p[:], lhsT=ones_row[:],
                                         rhs=b_s["bv1"][l][:], start=True, stop=False)
                    nc.tensor.matmul(vh1p[:], lhsT=ht[:], rhs=w_s["wv1"][l][:],
                                     start=not flags["bv1"], stop=True)
                    vo1 = wp.tile([128, 132], BF16, name="vo1", tag=f"vo1{b}")
                    nc.vector.memset(vo1[:], 1.0)
                    nc.vector.tensor_copy(
                        vo1[:].rearrange("p (h x) -> p h x", x=33)[:, :, 0:32],
                        vh1p[:].rearrange("p (h x) -> p h x", x=32))
                    r1b = None
                    if flags["bq1"]:
                        r1bp = ps_sm.tile([128, 4], F32, name="r1bp", tag="sm")
                        for h in range(NHEADS):
                            nc.tensor.matmul(r1bp[:, h:h + 1], lhsT=kh1h[h][:],
                                             rhs=b_s["bq1"][l][h][:],
                                             start=True, stop=True)
                        r1b = wp.tile([128, 4], F32, name="r1b", tag=f"r1b{b}")
                        nc.vector.tensor_scalar_mul(r1b[:], r1bp[:], SCALE)
                    g1_b[b], vo1_b[b], r1b_b[b] = g1, vo1, r1b

                # ===== MAB1 chunks: data attends to induced (b-interleaved) ==
                for g in range(NG):
                    gw = min(GW * 128, NTP - g * GW * 128)
                    for b in range(BPC):
                        ztg, ztbg = ZT[b], ZTB[b]
                        g1, vo1, r1b = g1_b[b], vo1_b[b], r1b_b[b]
                        e1h = []
                        for h in range(NHEADS):
                            s1 = ps_sc.tile([128, GW * 128], F32, name="s1", tag="sc")
                            nc.tensor.matmul(s1[:, :gw],
                                             lhsT=g1[:, h * 128:(h + 1) * 128],
                                             rhs=ztbg[g][:, :gw],
                                             start=True, stop=True)
                            e1 = sp.tile([128, GW * 128], BF16, name="e1", tag="e1")
                            if flags["bq1"]:
                                nc.scalar.activation(e1[:, :gw], s1[:, :gw], AF.Exp,
                                                     bias=r1b[:, h:h + 1], scale=SCALE)
                            else:
                                nc.scalar.activation(e1[:, :gw], s1[:, :gw], AF.Exp,
                                                     scale=SCALE)
                            e1h.append(e1)
                        for j in range(gw // 128):
                            c = g * GW + j
                            js = slice(j * 128, (j + 1) * 128)
                            num1 = ps_n1.tile([128, 132], F32, name="num1", tag="nacc")
                            for h in range(NHEADS):
                                nc.tensor.matmul(num1[:, h * 33:(h + 1) * 33],
                                                 lhsT=e1h[h][:, js],
                                                 rhs=vo1[:, h * 33:(h + 1) * 33],
                                                 start=True, stop=True)
                            qh1 = ps_sm.tile([128, 128], F32, name="qh1", tag="sm")
                            if flags["bq1"]:
                                nc.tensor.matmul(qh1[:], lhsT=ones_row[:],
                                                 rhs=b_s["bq1r"][l][:],
                                                 start=True, stop=False)
                            nc.tensor.matmul(qh1[:], lhsT=ztbg[g][:, js],
                                             rhs=w_s["wq1"][l][:],
                                             start=not flags["bq1"], stop=True)
                            rd1 = sp.tile([128, 4], F32, name="rd1", tag="rd1")
                            nc.vector.reciprocal(
                                rd1[:].rearrange("p (h x) -> p h x", x=1),
                                num1[:].rearrange("p (h x) -> p h x", x=33)[:, :, 32:33])
                            o1 = sp.tile([128, 128], F32, name="o1", tag="o1")
                            if not flags["bo1"]:
                                # masked-O1 path: invalid rows are exactly
                                # zeroed through fc_o since bo1 == 0
                                nc.vector.tensor_scalar_mul(
                                    rd1[:], rd1[:], mkp_s[b][:, c:c + 1])
                            nc.vector.tensor_tensor(
                                o1[:].rearrange("p (h x) -> p h x", x=32),
                                num1[:].rearrange("p (h x) -> p h x", x=33)[:, :, 0:32],
                                rd1[:].rearrange("p (h x) -> p h x", x=1).to_broadcast(
                                    [128, 4, 32]),
                                op=OP.mult)
                            if flags["bo1"]:
                                nc.vector.tensor_add(o1[:], o1[:], qh1[:])
                            else:
                                nc.vector.scalar_tensor_tensor(
                                    o1[:], in0=qh1[:], scalar=mkp_s[b][:, c:c + 1],
                                    in1=o1[:], op0=OP.mult, op1=OP.add)
                            o1tp = ps_sm.tile([128, 128], F32, name="o1tp", tag="sm")
                            nc.tensor.transpose(o1tp[:], o1[:], ident[:])
                            o1t = sp.tile([128, 128], BF16, name="o1t", tag="o1t")
                            nc.scalar.copy(o1t[:], o1tp[:])
                            fc1 = ps_sm.tile([128, 128], F32, name="fc1", tag="sm")
                            if flags["bo1"]:
                                nc.tensor.matmul(fc1[:], lhsT=ones_row[:],
                                                 rhs=b_s["bo1"][l][:],
                                                 start=True, stop=False)
                            nc.tensor.matmul(fc1[:], lhsT=o1t[:], rhs=w_s["wo1"][l][:],
                                             start=not flags["bo1"], stop=True)
                            u = sp.tile([128, 128], F32, name="u", tag="u")
                            nc.vector.scalar_tensor_tensor(
                                u[:], in0=fc1[:], scalar=0.0, in1=o1[:],
                                op0=OP.max, op1=OP.add)
                            if flags["bo1"]:
                                nc.vector.tensor_scalar_mul(u[:], u[:],
                                                            mkp_s[b][:, c:c + 1])
                            utp = ps_sm.tile([128, 128], F32, name="utp", tag="sm")
                            nc.tensor.transpose(utp[:], u[:], ident[:])
                            nc.vector.tensor_add(ztg[g][:, js], ztg[g][:, js], utp[:])
                            nc.gpsimd.tensor_copy(ztbg[g][:, js], ztg[g][:, js])

            # ---------------- output ----------------
            for c in range(NT):
                for b in range(BPC):
                    g, js = gslice(c)
                    zp = ps_sm.tile([128, 128], F32, name="zp", tag="sm")
                    nc.tensor.transpose(zp[:], ZT[b][g][:, js], ident[:])
                    zo = sp.tile([128, 128], F32, name="zo", tag="zo")
                    nc.vector.tensor_copy(zo[:], zp[:])
                    nc.sync.dma_start(
                        bass.AP(d_zout, (b * L + c * 128) * LATENT,
                                [[LATENT, 128], [1, LATENT]]),
                        zo[:])
            for b in range(BPC):
                # zero tail rows [NTP, L): 2KB-contiguous descriptor runs
                r = NTP
                while r < L:
                    n = min(512, L - r)   # rows; n*128 elems; dst stays contiguous
                    nelem = n * LATENT
                    inner = nelem // 128
                    nc.sync.dma_start(
                        bass.AP(d_zout, (b * L + r) * LATENT,
                                [[inner, 128], [1, inner]]),
                        zerot[:, :inner])
                    r += n
                # mask output (contiguous 512B rows from the token-major tile)
                nc.sync.dma_start(
                    bass.AP(d_mkout, b * L, [[128, NT], [1, 128]]), mkt_s[b][:])
                if CMAX > NT:
                    nc.sync.dma_start(
                        bass.AP(d_mkout, b * L + NTP, [[128, CMAX - NT], [1, 128]]),
                        zerot[0:CMAX - NT, 0:128])
    nc.compile()
    return nc


def _prep(inputs):
    """Host-side prep: compaction indices + weight folding (all O(small))."""
    time_x = np.ascontiguousarray(np.asarray(inputs["time_x"], np.float32))
    value_x = np.ascontiguousarray(np.asarray(inputs["value_x"], np.float32))
    mask_x = np.asarray(inputs["mask_x"])
    Wi = np.asarray(inputs["Wi"], np.float32)
    bi = np.asarray(inputs["bi"], np.float32)
    I = np.asarray(inputs["I"], np.float32)
    Wq = np.asarray(inputs["Wq"], np.float32)
    bq = np.asarray(inputs["bq"], np.float32)
    Wk = np.asarray(inputs["Wk"], np.float32)
    bk = np.asarray(inputs["bk"], np.float32)
    Wv = np.asarray(inputs["Wv"], np.float32)
    bv = np.asarray(inputs["bv"], np.float32)
    Wo = np.asarray(inputs["Wo"], np.float32)
    bo = np.asarray(inputs["bo"], np.float32)

    mflat = mask_x.reshape(B, L)
    order = np.argsort(1 - mflat, axis=1, kind="stable")
    nvalid = int(mflat.sum(axis=1).max())
    NT = max(1, min(CMAX, -(-nvalid // 128)))
    NTP = NT * 128

    ordp = order[:, :NTP]
    mkc = np.take_along_axis(mflat, ordp, axis=1).astype(np.float32)
    tfull = np.broadcast_to(time_x[:, :, None], (B, S, D)).reshape(B, L)
    tmk = np.take_along_axis(tfull, ordp, axis=1) * mkc
    umk = np.take_along_axis(value_x.reshape(B, L), ordp, axis=1) * mkc
    cidx = np.where(mkc > 0, (ordp % D).astype(np.float32), 63.0).astype(np.float32)

    wtab = np.concatenate([Wi[:D] + bi[None, :], Wi[D:D + 2]], 0)  # [43,128]

    g0 = np.zeros((NLAYERS, LATENT, 512), np.float32)
    qh0 = np.zeros((NLAYERS, NREF, LATENT), np.float32)
    r0 = np.zeros((NLAYERS, 512), np.float32)
    for l in range(NLAYERS):
        Q = I[l] @ Wq[l, 0] + bq[l, 0]
        qh0[l] = Q
        for h in range(NHEADS):
            hs = slice(h * DH, (h + 1) * DH)
            g0[l][:, h * NREF:(h + 1) * NREF] = Wk[l, 0][:, hs] @ Q[:, hs].T
            r0[l][h * NREF:(h + 1) * NREF] = bk[l, 0][hs] @ Q[:, hs].T

    w = dict(
        wv0=Wv[:, 0], wo0=Wo[:, 0], wq1=Wq[:, 1],
        wq1t=np.ascontiguousarray(Wq[:, 1].transpose(0, 2, 1)),
        wk1=Wk[:, 1], wv1=Wv[:, 1], wo1=Wo[:, 1],
    )
    bvec = dict(r0=r0, bv0=bv[:, 0], bo0=bo[:, 0], bq1=bq[:, 1],
                bk1=bk[:, 1], bv1=bv[:, 1], bo1=bo[:, 1])
    flags = {n: bool(np.any(v != 0)) for n, v in bvec.items()}
    return dict(NT=NT, NTP=NTP, tmk=tmk, umk=umk, mkc=mkc, cidx=cidx,
                wtab=wtab, g0=g0, qh0=qh0, w=w, bvec=bvec, flags=flags)


def kernel(**inputs):
    global LAST_RESULT
    p = _prep(inputs)

    key = (p["NT"], tuple(sorted(p["flags"].items())))
    if key not in _PROG_CACHE:
        _PROG_CACHE[key] = _build_program(p["NT"], p["flags"])
    nc = _PROG_CACHE[key]

    import ml_dtypes
    bf16 = ml_dtypes.bfloat16
    shared = dict(wtab=np.ascontiguousarray(p["wtab"]),
                  g0=np.ascontiguousarray(p["g0"].astype(bf16)), qh0=p["qh0"])
    for n, v in p["w"].items():
        shared[n] = np.ascontiguousarray(v.astype(bf16))
    for n, v in p["bvec"].items():
        shared[n] = np.ascontiguousarray(v)

    in_maps = []
    for m in range(NCORES):
        sl = slice(m * BPC, (m + 1) * BPC)
        im = dict(shared)
        im["cidx"] = np.ascontiguousarray(p["cidx"][sl])
        im["tmk"] = np.ascontiguousarray(p["tmk"][sl])
        im["umk"] = np.ascontiguousarray(p["umk"][sl])
        im["mk"] = np.ascontiguousarray(p["mkc"][sl])
        in_maps.append(im)

    res = run_bass_kernel_spmd(nc, in_maps, core_ids=list(range(NCORES)),
                               trace=TRACE)
    LAST_RESULT = res

    Z = np.concatenate([r["zout"] for r in res.results], axis=0)
    mk = np.concatenate([r["mkout"] for r in res.results], axis=0)[..., None]
    return Z.reshape(B, L, LATENT), mk.reshape(B, L, 1)


# revision 25
# speedup vs baseline: 1.8756x; 1.0006x over previous
"""Trainium2 Bass kernel for the masked set-transformer encoder (ISAB stack).

Strategy (pure data parallel, B=16 over 8 cores, 2 batch elements/core):
  * The compaction permutation commutes with the whole network: softmax over
    keys is permutation invariant, everything else is row-wise, and masked
    rows are exactly zero throughout.  So the host only computes the stable
    argsort *indices*; the device processes tokens in compacted order and the
    output is already compacted (zero tail appended on device).
  * Only NT = ceil(max_b nvalid_b / 128) tiles of 128 tokens are processed
    (~42 instead of 82 for random masks).  NT is a compile-time constant
    derived from the actual mask; the program is recompiled if it changes.
  * The one-hot input FF collapses to a [43,128] table matmul against a
    device-built X^T = [one_hot(c); t*mk; u*mk] (no gathers: one-hot rows are
    built with an is_equal against an iota column; invalid tokens get an
    out-of-range channel id so their X^T column is exactly zero).
  * Attention layouts keep softmax reductions on natural axes:
      MAB0 scores  S^T[tok,(h,q)] = Z @ G0,  G0 = fold(Wk, I@Wq+bq)  (host)
      MAB0 key masking is folded into the exp bias: exp(s*scale + (mk-1)*30)
      MAB0 num/den via lhsT=E^T_h, rhs=[Vh_h | 1], DVE-accumulated over chunks
      MAB1 scores  S1^T[k, tok] per head via lhsT=G1_h, rhs=Z^T (4-chunk tiles)
      MAB1 num/den via lhsT=E1^T_h, rhs=[Vh1_h | 1]
  * Z^T lives in SBUF as [128, 512] group tiles so MAB1 score matmuls stream
    512 tokens per instruction.
  * ACT does exp only; biases are all zero in practice (trace-time fallbacks
    emit extra ones-row matmuls / bias adds when they are not).
"""

import math

import numpy as np

import concourse.bacc as bacc
import concourse.bass as bass
import concourse.mybir as mybir
import concourse.tile as tile
from concourse.bass_utils import run_bass_kernel_spmd
from concourse.masks import make_identity

F32 = mybir.dt.float32
BF16 = mybir.dt.bfloat16
AF = mybir.ActivationFunctionType
OP = mybir.AluOpType

B, S, D = 16, 256, 41
L = S * D                      # 10496
LATENT, NREF, NLAYERS, NHEADS = 128, 128, 3, 4
DH = LATENT // NHEADS          # 32
SCALE = 1.0 / math.sqrt(LATENT)
NCORES = 8
BPC = B // NCORES              # 2
CMAX = L // 128                # 82
NEGBIG = -30.0                 # exp(-30) ~ 1e-13: masked-key contribution
GW = 4                         # chunks per Z^T group tile

# set by test harness to capture profiling info
TRACE = False
LAST_RESULT = None

_PROG_CACHE: dict = {}


def _build_program(NT: int, flags: dict, nlayers: int = NLAYERS):
    NTP = NT * 128
    NG = -(-NT // GW)          # number of Z^T group tiles
    nc = bacc.Bacc("TRN2")

    def gslice(c):
        """(group index, column slice within the group tile) for chunk c."""
        return c // GW, slice((c % GW) * 128, (c % GW) * 128 + 128)

    # ---------------- DRAM I/O ----------------
    d_cidx = nc.dram_tensor("cidx", [BPC, NTP], F32, kind="ExternalInput")
    d_tmk = nc.dram_tensor("tmk", [BPC, NTP], F32, kind="ExternalInput")
    d_umk = nc.dram_tensor("umk", [BPC, NTP], F32, kind="ExternalInput")
    d_mk = nc.dram_tensor("mk", [BPC, NTP], F32, kind="ExternalInput")
    d_wtab = nc.dram_tensor("wtab", [43, LATENT], F32, kind="ExternalInput")
    d_g0 = nc.dram_tensor("g0", [NLAYERS, LATENT, 512], BF16, kind="ExternalInput")
    d_qh0 = nc.dram_tensor("qh0", [NLAYERS, NREF, LATENT], F32, kind="ExternalInput")
    WNAMES = ["wv0", "wo0", "wq1", "wq1t", "wk1", "wv1", "wo1"]
    d_w = {
        n: nc.dram_tensor(n, [NLAYERS, LATENT, LATENT], BF16, kind="ExternalInput")
        for n in WNAMES
    }
    BNAMES = ["r0", "bv0", "bo0", "bq1", "bk1", "bv1", "bo1"]
    d_b = {
        n: nc.dram_tensor(n, [NLAYERS, 512 if n == "r0" else LATENT], F32,
                          kind="ExternalInput")
        for n in BNAMES
    }
    d_zout = nc.dram_tensor("zout", [BPC, L, LATENT], F32, kind="ExternalOutput")
    d_mkout = nc.dram_tensor("mkout", [BPC, L], F32, kind="ExternalOutput")

    with tile.TileContext(nc) as tc:
        with (
            tc.tile_pool(name="persist", bufs=1) as pp,
            tc.tile_pool(name="work", bufs=3) as wp,
            tc.tile_pool(name="stream", bufs=8) as sp,
            tc.tile_pool(name="ps_sc", bufs=2, space="PSUM") as ps_sc,
            tc.tile_pool(name="ps_n1", bufs=2, space="PSUM") as ps_n1,
            tc.tile_pool(name="ps_sm", bufs=4, space="PSUM") as ps_sm,
        ):
            # ---------------- constants & weights ----------------
            ident = pp.tile([128, 128], F32, name="ident")
            make_identity(nc, ident[:])

            iota_i = pp.tile([41, 1], mybir.dt.int32, name="iota_i")
            nc.gpsimd.iota(iota_i[:], [[1, 1]], channel_multiplier=1)
            iota_f = pp.tile([41, 1], F32, name="iota_f")
            nc.vector.tensor_copy(iota_f[:], iota_i[:])

            ones_row = pp.tile([1, 128], F32, name="ones_row")
            nc.vector.memset(ones_row[:], 1.0)
            zerot = pp.tile([128, 512], F32, name="zerot")
            nc.vector.memset(zerot[:], 0.0)

            wtab_s = pp.tile([43, LATENT], F32, name="wtab_s")
            nc.sync.dma_start(wtab_s[:], d_wtab[:, :])

            g0_s, qh0_s = [], []
            w_s = {n: [] for n in WNAMES}
            b_s = {n: [] for n in BNAMES}
            for l in range(NLAYERS):
                g = pp.tile([LATENT, 512], BF16, name=f"g0s{l}", tag=f"g0s{l}")
                nc.sync.dma_start(g[:], d_g0[l, :, :])
                g0_s.append(g)
                q = pp.tile([NREF, LATENT], F32, name=f"qh0s{l}", tag=f"qh0s{l}")
                nc.sync.dma_start(q[:], d_qh0[l, :, :])
                qh0_s.append(q)
                for n in WNAMES:
                    if n == "wq1t":
                        # per-head [32,128] tiles (PE weights must start at
                        # partition 0/32/64, so a [96:128] slice is illegal)
                        hh_tiles = []
                        for h in range(NHEADS):
                            t = pp.tile([DH, LATENT], BF16, name=f"wq1t{l}h{h}",
                                        tag=f"wq1t{l}h{h}")
                            nc.sync.dma_start(
                                t[:], d_w[n][l, h * DH:(h + 1) * DH, :])
                            hh_tiles.append(t)
                        w_s[n].append(hh_tiles)
                        continue
                    t = pp.tile([LATENT, LATENT], BF16, name=f"{n}s{l}", tag=f"{n}s{l}")
                    nc.sync.dma_start(t[:], d_w[n][l, :, :])
                    w_s[n].append(t)
                for n in BNAMES:
                    if not flags[n]:
                        b_s[n].append(None)
                        continue
                    if n in ("bk1",):        # needed as a [128,1] column
                        t = pp.tile([LATENT, 1], F32, name=f"{n}s{l}", tag=f"{n}s{l}")
                        nc.sync.dma_start(
                            t[:], bass.AP(d_b[n], l * LATENT, [[1, LATENT], [1, 1]]))
                    elif n == "bq1":         # per-head column tiles [32,1]
                        t = []
                        for h in range(NHEADS):
                            th = pp.tile([DH, 1], F32, name=f"{n}c{l}h{h}",
                                         tag=f"{n}c{l}h{h}")
                            nc.sync.dma_start(
                                th[:], bass.AP(d_b[n], l * LATENT + h * DH,
                                               [[1, DH], [1, 1]]))
                            t.append(th)
                    else:
                        w = 512 if n == "r0" else LATENT
                        t = pp.tile([1, w], F32, name=f"{n}s{l}", tag=f"{n}s{l}")
                        nc.sync.dma_start(t[:], d_b[n][l:l + 1, :])
                    b_s[n].append(t)
                if flags["bq1"]:  # row form for the ones-matmul into Qh1
                    t = pp.tile([1, LATENT], F32, name=f"bq1rs{l}", tag=f"bq1rs{l}")
                    nc.sync.dma_start(t[:], d_b["bq1"][l:l + 1, :])
                    b_s.setdefault("bq1r", []).append(t)

            # ---------------- per-batch setup + Z0 ----------------
            mkp_s, mkneg_s, mkt_s, ZT, ZTB = [], [], [], [], []
            for b in range(BPC):
                mkt = pp.tile([NT, 128], F32, name=f"mkt{b}", tag=f"mkt{b}")
                nc.sync.dma_start(mkt[:], bass.AP(d_mk, b * NTP, [[128, NT], [1, 128]]))
                mkt_s.append(mkt)
                mkpp = ps_sm.tile([128, NT], F32, name="mkpp", tag="sm")
                nc.tensor.transpose(mkpp[:], mkt[:], ident[0:NT, 0:NT])
                mkp = pp.tile([128, NT], F32, name=f"mkp{b}", tag=f"mkp{b}")
                nc.vector.tensor_copy(mkp[:], mkpp[:])
                mkp_s.append(mkp)
                mkneg = pp.tile([128, NT], F32, name=f"mkneg{b}", tag=f"mkneg{b}")
                nc.vector.tensor_scalar(
                    mkneg[:], mkp[:], -1.0, -NEGBIG, op0=OP.add, op1=OP.mult)
                mkneg_s.append(mkneg)

                xt = pp.tile([43, NTP], F32, name=f"xt{b}", tag="xt")
                crow = pp.tile([1, NTP], F32, name=f"crow{b}", tag="crow")
                nc.sync.dma_start(crow[:], d_cidx[b:b + 1, :])
                # replicate cidx row across 41 partitions via a K=1 matmul,
                # then one-hot it against the iota column
                for j in range(0, NTP, 512):
                    w = min(512, NTP - j)
                    cb = ps_sm.tile([41, 512], F32, name="cb", tag="sm")
                    nc.tensor.matmul(cb[:, :w], lhsT=ones_row[:, 0:41],
                                     rhs=crow[:, j:j + w], start=True, stop=True)
                    nc.vector.tensor_scalar(
                        xt[0:41, j:j + w], cb[:, :w], iota_f[:], None,
                        op0=OP.is_equal)
                nc.sync.dma_start(xt[41:42, :], d_tmk[b:b + 1, :])
                nc.sync.dma_start(xt[42:43, :], d_umk[b:b + 1, :])

                ztg, ztbg = [], []
                for g in range(NG):
                    zt = pp.tile([128, GW * 128], F32, name=f"zt{b}_{g}",
                                 tag=f"zt{b}_{g}")
                    ztg.append(zt)
                    ztb = pp.tile([128, GW * 128], BF16, name=f"ztb{b}_{g}",
                                  tag=f"ztb{b}_{g}")
                    ztbg.append(ztb)
                for c in range(NT):
                    g, js = gslice(c)
                    z0p = ps_sm.tile([128, 128], F32, name="z0p", tag="sm")
                    nc.tensor.matmul(
                        z0p[:], lhsT=wtab_s[:], rhs=xt[:, c * 128:(c + 1) * 128],
                        start=True, stop=True)
                    nc.vector.tensor_scalar_max(ztg[g][:, js], z0p[:], 0.0)
                    nc.gpsimd.tensor_copy(ztbg[g][:, js], ztg[g][:, js])
                ZT.append(ztg)
                ZTB.append(ztbg)

            # ---------------- layers ----------------
            for l in range(nlayers):
                num0_b, rd0_b, o0_b, hh_b, ht_b = {}, {}, {}, {}, {}
                kh1_b, g1_b, vo1_b, r1b_b = {}, {}, {}, {}
                for b in range(BPC):
                    num0 = wp.tile([128, 132], F32, name="num0", tag=f"num0{b}")
                    nc.vector.memset(num0[:], 0.0)
                    num0_b[b] = num0
                # ===== MAB0: induced points attend to data =====
                for c in range(NT):
                    for b in range(BPC):
                        ztg, ztbg = ZT[b], ZTB[b]
                        num0 = num0_b[b]
                        g, js = gslice(c)
                        s0 = ps_sc.tile([128, 512], F32, name="s0", tag="sc")
                        if flags["r0"]:
                            nc.tensor.matmul(s0[:], lhsT=ones_row[:],
                                             rhs=b_s["r0"][l][:],
                                             start=True, stop=False)
                        nc.tensor.matmul(s0[:], lhsT=ztbg[g][:, js], rhs=g0_s[l][:],
                                         start=not flags["r0"], stop=True)
                        et = sp.tile([128, 512], BF16, name="et", tag="et")
                        nc.scalar.activation(et[:], s0[:], AF.Exp,
                                             bias=mkneg_s[b][:, c:c + 1],
                                             scale=SCALE)
                        vh = ps_sm.tile([128, 128], F32, name="vh", tag="sm")
                        if flags["bv0"]:
                            nc.tensor.matmul(vh[:], lhsT=ones_row[:],
                                             rhs=b_s["bv0"][l][:],
                                             start=True, stop=False)
                        nc.tensor.matmul(vh[:], lhsT=ztbg[g][:, js],
                                         rhs=w_s["wv0"][l][:],
                                         start=not flags["bv0"], stop=True)
                        vo = sp.tile([128, 132], BF16, name="vo", tag="vo")
                        nc.gpsimd.memset(vo[:], 1.0)
                        nc.vector.tensor_copy(
                            vo[:].rearrange("p (h x) -> p h x", x=33)[:, :, 0:32],
                            vh[:].rearrange("p (h x) -> p h x", x=32))
                        n0c = ps_n1.tile([128, 132], F32, name="n0c", tag="nacc")
                        for h in range(NHEADS):
                            nc.tensor.matmul(
                                n0c[:, h * 33:(h + 1) * 33],
                                lhsT=et[:, h * 128:(h + 1) * 128],
                                rhs=vo[:, h * 33:(h + 1) * 33],
                                start=True, stop=True)
                        nc.vector.tensor_add(num0[:], num0[:], n0c[:])
                # ===== MAB0 tail + MAB1 prep (per batch) =====
                for b in range(BPC):
                    ztg = ZT[b]
                    num0 = num0_b[b]
                    rd0 = sp.tile([128, 4], F32, name="rd0", tag="rd0")
                    nc.vector.reciprocal(
                        rd0[:].rearrange("p (h x) -> p h x", x=1),
                        num0[:].rearrange("p (h x) -> p h x", x=33)[:, :, 32:33])
                    o0 = wp.tile([128, 128], F32, name="o0", tag=f"o0{b}")
                    nc.vector.tensor_tensor(
                        o0[:].rearrange("p (h x) -> p h x", x=32),
                        num0[:].rearrange("p (h x) -> p h x", x=33)[:, :, 0:32],
                        rd0[:].rearrange("p (h x) -> p h x", x=1).to_broadcast(
                            [128, 4, 32]),
                        op=OP.mult)
                    nc.vector.tensor_add(o0[:], o0[:], qh0_s[l][:])
                    o0tp = ps_sm.tile([128, 128], F32, name="o0tp", tag="sm")
                    nc.tensor.transpose(o0tp[:], o0[:], ident[:])
                    o0t = wp.tile([128, 128], BF16, name="o0t", tag=f"o0t{b}")
                    nc.vector.tensor_copy(o0t[:], o0tp[:])
                    fc0 = ps_sm.tile([128, 128], F32, name="fc0", tag="sm")
                    if flags["bo0"]:
                        nc.tensor.matmul(fc0[:], lhsT=ones_row[:],
                                         rhs=b_s["bo0"][l][:], start=True, stop=False)
                    nc.tensor.matmul(fc0[:], lhsT=o0t[:], rhs=w_s["wo0"][l][:],
                                     start=not flags["bo0"], stop=True)
                    hh = wp.tile([128, 128], F32, name="hh", tag=f"hh{b}")
                    nc.vector.scalar_tensor_tensor(
                        hh[:], in0=fc0[:], scalar=0.0, in1=o0[:],
                        op0=OP.max, op1=OP.add)
                    htp = ps_sm.tile([128, 128], F32, name="htp", tag="sm")
                    nc.tensor.transpose(htp[:], hh[:], ident[:])
                    ht = wp.tile([128, 128], BF16, name="ht", tag=f"ht{b}")
                    nc.vector.tensor_copy(ht[:], htp[:])

                    # ===== MAB1 prep =====
                    kh1p = ps_sm.tile([128, 128], F32, name="kh1p", tag="sm")
                    nc.tensor.matmul(kh1p[:], lhsT=w_s["wk1"][l][:], rhs=ht[:],
                                     start=True, stop=True)
                    kh1 = wp.tile([128, 128], BF16, name="kh1", tag=f"kh1{b}")
                    if flags["bk1"]:
                        nc.vector.tensor_scalar_add(kh1[:], kh1p[:], b_s["bk1"][l][:])
                    else:
                        nc.vector.tensor_copy(kh1[:], kh1p[:])
                    kh1h = []
                    for h in range(NHEADS):
                        t = wp.tile([DH, LATENT], BF16, name=f"kh1h{h}", tag=f"kh1h{h}")
                        nc.sync.dma_start(t[:], kh1[h * DH:(h + 1) * DH, :])
                        kh1h.append(t)
                    g1p = ps_sc.tile([128, 512], F32, name="g1p", tag="sc")
                    for h in range(NHEADS):
                        nc.tensor.matmul(g1p[:, h * 128:(h + 1) * 128],
                                         lhsT=w_s["wq1t"][l][h][:], rhs=kh1h[h][:],
                                         start=True, stop=True)
                    g1 = wp.tile([128, 512], BF16, name="g1", tag=f"g1{b}")
                    nc.vector.tensor_copy(g1[:], g1p[:])
                    vh1p = ps_sm.tile([128, 128], F32, name="vh1p", tag="sm")
                    if flags["bv1"]:
                        nc.tensor.matmul(vh1p[:], lhsT=ones_row[:],
                                         rhs=b_s["bv1"][l][:], start=True, stop=False)
                    nc.tensor.matmul(vh1p[:], lhsT=ht[:], rhs=w_s["wv1"][l][:],
                                     start=not flags["bv1"], stop=True)
                    vo1 = wp.tile([128, 132], BF16, name="vo1", tag=f"vo1{b}")
                    nc.vector.memset(vo1[:], 1.0)
                    nc.vector.tensor_copy(
                        vo1[:].rearrange("p (h x) -> p h x", x=33)[:, :, 0:32],
                        vh1p[:].rearrange("p (h x) -> p h x", x=32))
                    r1b = None
                    if flags["bq1"]:
                        r1bp = ps_sm.tile([128, 4], F32, name="r1bp", tag="sm")
                        for h in range(NHEADS):
                            nc.tensor.matmul(r1bp[:, h:h + 1], lhsT=kh1h[h][:],
                                             rhs=b_s["bq1"][l][h][:],
                                             start=True, stop=True)
                        r1b = wp.tile([128, 4], F32, name="r1b", tag=f"r1b{b}")
                        nc.vector.tensor_scalar_mul(r1b[:], r1bp[:], SCALE)
                    g1_b[b], vo1_b[b], r1b_b[b] = g1, vo1, r1b

                # ===== MAB1 chunks: data attends to induced (b-interleaved) ==
                for g in range(NG):
                    gw = min(GW * 128, NTP - g * GW * 128)
                    for b in range(BPC):
                        ztg, ztbg = ZT[b], ZTB[b]
                        g1, vo1, r1b = g1_b[b], vo1_b[b], r1b_b[b]
                        e1h = []
                        for h in range(NHEADS):
                            s1 = ps_sc.tile([128, GW * 128], F32, name="s1", tag="sc")
                            nc.tensor.matmul(s1[:, :gw],
                                             lhsT=g1[:, h * 128:(h + 1) * 128],
                                             rhs=ztbg[g][:, :gw],
                                             start=True, stop=True)
                            e1 = sp.tile([128, GW * 128], BF16, name="e1", tag="e1")
                            if flags["bq1"]:
                                nc.scalar.activation(e1[:, :gw], s1[:, :gw], AF.Exp,
                                                     bias=r1b[:, h:h + 1], scale=SCALE)
                            else:
                                nc.scalar.activation(e1[:, :gw], s1[:, :gw], AF.Exp,
                                                     scale=SCALE)
                            e1h.append(e1)
                        for j in range(gw // 128):
                            c = g * GW + j
                            js = slice(j * 128, (j + 1) * 128)
                            num1 = ps_n1.tile([128, 132], F32, name="num1", tag="nacc")
                            for h in range(NHEADS):
                                nc.tensor.matmul(num1[:, h * 33:(h + 1) * 33],
                                                 lhsT=e1h[h][:, js],
                                                 rhs=vo1[:, h * 33:(h + 1) * 33],
                                                 start=True, stop=True)
                            qh1 = ps_sm.tile([128, 128], F32, name="qh1", tag="sm")
                            if flags["bq1"]:
                                nc.tensor.matmul(qh1[:], lhsT=ones_row[:],
                                                 rhs=b_s["bq1r"][l][:],
                                                 start=True, stop=False)
                            nc.tensor.matmul(qh1[:], lhsT=ztbg[g][:, js],
                                             rhs=w_s["wq1"][l][:],
                                             start=not flags["bq1"], stop=True)
                            rd1 = sp.tile([128, 4], F32, name="rd1", tag="rd1")
                            nc.vector.reciprocal(
                                rd1[:].rearrange("p (h x) -> p h x", x=1),
                                num1[:].rearrange("p (h x) -> p h x", x=33)[:, :, 32:33])
                            o1 = sp.tile([128, 128], F32, name="o1", tag="o1")
                            if not flags["bo1"]:
                                # masked-O1 path: invalid rows are exactly
                                # zeroed through fc_o since bo1 == 0
                                nc.vector.tensor_scalar_mul(
                                    rd1[:], rd1[:], mkp_s[b][:, c:c + 1])
                            nc.vector.tensor_tensor(
                                o1[:].rearrange("p (h x) -> p h x", x=32),
                                num1[:].rearrange("p (h x) -> p h x", x=33)[:, :, 0:32],
                                rd1[:].rearrange("p (h x) -> p h x", x=1).to_broadcast(
                                    [128, 4, 32]),
                                op=OP.mult)
                            if flags["bo1"]:
                                nc.vector.tensor_add(o1[:], o1[:], qh1[:])
                            else:
                                nc.vector.scalar_tensor_tensor(
                                    o1[:], in0=qh1[:], scalar=mkp_s[b][:, c:c + 1],
                                    in1=o1[:], op0=OP.mult, op1=OP.add)
                            o1tp = ps_sm.tile([128, 128], F32, name="o1tp", tag="sm")
                            nc.tensor.transpose(o1tp[:], o1[:], ident[:])
                            o1t = sp.tile([128, 128], BF16, name="o1t", tag="o1t")
                            nc.scalar.copy(o1t[:], o1tp[:])
                            fc1 = ps_sm.tile([128, 128], F32, name="fc1", tag="sm")
                            if flags["bo1"]:
                                nc.tensor.matmul(fc1[:], lhsT=ones_row[:],
                                                 rhs=b_s["bo1"][l][:],
                                                 start=True, stop=False)
                            nc.tensor.matmul(fc1[:], lhsT=o1t[:], rhs=w_s["wo1"][l][:],
                                             start=not flags["bo1"], stop=True)
                            u = sp.tile([128, 128], F32, name="u", tag="u")
                            nc.vector.scalar_tensor_tensor(
                                u[:], in0=fc1[:], scalar=0.0, in1=o1[:],
                                op0=OP.max, op1=OP.add)
                            if flags["bo1"]:
                                nc.vector.tensor_scalar_mul(u[:], u[:],
                                                            mkp_s[b][:, c:c + 1])
                            utp = ps_sm.tile([128, 128], F32, name="utp", tag="sm")
                            nc.tensor.transpose(utp[:], u[:], ident[:])
                            nc.vector.tensor_add(ztg[g][:, js], ztg[g][:, js], utp[:])
                            nc.gpsimd.tensor_copy(ztbg[g][:, js], ztg[g][:, js])

            # ---------------- output ----------------
            for c in range(NT):
                for b in range(BPC):
                    g, js = gslice(c)
                    zp = ps_sm.tile([128, 128], F32, name="zp", tag="sm")
                    nc.tensor.transpose(zp[:], ZT[b][g][:, js], ident[:])
                    zo = sp.tile([128, 128], F32, name="zo", tag="zo")
                    nc.vector.tensor_copy(zo[:], zp[:])
                    nc.sync.dma_start(
                        bass.AP(d_zout, (b * L + c * 128) * LATENT,
                                [[LATENT, 128], [1, LATENT]]),
                        zo[:])
            for b in range(BPC):
                # zero tail rows [NTP, L): 2KB-contiguous descriptor runs
                r = NTP
                while r < L:
                    n = min(512, L - r)   # rows; n*128 elems; dst stays contiguous
                    nelem = n * LATENT
                    inner = nelem // 128
                    nc.sync.dma_start(
                        bass.AP(d_zout, (b * L + r) * LATENT,
                                [[inner, 128], [1, inner]]),
                        zerot[:, :inner])
                    r += n
                # mask output (contiguous 512B rows from the token-major tile)
                nc.sync.dma_start(
                    bass.AP(d_mkout, b * L, [[128, NT], [1, 128]]), mkt_s[b][:])
                if CMAX > NT:
                    nc.sync.dma_start(
                        bass.AP(d_mkout, b * L + NTP, [[128, CMAX - NT], [1, 128]]),
                        zerot[0:CMAX - NT, 0:128])
    nc.compile()
    return nc


def _prep(inputs):
    """Host-side prep: compaction indices + weight folding (all O(small))."""
    time_x = np.ascontiguousarray(np.asarray(inputs["time_x"], np.float32))
    value_x = np.ascontiguousarray(np.asarray(inputs["value_x"], np.float32))
    mask_x = np.asarray(inputs["mask_x"])
    Wi = np.asarray(inputs["Wi"], np.float32)
    bi = np.asarray(inputs["bi"], np.float32)
    I = np.asarray(inputs["I"], np.float32)
    Wq = np.asarray(inputs["Wq"], np.float32)
    bq = np.asarray(inputs["bq"], np.float32)
    Wk = np.asarray(inputs["Wk"], np.float32)
    bk = np.asarray(inputs["bk"], np.float32)
    Wv = np.asarray(inputs["Wv"], np.float32)
    bv = np.asarray(inputs["bv"], np.float32)
    Wo = np.asarray(inputs["Wo"], np.float32)
    bo = np.asarray(inputs["bo"], np.float32)

    mflat = mask_x.reshape(B, L)
    order = np.argsort(1 - mflat, axis=1, kind="stable")
    nvalid = int(mflat.sum(axis=1).max())
    NT = max(1, min(CMAX, -(-nvalid // 128)))
    NTP = NT * 128

    ordp = order[:, :NTP]
    mkc = np.take_along_axis(mflat, ordp, axis=1).astype(np.float32)
    tfull = np.broadcast_to(time_x[:, :, None], (B, S, D)).reshape(B, L)
    tmk = np.take_along_axis(tfull, ordp, axis=1) * mkc
    umk = np.take_along_axis(value_x.reshape(B, L), ordp, axis=1) * mkc
    cidx = np.where(mkc > 0, (ordp % D).astype(np.float32), 63.0).astype(np.float32)

    wtab = np.concatenate([Wi[:D] + bi[None, :], Wi[D:D + 2]], 0)  # [43,128]

    g0 = np.zeros((NLAYERS, LATENT, 512), np.float32)
    qh0 = np.zeros((NLAYERS, NREF, LATENT), np.float32)
    r0 = np.zeros((NLAYERS, 512), np.float32)
    for l in range(NLAYERS):
        Q = I[l] @ Wq[l, 0] + bq[l, 0]
        qh0[l] = Q
        for h in range(NHEADS):
            hs = slice(h * DH, (h + 1) * DH)
            g0[l][:, h * NREF:(h + 1) * NREF] = Wk[l, 0][:, hs] @ Q[:, hs].T
            r0[l][h * NREF:(h + 1) * NREF] = bk[l, 0][hs] @ Q[:, hs].T

    w = dict(
        wv0=Wv[:, 0], wo0=Wo[:, 0], wq1=Wq[:, 1],
        wq1t=np.ascontiguousarray(Wq[:, 1].transpose(0, 2, 1)),
        wk1=Wk[:, 1], wv1=Wv[:, 1], wo1=Wo[:, 1],
    )
    bvec = dict(r0=r0, bv0=bv[:, 0], bo0=bo[:, 0], bq1=bq[:, 1],
                bk1=bk[:, 1], bv1=bv[:, 1], bo1=bo[:, 1])
    flags = {n: bool(np.any(v != 0)) for n, v in bvec.items()}
    return dict(NT=NT, NTP=NTP, tmk=tmk, umk=umk, mkc=mkc, cidx=cidx,
                wtab=wtab, g0=g0, qh0=qh0, w=w, bvec=bvec, flags=flags)


def kernel(**inputs):
    global LAST_RESULT
    p = _prep(inputs)

    key = (p["NT"], tuple(sorted(p["flags"].items())))
    if key not in _PROG_CACHE:
        _PROG_CACHE[key] = _build_program(p["NT"], p["flags"])
    nc = _PROG_CACHE[key]

    import ml_dtypes
    bf16 = ml_dtypes.bfloat16
    shared = dict(wtab=np.ascontiguousarray(p["wtab"]),
                  g0=np.ascontiguousarray(p["g0"].astype(bf16)), qh0=p["qh0"])
    for n, v in p["w"].items():
        shared[n] = np.ascontiguousarray(v.astype(bf16))
    for n, v in p["bvec"].items():
        shared[n] = np.ascontiguousarray(v)

    in_maps = []
    for m in range(NCORES):
        sl = slice(m * BPC, (m + 1) * BPC)
        im = dict(shared)
        im["cidx"] = np.ascontiguousarray(p["cidx"][sl])
        im["tmk"] = np.ascontiguousarray(p["tmk"][sl])
        im["umk"] = np.ascontiguousarray(p["umk"][sl])
        im["mk"] = np.ascontiguousarray(p["mkc"][sl])
        in_maps.append(im)

    res = run_bass_kernel_spmd(nc, in_maps, core_ids=list(range(NCORES)),
                               trace=TRACE)
    LAST_RESULT = res

    Z = np.concatenate([r["zout"] for r in res.results], axis=0)
    mk = np.concatenate([r["mkout"] for r in res.results], axis=0)[..., None]
    return Z.reshape(B, L, LATENT), mk.reshape(B, L, 1)


# revision 26
# speedup vs baseline: 2.3987x; 1.2789x over previous
"""Trainium2 Bass kernel for the masked set-transformer encoder (ISAB stack).

Strategy (pure data parallel, B=16 over 8 cores, 2 batch elements/core):
  * The compaction permutation commutes with the whole network: softmax over
    keys is permutation invariant, everything else is row-wise, and masked
    rows are exactly zero throughout.  So the host only computes the stable
    argsort *indices*; the device processes tokens in compacted order and the
    output is already compacted (zero tail appended on device).
  * Only NT = ceil(max_b nvalid_b / 128) tiles of 128 tokens are processed
    (~42 instead of 82 for random masks).  NT is a compile-time constant
    derived from the actual mask; the program is recompiled if it changes.
  * The one-hot input FF collapses to a [43,128] table matmul against a
    device-built X^T = [one_hot(c); t*mk; u*mk] (no gathers: one-hot rows are
    built with an is_equal against an iota column; invalid tokens get an
    out-of-range channel id so their X^T column is exactly zero).
  * Attention layouts keep softmax reductions on natural axes:
      MAB0 scores  S^T[tok,(h,q)] = Z @ G0,  G0 = fold(Wk, I@Wq+bq)  (host)
      MAB0 key masking is folded into the exp bias: exp(s*scale + (mk-1)*30)
      MAB0 num/den via lhsT=E^T_h, rhs=[Vh_h | 1], DVE-accumulated over chunks
      MAB1 scores  S1^T[k, tok] per head via lhsT=G1_h, rhs=Z^T (4-chunk tiles)
      MAB1 num/den via lhsT=E1^T_h, rhs=[Vh1_h | 1]
  * Z^T lives in SBUF as [128, 512] group tiles so MAB1 score matmuls stream
    512 tokens per instruction.
  * ACT does exp only; biases are all zero in practice (trace-time fallbacks
    emit extra ones-row matmuls / bias adds when they are not).
"""

import math

import numpy as np

import concourse.bacc as bacc
import concourse.bass as bass
import concourse.mybir as mybir
import concourse.tile as tile
from concourse.bass_utils import run_bass_kernel_spmd
from concourse.masks import make_identity

F32 = mybir.dt.float32
BF16 = mybir.dt.bfloat16
AF = mybir.ActivationFunctionType
OP = mybir.AluOpType

B, S, D = 16, 256, 41
L = S * D                      # 10496
LATENT, NREF, NLAYERS, NHEADS = 128, 128, 3, 4
DH = LATENT // NHEADS          # 32
SCALE = 1.0 / math.sqrt(LATENT)
NCORES = 8
BPC = B // NCORES              # 2
CMAX = L // 128                # 82
NEGBIG = -30.0                 # exp(-30) ~ 1e-13: masked-key contribution
GW = 4                         # chunks per Z^T group tile

# set by test harness to capture profiling info
TRACE = False
LAST_RESULT = None

_PROG_CACHE: dict = {}


def _build_program(NT: int, flags: dict, nlayers: int = NLAYERS):
    NTP = NT * 128
    NG = -(-NT // GW)          # number of Z^T group tiles
    nc = bacc.Bacc("TRN2")

    def gslice(c):
        """(group index, column slice within the group tile) for chunk c."""
        return c // GW, slice((c % GW) * 128, (c % GW) * 128 + 128)

    # ---------------- DRAM I/O ----------------
    d_cidx = nc.dram_tensor("cidx", [BPC, NTP], F32, kind="ExternalInput")
    d_tmk = nc.dram_tensor("tmk", [BPC, NTP], F32, kind="ExternalInput")
    d_umk = nc.dram_tensor("umk", [BPC, NTP], F32, kind="ExternalInput")
    d_mk = nc.dram_tensor("mk", [BPC, NTP], F32, kind="ExternalInput")
    d_wtab = nc.dram_tensor("wtab", [43, LATENT], F32, kind="ExternalInput")
    d_g0 = nc.dram_tensor("g0", [NLAYERS, LATENT, 512], BF16, kind="ExternalInput")
    d_qh0 = nc.dram_tensor("qh0", [NLAYERS, NREF, LATENT], F32, kind="ExternalInput")
    WNAMES = ["wv0", "wo0", "wq1", "wq1t", "wk1", "wv1", "wo1"]
    d_w = {
        n: nc.dram_tensor(n, [NLAYERS, LATENT, LATENT], BF16, kind="ExternalInput")
        for n in WNAMES
    }
    BNAMES = ["r0", "bv0", "bo0", "bq1", "bk1", "bv1", "bo1"]
    d_b = {
        n: nc.dram_tensor(n, [NLAYERS, 512 if n == "r0" else LATENT], F32,
                          kind="ExternalInput")
        for n in BNAMES
    }
    d_zout = nc.dram_tensor("zout", [BPC, L, LATENT], F32, kind="ExternalOutput")
    d_mkout = nc.dram_tensor("mkout", [BPC, L], F32, kind="ExternalOutput")

    with tile.TileContext(nc) as tc:
        with (
            tc.tile_pool(name="persist", bufs=1) as pp,
            tc.tile_pool(name="work", bufs=3) as wp,
            tc.tile_pool(name="stream", bufs=8) as sp,
            tc.tile_pool(name="ps_sc", bufs=2, space="PSUM") as ps_sc,
            tc.tile_pool(name="ps_n1", bufs=2, space="PSUM") as ps_n1,
            tc.tile_pool(name="ps_sm", bufs=4, space="PSUM") as ps_sm,
        ):
            # ---------------- constants & weights ----------------
            ident = pp.tile([128, 128], F32, name="ident")
            make_identity(nc, ident[:])

            iota_i = pp.tile([41, 1], mybir.dt.int32, name="iota_i")
            nc.gpsimd.iota(iota_i[:], [[1, 1]], channel_multiplier=1)
            iota_f = pp.tile([41, 1], F32, name="iota_f")
            nc.vector.tensor_copy(iota_f[:], iota_i[:])

            ones_row = pp.tile([1, 128], F32, name="ones_row")
            nc.vector.memset(ones_row[:], 1.0)
            zerot = pp.tile([128, 512], F32, name="zerot")
            nc.vector.memset(zerot[:], 0.0)

            wtab_s = pp.tile([43, LATENT], F32, name="wtab_s")
            nc.sync.dma_start(wtab_s[:], d_wtab[:, :])

            g0_s, qh0_s = [], []
            w_s = {n: [] for n in WNAMES}
            b_s = {n: [] for n in BNAMES}
            for l in range(NLAYERS):
                g = pp.tile([LATENT, 512], BF16, name=f"g0s{l}", tag=f"g0s{l}")
                nc.sync.dma_start(g[:], d_g0[l, :, :])
                g0_s.append(g)
                q = pp.tile([NREF, LATENT], F32, name=f"qh0s{l}", tag=f"qh0s{l}")
                nc.sync.dma_start(q[:], d_qh0[l, :, :])
                qh0_s.append(q)
                for n in WNAMES:
                    if n == "wq1t":
                        # per-head [32,128] tiles (PE weights must start at
                        # partition 0/32/64, so a [96:128] slice is illegal)
                        hh_tiles = []
                        for h in range(NHEADS):
                            t = pp.tile([DH, LATENT], BF16, name=f"wq1t{l}h{h}",
                                        tag=f"wq1t{l}h{h}")
                            nc.sync.dma_start(
                                t[:], d_w[n][l, h * DH:(h + 1) * DH, :])
                            hh_tiles.append(t)
                        w_s[n].append(hh_tiles)
                        continue
                    t = pp.tile([LATENT, LATENT], BF16, name=f"{n}s{l}", tag=f"{n}s{l}")
                    nc.sync.dma_start(t[:], d_w[n][l, :, :])
                    w_s[n].append(t)
                for n in BNAMES:
                    if not flags[n]:
                        b_s[n].append(None)
                        continue
                    if n in ("bk1",):        # needed as a [128,1] column
                        t = pp.tile([LATENT, 1], F32, name=f"{n}s{l}", tag=f"{n}s{l}")
                        nc.sync.dma_start(
                            t[:], bass.AP(d_b[n], l * LATENT, [[1, LATENT], [1, 1]]))
                    elif n == "bq1":         # per-head column tiles [32,1]
                        t = []
                        for h in range(NHEADS):
                            th = pp.tile([DH, 1], F32, name=f"{n}c{l}h{h}",
                                         tag=f"{n}c{l}h{h}")
                            nc.sync.dma_start(
                                th[:], bass.AP(d_b[n], l * LATENT + h * DH,
                                               [[1, DH], [1, 1]]))
                            t.append(th)
                    else:
                        w = 512 if n == "r0" else LATENT
                        t = pp.tile([1, w], F32, name=f"{n}s{l}", tag=f"{n}s{l}")
                        nc.sync.dma_start(t[:], d_b[n][l:l + 1, :])
                    b_s[n].append(t)
                if flags["bq1"]:  # row form for the ones-matmul into Qh1
                    t = pp.tile([1, LATENT], F32, name=f"bq1rs{l}", tag=f"bq1rs{l}")
                    nc.sync.dma_start(t[:], d_b["bq1"][l:l + 1, :])
                    b_s.setdefault("bq1r", []).append(t)

            # ---------------- per-batch setup + Z0 ----------------
            mkp_s, mkneg_s, mkt_s, ZT, ZTB = [], [], [], [], []
            for b in range(BPC):
                mkt = pp.tile([NT, 128], F32, name=f"mkt{b}", tag=f"mkt{b}")
                nc.sync.dma_start(mkt[:], bass.AP(d_mk, b * NTP, [[128, NT], [1, 128]]))
                mkt_s.append(mkt)
                mkpp = ps_sm.tile([128, NT], F32, name="mkpp", tag="sm")
                nc.tensor.transpose(mkpp[:], mkt[:], ident[0:NT, 0:NT])
                mkp = pp.tile([128, NT], F32, name=f"mkp{b}", tag=f"mkp{b}")
                nc.vector.tensor_copy(mkp[:], mkpp[:])
                mkp_s.append(mkp)
                mkneg = pp.tile([128, NT], F32, name=f"mkneg{b}", tag=f"mkneg{b}")
                nc.vector.tensor_scalar(
                    mkneg[:], mkp[:], -1.0, -NEGBIG, op0=OP.add, op1=OP.mult)
                mkneg_s.append(mkneg)

                xt = pp.tile([43, NTP], F32, name=f"xt{b}", tag="xt")
                crow = pp.tile([1, NTP], F32, name=f"crow{b}", tag="crow")
                nc.sync.dma_start(crow[:], d_cidx[b:b + 1, :])
                # replicate cidx row across 41 partitions via a K=1 matmul,
                # then one-hot it against the iota column
                for j in range(0, NTP, 512):
                    w = min(512, NTP - j)
                    cb = ps_sm.tile([41, 512], F32, name="cb", tag="sm")
                    nc.tensor.matmul(cb[:, :w], lhsT=ones_row[:, 0:41],
                                     rhs=crow[:, j:j + w], start=True, stop=True)
                    nc.vector.tensor_scalar(
                        xt[0:41, j:j + w], cb[:, :w], iota_f[:], None,
                        op0=OP.is_equal)
                nc.sync.dma_start(xt[41:42, :], d_tmk[b:b + 1, :])
                nc.sync.dma_start(xt[42:43, :], d_umk[b:b + 1, :])

                ztg, ztbg = [], []
                for g in range(NG):
                    zt = pp.tile([128, GW * 128], F32, name=f"zt{b}_{g}",
                                 tag=f"zt{b}_{g}")
                    ztg.append(zt)
                    ztb = pp.tile([128, GW * 128], BF16, name=f"ztb{b}_{g}",
                                  tag=f"ztb{b}_{g}")
                    ztbg.append(ztb)
                for c in range(NT):
                    g, js = gslice(c)
                    z0p = ps_sm.tile([128, 128], F32, name="z0p", tag="sm")
                    nc.tensor.matmul(
                        z0p[:], lhsT=wtab_s[:], rhs=xt[:, c * 128:(c + 1) * 128],
                        start=True, stop=True)
                    nc.vector.tensor_scalar_max(ztg[g][:, js], z0p[:], 0.0)
                    nc.gpsimd.tensor_copy(ztbg[g][:, js], ztg[g][:, js])
                ZT.append(ztg)
                ZTB.append(ztbg)

            # ---------------- layers ----------------
            for l in range(nlayers):
                num0_b, rd0_b, o0_b, hh_b, ht_b = {}, {}, {}, {}, {}
                kh1_b, g1_b, vo1_b, r1b_b = {}, {}, {}, {}
                for b in range(BPC):
                    num0 = wp.tile([128, 132], F32, name="num0", tag=f"num0{b}")
                    nc.vector.memset(num0[:], 0.0)
                    num0_b[b] = num0
                # ===== MAB0: induced points attend to data =====
                for c in range(NT):
                    for b in range(BPC):
                        ztg, ztbg = ZT[b], ZTB[b]
                        num0 = num0_b[b]
                        g, js = gslice(c)
                        s0 = ps_sc.tile([128, 512], F32, name="s0", tag="sc")
                        if flags["r0"]:
                            nc.tensor.matmul(s0[:], lhsT=ones_row[:],
                                             rhs=b_s["r0"][l][:],
                                             start=True, stop=False)
                        nc.tensor.matmul(s0[:], lhsT=ztbg[g][:, js], rhs=g0_s[l][:],
                                         start=not flags["r0"], stop=True)
                        et = sp.tile([128, 512], BF16, name="et", tag="et")
                        nc.scalar.activation(et[:], s0[:], AF.Exp,
                                             bias=mkneg_s[b][:, c:c + 1],
                                             scale=SCALE)
                        vh = ps_sm.tile([128, 128], F32, name="vh", tag="sm")
                        if flags["bv0"]:
                            nc.tensor.matmul(vh[:], lhsT=ones_row[:],
                                             rhs=b_s["bv0"][l][:],
                                             start=True, stop=False)
                        nc.tensor.matmul(vh[:], lhsT=ztbg[g][:, js],
                                         rhs=w_s["wv0"][l][:],
                                         start=not flags["bv0"], stop=True)
                        vo = sp.tile([128, 132], BF16, name="vo", tag="vo")
                        nc.gpsimd.memset(vo[:], 1.0)
                        nc.vector.tensor_copy(
                            vo[:].rearrange("p (h x) -> p h x", x=33)[:, :, 0:32],
                            vh[:].rearrange("p (h x) -> p h x", x=32))
                        n0c = ps_n1.tile([128, 132], F32, name="n0c", tag="nacc")
                        for h in range(NHEADS):
                            nc.tensor.matmul(
                                n0c[:, h * 33:(h + 1) * 33],
                                lhsT=et[:, h * 128:(h + 1) * 128],
                                rhs=vo[:, h * 33:(h + 1) * 33],
                                start=True, stop=True)
                        nc.vector.tensor_add(num0[:], num0[:], n0c[:])
                # ===== MAB0 tail + MAB1 prep (per batch) =====
                for b in range(BPC):
                    ztg = ZT[b]
                    num0 = num0_b[b]
                    rd0 = sp.tile([128, 4], F32, name="rd0", tag="rd0")
                    nc.vector.reciprocal(
                        rd0[:].rearrange("p (h x) -> p h x", x=1),
                        num0[:].rearrange("p (h x) -> p h x", x=33)[:, :, 32:33])
                    o0 = wp.tile([128, 128], F32, name="o0", tag=f"o0{b}")
                    nc.vector.tensor_tensor(
                        o0[:].rearrange("p (h x) -> p h x", x=32),
                        num0[:].rearrange("p (h x) -> p h x", x=33)[:, :, 0:32],
                        rd0[:].rearrange("p (h x) -> p h x", x=1).to_broadcast(
                            [128, 4, 32]),
                        op=OP.mult)
                    nc.vector.tensor_add(o0[:], o0[:], qh0_s[l][:])
                    o0tp = ps_sm.tile([128, 128], F32, name="o0tp", tag="sm")
                    nc.tensor.transpose(o0tp[:], o0[:], ident[:])
                    o0t = wp.tile([128, 128], BF16, name="o0t", tag=f"o0t{b}")
                    nc.vector.tensor_copy(o0t[:], o0tp[:])
                    fc0 = ps_sm.tile([128, 128], F32, name="fc0", tag="sm")
                    if flags["bo0"]:
                        nc.tensor.matmul(fc0[:], lhsT=ones_row[:],
                                         rhs=b_s["bo0"][l][:], start=True, stop=False)
                    nc.tensor.matmul(fc0[:], lhsT=o0t[:], rhs=w_s["wo0"][l][:],
                                     start=not flags["bo0"], stop=True)
                    hh = wp.tile([128, 128], F32, name="hh", tag=f"hh{b}")
                    nc.vector.scalar_tensor_tensor(
                        hh[:], in0=fc0[:], scalar=0.0, in1=o0[:],
                        op0=OP.max, op1=OP.add)
                    htp = ps_sm.tile([128, 128], F32, name="htp", tag="sm")
                    nc.tensor.transpose(htp[:], hh[:], ident[:])
                    ht = wp.tile([128, 128], BF16, name="ht", tag=f"ht{b}")
                    nc.vector.tensor_copy(ht[:], htp[:])

                    # ===== MAB1 prep =====
                    kh1p = ps_sm.tile([128, 128], F32, name="kh1p", tag="sm")
                    nc.tensor.matmul(kh1p[:], lhsT=w_s["wk1"][l][:], rhs=ht[:],
                                     start=True, stop=True)
                    kh1 = wp.tile([128, 128], BF16, name="kh1", tag=f"kh1{b}")
                    if flags["bk1"]:
                        nc.vector.tensor_scalar_add(kh1[:], kh1p[:], b_s["bk1"][l][:])
                    else:
                        nc.vector.tensor_copy(kh1[:], kh1p[:])
                    kh1h = []
                    for h in range(NHEADS):
                        t = wp.tile([DH, LATENT], BF16, name=f"kh1h{h}", tag=f"kh1h{h}")
                        nc.sync.dma_start(t[:], kh1[h * DH:(h + 1) * DH, :])
                        kh1h.append(t)
                    g1p = ps_sc.tile([128, 512], F32, name="g1p", tag="sc")
                    for h in range(NHEADS):
                        nc.tensor.matmul(g1p[:, h * 128:(h + 1) * 128],
                                         lhsT=w_s["wq1t"][l][h][:], rhs=kh1h[h][:],
                                         start=True, stop=True)
                    g1 = wp.tile([128, 512], BF16, name="g1", tag=f"g1{b}")
                    nc.vector.tensor_copy(g1[:], g1p[:])
                    vh1p = ps_sm.tile([128, 128], F32, name="vh1p", tag="sm")
                    if flags["bv1"]:
                        nc.tensor.matmul(vh1p[:], lhsT=ones_row[:],
                                         rhs=b_s["bv1"][l][:], start=True, stop=False)
                    nc.tensor.matmul(vh1p[:], lhsT=ht[:], rhs=w_s["wv1"][l][:],
                                     start=not flags["bv1"], stop=True)
                    vo1 = wp.tile([128, 132], BF16, name="vo1", tag=f"vo1{b}")
                    nc.vector.memset(vo1[:], 1.0)
                    nc.vector.tensor_copy(
                        vo1[:].rearrange("p (h x) -> p h x", x=33)[:, :, 0:32],
                        vh1p[:].rearrange("p (h x) -> p h x", x=32))
                    r1b = None
                    if flags["bq1"]:
                        r1bp = ps_sm.tile([128, 4], F32, name="r1bp", tag="sm")
                        for h in range(NHEADS):
                            nc.tensor.matmul(r1bp[:, h:h + 1], lhsT=kh1h[h][:],
                                             rhs=b_s["bq1"][l][h][:],
                                             start=True, stop=True)
                        r1b = wp.tile([128, 4], F32, name="r1b", tag=f"r1b{b}")
                        nc.vector.tensor_scalar_mul(r1b[:], r1bp[:], SCALE)
                    g1_b[b], vo1_b[b], r1b_b[b] = g1, vo1, r1b

                # ===== MAB1 chunks: data attends to induced (b-interleaved) ==
                for g in range(NG):
                    gw = min(GW * 128, NTP - g * GW * 128)
                    for b in range(BPC):
                        ztg, ztbg = ZT[b], ZTB[b]
                        g1, vo1, r1b = g1_b[b], vo1_b[b], r1b_b[b]
                        e1h = []
                        for h in range(NHEADS):
                            s1 = ps_sc.tile([128, GW * 128], F32, name="s1", tag="sc")
                            nc.tensor.matmul(s1[:, :gw],
                                             lhsT=g1[:, h * 128:(h + 1) * 128],
                                             rhs=ztbg[g][:, :gw],
                                             start=True, stop=True)
                            e1 = sp.tile([128, GW * 128], BF16, name="e1", tag="e1")
                            if flags["bq1"]:
                                nc.scalar.activation(e1[:, :gw], s1[:, :gw], AF.Exp,
                                                     bias=r1b[:, h:h + 1], scale=SCALE)
                            else:
                                nc.scalar.activation(e1[:, :gw], s1[:, :gw], AF.Exp,
                                                     scale=SCALE)
                            e1h.append(e1)
                        for j in range(gw // 128):
                            c = g * GW + j
                            js = slice(j * 128, (j + 1) * 128)
                            num1 = ps_n1.tile([128, 132], F32, name="num1", tag="nacc")
                            for h in range(NHEADS):
                                nc.tensor.matmul(num1[:, h * 33:(h + 1) * 33],
                                                 lhsT=e1h[h][:, js],
                                                 rhs=vo1[:, h * 33:(h + 1) * 33],
                                                 start=True, stop=True)
                            qh1 = ps_sm.tile([128, 128], F32, name="qh1", tag="sm")
                            if flags["bq1"]:
                                nc.tensor.matmul(qh1[:], lhsT=ones_row[:],
                                                 rhs=b_s["bq1r"][l][:],
                                                 start=True, stop=False)
                            nc.tensor.matmul(qh1[:], lhsT=ztbg[g][:, js],
                                             rhs=w_s["wq1"][l][:],
                                             start=not flags["bq1"], stop=True)
                            rd1 = sp.tile([128, 4], F32, name="rd1", tag="rd1")
                            nc.vector.reciprocal(
                                rd1[:].rearrange("p (h x) -> p h x", x=1),
                                num1[:].rearrange("p (h x) -> p h x", x=33)[:, :, 32:33])
                            o1 = sp.tile([128, 128], F32, name="o1", tag="o1")
                            if not flags["bo1"]:
                                # masked-O1 path: invalid rows are exactly
                                # zeroed through fc_o since bo1 == 0
                                nc.vector.tensor_scalar_mul(
                                    rd1[:], rd1[:], mkp_s[b][:, c:c + 1])
                            nc.vector.tensor_tensor(
                                o1[:].rearrange("p (h x) -> p h x", x=32),
                                num1[:].rearrange("p (h x) -> p h x", x=33)[:, :, 0:32],
                                rd1[:].rearrange("p (h x) -> p h x", x=1).to_broadcast(
                                    [128, 4, 32]),
                                op=OP.mult)
                            if flags["bo1"]:
                                nc.vector.tensor_add(o1[:], o1[:], qh1[:])
                            else:
                                nc.vector.scalar_tensor_tensor(
                                    o1[:], in0=qh1[:], scalar=mkp_s[b][:, c:c + 1],
                                    in1=o1[:], op0=OP.mult, op1=OP.add)
                            o1tp = ps_sm.tile([128, 128], F32, name="o1tp", tag="sm")
                            nc.tensor.transpose(o1tp[:], o1[:], ident[:])
                            o1t = sp.tile([128, 128], BF16, name="o1t", tag="o1t")
                            nc.scalar.copy(o1t[:], o1tp[:])
                            fc1 = ps_sm.tile([128, 128], F32, name="fc1", tag="sm")
                            if flags["bo1"]:
                                # token-major fallback path with explicit mask
                                nc.tensor.matmul(fc1[:], lhsT=ones_row[:],
                                                 rhs=b_s["bo1"][l][:],
                                                 start=True, stop=False)
                                nc.tensor.matmul(fc1[:], lhsT=o1t[:],
                                                 rhs=w_s["wo1"][l][:],
                                                 start=False, stop=True)
                                u = sp.tile([128, 128], F32, name="u", tag="u")
                                nc.vector.scalar_tensor_tensor(
                                    u[:], in0=fc1[:], scalar=0.0, in1=o1[:],
                                    op0=OP.max, op1=OP.add)
                                nc.vector.tensor_scalar_mul(u[:], u[:],
                                                            mkp_s[b][:, c:c + 1])
                                utp = ps_sm.tile([128, 128], F32, name="utp", tag="sm")
                                nc.tensor.transpose(utp[:], u[:], ident[:])
                                nc.vector.tensor_add(ztg[g][:, js], ztg[g][:, js],
                                                     utp[:])
                            else:
                                # fc computed lat-major (lhsT = Wo1 fixed);
                                # O1m is pre-masked so U = O1m + relu(fc)
                                # needs no further masking: fold the residual
                                # as two in-place adds on Z^T (no U transpose)
                                nc.tensor.matmul(fc1[:], lhsT=w_s["wo1"][l][:],
                                                 rhs=o1t[:], start=True, stop=True)
                                nc.vector.tensor_add(ztg[g][:, js], ztg[g][:, js],
                                                     o1tp[:])
                                nc.vector.scalar_tensor_tensor(
                                    ztg[g][:, js], in0=fc1[:], scalar=0.0,
                                    in1=ztg[g][:, js], op0=OP.max, op1=OP.add)
                            nc.gpsimd.tensor_copy(ztbg[g][:, js], ztg[g][:, js])

            # ---------------- output ----------------
            for c in range(NT):
                for b in range(BPC):
                    g, js = gslice(c)
                    zp = ps_sm.tile([128, 128], F32, name="zp", tag="sm")
                    nc.tensor.transpose(zp[:], ZT[b][g][:, js], ident[:])
                    zo = sp.tile([128, 128], F32, name="zo", tag="zo")
                    nc.vector.tensor_copy(zo[:], zp[:])
                    nc.sync.dma_start(
                        bass.AP(d_zout, (b * L + c * 128) * LATENT,
                                [[LATENT, 128], [1, LATENT]]),
                        zo[:])
            for b in range(BPC):
                # zero tail rows [NTP, L): 2KB-contiguous descriptor runs
                r = NTP
                while r < L:
                    n = min(512, L - r)   # rows; n*128 elems; dst stays contiguous
                    nelem = n * LATENT
                    inner = nelem // 128
                    nc.sync.dma_start(
                        bass.AP(d_zout, (b * L + r) * LATENT,
                                [[inner, 128], [1, inner]]),
                        zerot[:, :inner])
                    r += n
                # mask output (contiguous 512B rows from the token-major tile)
                nc.sync.dma_start(
                    bass.AP(d_mkout, b * L, [[128, NT], [1, 128]]), mkt_s[b][:])
                if CMAX > NT:
                    nc.sync.dma_start(
                        bass.AP(d_mkout, b * L + NTP, [[128, CMAX - NT], [1, 128]]),
                        zerot[0:CMAX - NT, 0:128])
    nc.compile()
    return nc


def _prep(inputs):
    """Host-side prep: compaction indices + weight folding (all O(small))."""
    time_x = np.ascontiguousarray(np.asarray(inputs["time_x"], np.float32))
    value_x = np.ascontiguousarray(np.asarray(inputs["value_x"], np.float32))
    mask_x = np.asarray(inputs["mask_x"])
    Wi = np.asarray(inputs["Wi"], np.float32)
    bi = np.asarray(inputs["bi"], np.float32)
    I = np.asarray(inputs["I"], np.float32)
    Wq = np.asarray(inputs["Wq"], np.float32)
    bq = np.asarray(inputs["bq"], np.float32)
    Wk = np.asarray(inputs["Wk"], np.float32)
    bk = np.asarray(inputs["bk"], np.float32)
    Wv = np.asarray(inputs["Wv"], np.float32)
    bv = np.asarray(inputs["bv"], np.float32)
    Wo = np.asarray(inputs["Wo"], np.float32)
    bo = np.asarray(inputs["bo"], np.float32)

    mflat = mask_x.reshape(B, L)
    order = np.argsort(1 - mflat, axis=1, kind="stable")
    nvalid = int(mflat.sum(axis=1).max())
    NT = max(1, min(CMAX, -(-nvalid // 128)))
    NTP = NT * 128

    ordp = order[:, :NTP]
    mkc = np.take_along_axis(mflat, ordp, axis=1).astype(np.float32)
    tfull = np.broadcast_to(time_x[:, :, None], (B, S, D)).reshape(B, L)
    tmk = np.take_along_axis(tfull, ordp, axis=1) * mkc
    umk = np.take_along_axis(value_x.reshape(B, L), ordp, axis=1) * mkc
    cidx = np.where(mkc > 0, (ordp % D).astype(np.float32), 63.0).astype(np.float32)

    wtab = np.concatenate([Wi[:D] + bi[None, :], Wi[D:D + 2]], 0)  # [43,128]

    g0 = np.zeros((NLAYERS, LATENT, 512), np.float32)
    qh0 = np.zeros((NLAYERS, NREF, LATENT), np.float32)
    r0 = np.zeros((NLAYERS, 512), np.float32)
    for l in range(NLAYERS):
        Q = I[l] @ Wq[l, 0] + bq[l, 0]
        qh0[l] = Q
        for h in range(NHEADS):
            hs = slice(h * DH, (h + 1) * DH)
            g0[l][:, h * NREF:(h + 1) * NREF] = Wk[l, 0][:, hs] @ Q[:, hs].T
            r0[l][h * NREF:(h + 1) * NREF] = bk[l, 0][hs] @ Q[:, hs].T

    w = dict(
        wv0=Wv[:, 0], wo0=Wo[:, 0], wq1=Wq[:, 1],
        wq1t=np.ascontiguousarray(Wq[:, 1].transpose(0, 2, 1)),
        wk1=Wk[:, 1], wv1=Wv[:, 1], wo1=Wo[:, 1],
    )
    bvec = dict(r0=r0, bv0=bv[:, 0], bo0=bo[:, 0], bq1=bq[:, 1],
                bk1=bk[:, 1], bv1=bv[:, 1], bo1=bo[:, 1])
    flags = {n: bool(np.any(v != 0)) for n, v in bvec.items()}
    return dict(NT=NT, NTP=NTP, tmk=tmk, umk=umk, mkc=mkc, cidx=cidx,
                wtab=wtab, g0=g0, qh0=qh0, w=w, bvec=bvec, flags=flags)


def kernel(**inputs):
    global LAST_RESULT
    p = _prep(inputs)

    key = (p["NT"], tuple(sorted(p["flags"].items())))
    if key not in _PROG_CACHE:
        _PROG_CACHE[key] = _build_program(p["NT"], p["flags"])
    nc = _PROG_CACHE[key]

    import ml_dtypes
    bf16 = ml_dtypes.bfloat16
    shared = dict(wtab=np.ascontiguousarray(p["wtab"]),
                  g0=np.ascontiguousarray(p["g0"].astype(bf16)), qh0=p["qh0"])
    for n, v in p["w"].items():
        shared[n] = np.ascontiguousarray(v.astype(bf16))
    for n, v in p["bvec"].items():
        shared[n] = np.ascontiguousarray(v)

    in_maps = []
    for m in range(NCORES):
        sl = slice(m * BPC, (m + 1) * BPC)
        im = dict(shared)
        im["cidx"] = np.ascontiguousarray(p["cidx"][sl])
        im["tmk"] = np.ascontiguousarray(p["tmk"][sl])
        im["umk"] = np.ascontiguousarray(p["umk"][sl])
        im["mk"] = np.ascontiguousarray(p["mkc"][sl])
        in_maps.append(im)

    res = run_bass_kernel_spmd(nc, in_maps, core_ids=list(range(NCORES)),
                               trace=TRACE)
    LAST_RESULT = res

    Z = np.concatenate([r["zout"] for r in res.results], axis=0)
    mk = np.concatenate([r["mkout"] for r in res.results], axis=0)[..., None]
    return Z.reshape(B, L, LATENT), mk.reshape(B, L, 1)


# revision 27
# speedup vs baseline: 2.4085x; 1.0041x over previous
"""Trainium2 Bass kernel for the masked set-transformer encoder (ISAB stack).

Strategy (pure data parallel, B=16 over 8 cores, 2 batch elements/core):
  * The compaction permutation commutes with the whole network: softmax over
    keys is permutation invariant, everything else is row-wise, and masked
    rows are exactly zero throughout.  So the host only computes the stable
    argsort *indices*; the device processes tokens in compacted order and the
    output is already compacted (zero tail appended on device).
  * Only NT = ceil(max_b nvalid_b / 128) tiles of 128 tokens are processed
    (~42 instead of 82 for random masks).  NT is a compile-time constant
    derived from the actual mask; the program is recompiled if it changes.
  * The one-hot input FF collapses to a [43,128] table matmul against a
    device-built X^T = [one_hot(c); t*mk; u*mk] (no gathers: one-hot rows are
    built with an is_equal against an iota column; invalid tokens get an
    out-of-range channel id so their X^T column is exactly zero).
  * Attention layouts keep softmax reductions on natural axes:
      MAB0 scores  S^T[tok,(h,q)] = Z @ G0,  G0 = fold(Wk, I@Wq+bq)  (host)
      MAB0 key masking is folded into the exp bias: exp(s*scale + (mk-1)*30)
      MAB0 num/den via lhsT=E^T_h, rhs=[Vh_h | 1], DVE-accumulated over chunks
      MAB1 scores  S1^T[k, tok] per head via lhsT=G1_h, rhs=Z^T (4-chunk tiles)
      MAB1 num/den via lhsT=E1^T_h, rhs=[Vh1_h | 1]
  * Z^T lives in SBUF as [128, 512] group tiles so MAB1 score matmuls stream
    512 tokens per instruction.
  * ACT does exp only; biases are all zero in practice (trace-time fallbacks
    emit extra ones-row matmuls / bias adds when they are not).
"""

import math

import numpy as np

import concourse.bacc as bacc
import concourse.bass as bass
import concourse.mybir as mybir
import concourse.tile as tile
from concourse.bass_utils import run_bass_kernel_spmd
from concourse.masks import make_identity

F32 = mybir.dt.float32
BF16 = mybir.dt.bfloat16
AF = mybir.ActivationFunctionType
OP = mybir.AluOpType

B, S, D = 16, 256, 41
L = S * D                      # 10496
LATENT, NREF, NLAYERS, NHEADS = 128, 128, 3, 4
DH = LATENT // NHEADS          # 32
SCALE = 1.0 / math.sqrt(LATENT)
NCORES = 8
BPC = B // NCORES              # 2
CMAX = L // 128                # 82
NEGBIG = -30.0                 # exp(-30) ~ 1e-13: masked-key contribution
GW = 4                         # chunks per Z^T group tile

# set by test harness to capture profiling info
TRACE = False
LAST_RESULT = None

_PROG_CACHE: dict = {}


def _build_program(NT: int, flags: dict, nlayers: int = NLAYERS):
    NTP = NT * 128
    NG = -(-NT // GW)          # number of Z^T group tiles
    nc = bacc.Bacc("TRN2")

    def gslice(c):
        """(group index, column slice within the group tile) for chunk c."""
        return c // GW, slice((c % GW) * 128, (c % GW) * 128 + 128)

    # ---------------- DRAM I/O ----------------
    d_cidx = nc.dram_tensor("cidx", [BPC, NTP], F32, kind="ExternalInput")
    d_tmk = nc.dram_tensor("tmk", [BPC, NTP], F32, kind="ExternalInput")
    d_umk = nc.dram_tensor("umk", [BPC, NTP], F32, kind="ExternalInput")
    d_mk = nc.dram_tensor("mk", [BPC, NTP], F32, kind="ExternalInput")
    d_wtab = nc.dram_tensor("wtab", [43, LATENT], F32, kind="ExternalInput")
    d_g0 = nc.dram_tensor("g0", [NLAYERS, LATENT, 512], BF16, kind="ExternalInput")
    d_qh0 = nc.dram_tensor("qh0", [NLAYERS, NREF, LATENT], F32, kind="ExternalInput")
    WNAMES = ["wv0", "wo0", "wq1", "wq1t", "wk1", "wv1", "wo1"]
    d_w = {
        n: nc.dram_tensor(n, [NLAYERS, LATENT, LATENT], BF16, kind="ExternalInput")
        for n in WNAMES
    }
    BNAMES = ["r0", "bv0", "bo0", "bq1", "bk1", "bv1", "bo1"]
    d_b = {
        n: nc.dram_tensor(n, [NLAYERS, 512 if n == "r0" else LATENT], F32,
                          kind="ExternalInput")
        for n in BNAMES
    }
    d_zout = nc.dram_tensor("zout", [BPC, L, LATENT], F32, kind="ExternalOutput")
    d_mkout = nc.dram_tensor("mkout", [BPC, L], F32, kind="ExternalOutput")

    with tile.TileContext(nc) as tc:
        with (
            tc.tile_pool(name="persist", bufs=1) as pp,
            tc.tile_pool(name="work", bufs=3) as wp,
            tc.tile_pool(name="stream", bufs=8) as sp,
            tc.tile_pool(name="ps_sc", bufs=2, space="PSUM") as ps_sc,
            tc.tile_pool(name="ps_n1", bufs=2, space="PSUM") as ps_n1,
            tc.tile_pool(name="ps_sm", bufs=4, space="PSUM") as ps_sm,
        ):
            # ---------------- constants & weights ----------------
            ident = pp.tile([128, 128], F32, name="ident")
            make_identity(nc, ident[:])

            iota_i = pp.tile([41, 1], mybir.dt.int32, name="iota_i")
            nc.gpsimd.iota(iota_i[:], [[1, 1]], channel_multiplier=1)
            iota_f = pp.tile([41, 1], F32, name="iota_f")
            nc.vector.tensor_copy(iota_f[:], iota_i[:])

            ones_row = pp.tile([1, 128], F32, name="ones_row")
            nc.vector.memset(ones_row[:], 1.0)
            zerot = pp.tile([128, 512], F32, name="zerot")
            nc.vector.memset(zerot[:], 0.0)

            wtab_s = pp.tile([43, LATENT], F32, name="wtab_s")
            nc.sync.dma_start(wtab_s[:], d_wtab[:, :])

            g0_s, qh0_s = [], []
            w_s = {n: [] for n in WNAMES}
            b_s = {n: [] for n in BNAMES}
            for l in range(NLAYERS):
                g = pp.tile([LATENT, 512], BF16, name=f"g0s{l}", tag=f"g0s{l}")
                nc.sync.dma_start(g[:], d_g0[l, :, :])
                g0_s.append(g)
                q = pp.tile([NREF, LATENT], F32, name=f"qh0s{l}", tag=f"qh0s{l}")
                nc.sync.dma_start(q[:], d_qh0[l, :, :])
                qh0_s.append(q)
                for n in WNAMES:
                    if n == "wq1t":
                        # per-head [32,128] tiles (PE weights must start at
                        # partition 0/32/64, so a [96:128] slice is illegal)
                        hh_tiles = []
                        for h in range(NHEADS):
                            t = pp.tile([DH, LATENT], BF16, name=f"wq1t{l}h{h}",
                                        tag=f"wq1t{l}h{h}")
                            nc.sync.dma_start(
                                t[:], d_w[n][l, h * DH:(h + 1) * DH, :])
                            hh_tiles.append(t)
                        w_s[n].append(hh_tiles)
                        continue
                    t = pp.tile([LATENT, LATENT], BF16, name=f"{n}s{l}", tag=f"{n}s{l}")
                    nc.sync.dma_start(t[:], d_w[n][l, :, :])
                    w_s[n].append(t)
                for n in BNAMES:
                    if not flags[n]:
                        b_s[n].append(None)
                        continue
                    if n in ("bk1",):        # needed as a [128,1] column
                        t = pp.tile([LATENT, 1], F32, name=f"{n}s{l}", tag=f"{n}s{l}")
                        nc.sync.dma_start(
                            t[:], bass.AP(d_b[n], l * LATENT, [[1, LATENT], [1, 1]]))
                    elif n == "bq1":         # per-head column tiles [32,1]
                        t = []
                        for h in range(NHEADS):
                            th = pp.tile([DH, 1], F32, name=f"{n}c{l}h{h}",
                                         tag=f"{n}c{l}h{h}")
                            nc.sync.dma_start(
                                th[:], bass.AP(d_b[n], l * LATENT + h * DH,
                                               [[1, DH], [1, 1]]))
                            t.append(th)
                    else:
                        w = 512 if n == "r0" else LATENT
                        t = pp.tile([1, w], F32, name=f"{n}s{l}", tag=f"{n}s{l}")
                        nc.sync.dma_start(t[:], d_b[n][l:l + 1, :])
                    b_s[n].append(t)
                if flags["bq1"]:  # row form for the ones-matmul into Qh1
                    t = pp.tile([1, LATENT], F32, name=f"bq1rs{l}", tag=f"bq1rs{l}")
                    nc.sync.dma_start(t[:], d_b["bq1"][l:l + 1, :])
                    b_s.setdefault("bq1r", []).append(t)

            # ---------------- per-batch setup + Z0 ----------------
            mkp_s, mkneg_s, mkt_s, ZT, ZTB = [], [], [], [], []
            for b in range(BPC):
                mkt = pp.tile([NT, 128], F32, name=f"mkt{b}", tag=f"mkt{b}")
                nc.sync.dma_start(mkt[:], bass.AP(d_mk, b * NTP, [[128, NT], [1, 128]]))
                mkt_s.append(mkt)
                mkpp = ps_sm.tile([128, NT], F32, name="mkpp", tag="sm")
                nc.tensor.transpose(mkpp[:], mkt[:], ident[0:NT, 0:NT])
                mkp = pp.tile([128, NT], F32, name=f"mkp{b}", tag=f"mkp{b}")
                nc.vector.tensor_copy(mkp[:], mkpp[:])
                mkp_s.append(mkp)
                mkneg = pp.tile([128, NT], F32, name=f"mkneg{b}", tag=f"mkneg{b}")
                nc.vector.tensor_scalar(
                    mkneg[:], mkp[:], -1.0, -NEGBIG, op0=OP.add, op1=OP.mult)
                mkneg_s.append(mkneg)

                xt = pp.tile([43, NTP], F32, name=f"xt{b}", tag="xt")
                crow = pp.tile([1, NTP], F32, name=f"crow{b}", tag="crow")
                nc.sync.dma_start(crow[:], d_cidx[b:b + 1, :])
                # replicate cidx row across 41 partitions via a K=1 matmul,
                # then one-hot it against the iota column
                for j in range(0, NTP, 512):
                    w = min(512, NTP - j)
                    cb = ps_sm.tile([41, 512], F32, name="cb", tag="sm")
                    nc.tensor.matmul(cb[:, :w], lhsT=ones_row[:, 0:41],
                                     rhs=crow[:, j:j + w], start=True, stop=True)
                    nc.vector.tensor_scalar(
                        xt[0:41, j:j + w], cb[:, :w], iota_f[:], None,
                        op0=OP.is_equal)
                nc.sync.dma_start(xt[41:42, :], d_tmk[b:b + 1, :])
                nc.sync.dma_start(xt[42:43, :], d_umk[b:b + 1, :])

                ztg, ztbg = [], []
                for g in range(NG):
                    zt = pp.tile([128, GW * 128], F32, name=f"zt{b}_{g}",
                                 tag=f"zt{b}_{g}")
                    ztg.append(zt)
                    ztb = pp.tile([128, GW * 128], BF16, name=f"ztb{b}_{g}",
                                  tag=f"ztb{b}_{g}")
                    ztbg.append(ztb)
                for c in range(NT):
                    g, js = gslice(c)
                    z0p = ps_sm.tile([128, 128], F32, name="z0p", tag="sm")
                    nc.tensor.matmul(
                        z0p[:], lhsT=wtab_s[:], rhs=xt[:, c * 128:(c + 1) * 128],
                        start=True, stop=True)
                    nc.vector.tensor_scalar_max(ztg[g][:, js], z0p[:], 0.0)
                    nc.gpsimd.tensor_copy(ztbg[g][:, js], ztg[g][:, js])
                ZT.append(ztg)
                ZTB.append(ztbg)

            # ---------------- layers ----------------
            for l in range(nlayers):
                g1_b, vo1_b, r1b_b = {}, {}, {}
                num0p = wp.tile([128, 264], F32, name="num0p", tag="num0p")
                nc.vector.memset(num0p[:], 0.0)
                # ===== MAB0: induced points attend to data =====
                # batches paired: one [128,264] VO build + one accumulator
                # add per chunk instead of two of each (DVE-bound phase)
                for c in range(NT):
                    ets = {}
                    vhp = ps_sm.tile([128, 256], F32, name="vhp", tag="sm")
                    for b in range(BPC):
                        ztbg = ZTB[b]
                        g, js = gslice(c)
                        s0 = ps_sc.tile([128, 512], F32, name="s0", tag="sc")
                        if flags["r0"]:
                            nc.tensor.matmul(s0[:], lhsT=ones_row[:],
                                             rhs=b_s["r0"][l][:],
                                             start=True, stop=False)
                        nc.tensor.matmul(s0[:], lhsT=ztbg[g][:, js], rhs=g0_s[l][:],
                                         start=not flags["r0"], stop=True)
                        et = sp.tile([128, 512], BF16, name="et", tag="et")
                        nc.scalar.activation(et[:], s0[:], AF.Exp,
                                             bias=mkneg_s[b][:, c:c + 1],
                                             scale=SCALE)
                        ets[b] = et
                        if flags["bv0"]:
                            nc.tensor.matmul(vhp[:, b * 128:(b + 1) * 128],
                                             lhsT=ones_row[:],
                                             rhs=b_s["bv0"][l][:],
                                             start=True, stop=False)
                        nc.tensor.matmul(vhp[:, b * 128:(b + 1) * 128],
                                         lhsT=ztbg[g][:, js],
                                         rhs=w_s["wv0"][l][:],
                                         start=not flags["bv0"], stop=True)
                    vo = sp.tile([128, 264], BF16, name="vo", tag="vo")
                    nc.gpsimd.memset(vo[:], 1.0)
                    nc.vector.tensor_copy(
                        vo[:].rearrange("p (x h v) -> p x h v", x=2, v=33)[:, :, :, 0:32],
                        vhp[:].rearrange("p (x h v) -> p x h v", x=2, v=32))
                    n0c = ps_n1.tile([128, 264], F32, name="n0c", tag="nacc")
                    for b in range(BPC):
                        for h in range(NHEADS):
                            o = b * 132 + h * 33
                            nc.tensor.matmul(
                                n0c[:, o:o + 33],
                                lhsT=ets[b][:, h * 128:(h + 1) * 128],
                                rhs=vo[:, o:o + 33],
                                start=True, stop=True)
                    nc.vector.tensor_add(num0p[:], num0p[:], n0c[:])
                # ===== MAB0 tail + MAB1 prep (per batch) =====
                for b in range(BPC):
                    ztg = ZT[b]
                    num0 = num0p[:, b * 132:(b + 1) * 132]
                    rd0 = sp.tile([128, 4], F32, name="rd0", tag="rd0")
                    nc.vector.reciprocal(
                        rd0[:].rearrange("p (h x) -> p h x", x=1),
                        num0.rearrange("p (h x) -> p h x", x=33)[:, :, 32:33])
                    o0 = wp.tile([128, 128], F32, name="o0", tag=f"o0{b}")
                    nc.vector.tensor_tensor(
                        o0[:].rearrange("p (h x) -> p h x", x=32),
                        num0.rearrange("p (h x) -> p h x", x=33)[:, :, 0:32],
                        rd0[:].rearrange("p (h x) -> p h x", x=1).to_broadcast(
                            [128, 4, 32]),
                        op=OP.mult)
                    nc.vector.tensor_add(o0[:], o0[:], qh0_s[l][:])
                    o0tp = ps_sm.tile([128, 128], F32, name="o0tp", tag="sm")
                    nc.tensor.transpose(o0tp[:], o0[:], ident[:])
                    o0t = wp.tile([128, 128], BF16, name="o0t", tag=f"o0t{b}")
                    nc.vector.tensor_copy(o0t[:], o0tp[:])
                    fc0 = ps_sm.tile([128, 128], F32, name="fc0", tag="sm")
                    if flags["bo0"]:
                        nc.tensor.matmul(fc0[:], lhsT=ones_row[:],
                                         rhs=b_s["bo0"][l][:], start=True, stop=False)
                    nc.tensor.matmul(fc0[:], lhsT=o0t[:], rhs=w_s["wo0"][l][:],
                                     start=not flags["bo0"], stop=True)
                    hh = wp.tile([128, 128], F32, name="hh", tag=f"hh{b}")
                    nc.vector.scalar_tensor_tensor(
                        hh[:], in0=fc0[:], scalar=0.0, in1=o0[:],
                        op0=OP.max, op1=OP.add)
                    htp = ps_sm.tile([128, 128], F32, name="htp", tag="sm")
                    nc.tensor.transpose(htp[:], hh[:], ident[:])
                    ht = wp.tile([128, 128], BF16, name="ht", tag=f"ht{b}")
                    nc.vector.tensor_copy(ht[:], htp[:])

                    # ===== MAB1 prep =====
                    kh1p = ps_sm.tile([128, 128], F32, name="kh1p", tag="sm")
                    nc.tensor.matmul(kh1p[:], lhsT=w_s["wk1"][l][:], rhs=ht[:],
                                     start=True, stop=True)
                    kh1 = wp.tile([128, 128], BF16, name="kh1", tag=f"kh1{b}")
                    if flags["bk1"]:
                        nc.vector.tensor_scalar_add(kh1[:], kh1p[:], b_s["bk1"][l][:])
                    else:
                        nc.vector.tensor_copy(kh1[:], kh1p[:])
                    kh1h = []
                    for h in range(NHEADS):
                        t = wp.tile([DH, LATENT], BF16, name=f"kh1h{h}", tag=f"kh1h{h}")
                        nc.sync.dma_start(t[:], kh1[h * DH:(h + 1) * DH, :])
                        kh1h.append(t)
                    g1p = ps_sc.tile([128, 512], F32, name="g1p", tag="sc")
                    for h in range(NHEADS):
                        nc.tensor.matmul(g1p[:, h * 128:(h + 1) * 128],
                                         lhsT=w_s["wq1t"][l][h][:], rhs=kh1h[h][:],
                                         start=True, stop=True)
                    g1 = wp.tile([128, 512], BF16, name="g1", tag=f"g1{b}")
                    nc.vector.tensor_copy(g1[:], g1p[:])
                    vh1p = ps_sm.tile([128, 128], F32, name="vh1p", tag="sm")
                    if flags["bv1"]:
                        nc.tensor.matmul(vh1p[:], lhsT=ones_row[:],
                                         rhs=b_s["bv1"][l][:], start=True, stop=False)
                    nc.tensor.matmul(vh1p[:], lhsT=ht[:], rhs=w_s["wv1"][l][:],
                                     start=not flags["bv1"], stop=True)
                    vo1 = wp.tile([128, 132], BF16, name="vo1", tag=f"vo1{b}")
                    nc.vector.memset(vo1[:], 1.0)
                    nc.vector.tensor_copy(
                        vo1[:].rearrange("p (h x) -> p h x", x=33)[:, :, 0:32],
                        vh1p[:].rearrange("p (h x) -> p h x", x=32))
                    r1b = None
                    if flags["bq1"]:
                        r1bp = ps_sm.tile([128, 4], F32, name="r1bp", tag="sm")
                        for h in range(NHEADS):
                            nc.tensor.matmul(r1bp[:, h:h + 1], lhsT=kh1h[h][:],
                                             rhs=b_s["bq1"][l][h][:],
                                             start=True, stop=True)
                        r1b = wp.tile([128, 4], F32, name="r1b", tag=f"r1b{b}")
                        nc.vector.tensor_scalar_mul(r1b[:], r1bp[:], SCALE)
                    g1_b[b], vo1_b[b], r1b_b[b] = g1, vo1, r1b

                # ===== MAB1 chunks: data attends to induced (b-interleaved) ==
                for g in range(NG):
                    gw = min(GW * 128, NTP - g * GW * 128)
                    for b in range(BPC):
                        ztg, ztbg = ZT[b], ZTB[b]
                        g1, vo1, r1b = g1_b[b], vo1_b[b], r1b_b[b]
                        e1h = []
                        for h in range(NHEADS):
                            s1 = ps_sc.tile([128, GW * 128], F32, name="s1", tag="sc")
                            nc.tensor.matmul(s1[:, :gw],
                                             lhsT=g1[:, h * 128:(h + 1) * 128],
                                             rhs=ztbg[g][:, :gw],
                                             start=True, stop=True)
                            e1 = sp.tile([128, GW * 128], BF16, name="e1", tag="e1")
                            if flags["bq1"]:
                                nc.scalar.activation(e1[:, :gw], s1[:, :gw], AF.Exp,
                                                     bias=r1b[:, h:h + 1], scale=SCALE)
                            else:
                                nc.scalar.activation(e1[:, :gw], s1[:, :gw], AF.Exp,
                                                     scale=SCALE)
                            e1h.append(e1)
                        for j in range(gw // 128):
                            c = g * GW + j
                            js = slice(j * 128, (j + 1) * 128)
                            num1 = ps_n1.tile([128, 132], F32, name="num1", tag="nacc")
                            for h in range(NHEADS):
                                nc.tensor.matmul(num1[:, h * 33:(h + 1) * 33],
                                                 lhsT=e1h[h][:, js],
                                                 rhs=vo1[:, h * 33:(h + 1) * 33],
                                                 start=True, stop=True)
                            qh1 = ps_sm.tile([128, 128], F32, name="qh1", tag="sm")
                            if flags["bq1"]:
                                nc.tensor.matmul(qh1[:], lhsT=ones_row[:],
                                                 rhs=b_s["bq1r"][l][:],
                                                 start=True, stop=False)
                            nc.tensor.matmul(qh1[:], lhsT=ztbg[g][:, js],
                                             rhs=w_s["wq1"][l][:],
                                             start=not flags["bq1"], stop=True)
                            rd1 = sp.tile([128, 4], F32, name="rd1", tag="rd1")
                            nc.vector.reciprocal(
                                rd1[:].rearrange("p (h x) -> p h x", x=1),
                                num1[:].rearrange("p (h x) -> p h x", x=33)[:, :, 32:33])
                            o1 = sp.tile([128, 128], F32, name="o1", tag="o1")
                            if not flags["bo1"]:
                                # masked-O1 path: invalid rows are exactly
                                # zeroed through fc_o since bo1 == 0
                                nc.vector.tensor_scalar_mul(
                                    rd1[:], rd1[:], mkp_s[b][:, c:c + 1])
                            nc.vector.tensor_tensor(
                                o1[:].rearrange("p (h x) -> p h x", x=32),
                                num1[:].rearrange("p (h x) -> p h x", x=33)[:, :, 0:32],
                                rd1[:].rearrange("p (h x) -> p h x", x=1).to_broadcast(
                                    [128, 4, 32]),
                                op=OP.mult)
                            if flags["bo1"]:
                                nc.vector.tensor_add(o1[:], o1[:], qh1[:])
                            else:
                                nc.vector.scalar_tensor_tensor(
                                    o1[:], in0=qh1[:], scalar=mkp_s[b][:, c:c + 1],
                                    in1=o1[:], op0=OP.mult, op1=OP.add)
                            o1tp = ps_sm.tile([128, 128], F32, name="o1tp", tag="sm")
                            nc.tensor.transpose(o1tp[:], o1[:], ident[:])
                            o1t = sp.tile([128, 128], BF16, name="o1t", tag="o1t")
                            nc.scalar.copy(o1t[:], o1tp[:])
                            fc1 = ps_sm.tile([128, 128], F32, name="fc1", tag="sm")
                            if flags["bo1"]:
                                # token-major fallback path with explicit mask
                                nc.tensor.matmul(fc1[:], lhsT=ones_row[:],
                                                 rhs=b_s["bo1"][l][:],
                                                 start=True, stop=False)
                                nc.tensor.matmul(fc1[:], lhsT=o1t[:],
                                                 rhs=w_s["wo1"][l][:],
                                                 start=False, stop=True)
                                u = sp.tile([128, 128], F32, name="u", tag="u")
                                nc.vector.scalar_tensor_tensor(
                                    u[:], in0=fc1[:], scalar=0.0, in1=o1[:],
                                    op0=OP.max, op1=OP.add)
                                nc.vector.tensor_scalar_mul(u[:], u[:],
                                                            mkp_s[b][:, c:c + 1])
                                utp = ps_sm.tile([128, 128], F32, name="utp", tag="sm")
                                nc.tensor.transpose(utp[:], u[:], ident[:])
                                nc.vector.tensor_add(ztg[g][:, js], ztg[g][:, js],
                                                     utp[:])
                            else:
                                # fc computed lat-major (lhsT = Wo1 fixed);
                                # O1m is pre-masked so U = O1m + relu(fc)
                                # needs no further masking: fold the residual
                                # as two in-place adds on Z^T (no U transpose)
                                nc.tensor.matmul(fc1[:], lhsT=w_s["wo1"][l][:],
                                                 rhs=o1t[:], start=True, stop=True)
                                nc.vector.tensor_add(ztg[g][:, js], ztg[g][:, js],
                                                     o1tp[:])
                                nc.vector.scalar_tensor_tensor(
                                    ztg[g][:, js], in0=fc1[:], scalar=0.0,
                                    in1=ztg[g][:, js], op0=OP.max, op1=OP.add)
                            nc.gpsimd.tensor_copy(ztbg[g][:, js], ztg[g][:, js])

            # ---------------- output ----------------
            for c in range(NT):
                for b in range(BPC):
                    g, js = gslice(c)
                    zp = ps_sm.tile([128, 128], F32, name="zp", tag="sm")
                    nc.tensor.transpose(zp[:], ZT[b][g][:, js], ident[:])
                    zo = sp.tile([128, 128], F32, name="zo", tag="zo")
                    nc.vector.tensor_copy(zo[:], zp[:])
                    nc.sync.dma_start(
                        bass.AP(d_zout, (b * L + c * 128) * LATENT,
                                [[LATENT, 128], [1, LATENT]]),
                        zo[:])
            for b in range(BPC):
                # zero tail rows [NTP, L): 2KB-contiguous descriptor runs
                r = NTP
                while r < L:
                    n = min(512, L - r)   # rows; n*128 elems; dst stays contiguous
                    nelem = n * LATENT
                    inner = nelem // 128
                    nc.sync.dma_start(
                        bass.AP(d_zout, (b * L + r) * LATENT,
                                [[inner, 128], [1, inner]]),
                        zerot[:, :inner])
                    r += n
                # mask output (contiguous 512B rows from the token-major tile)
                nc.sync.dma_start(
                    bass.AP(d_mkout, b * L, [[128, NT], [1, 128]]), mkt_s[b][:])
                if CMAX > NT:
                    nc.sync.dma_start(
                        bass.AP(d_mkout, b * L + NTP, [[128, CMAX - NT], [1, 128]]),
                        zerot[0:CMAX - NT, 0:128])
    nc.compile()
    return nc


def _prep(inputs):
    """Host-side prep: compaction indices + weight folding (all O(small))."""
    time_x = np.ascontiguousarray(np.asarray(inputs["time_x"], np.float32))
    value_x = np.ascontiguousarray(np.asarray(inputs["value_x"], np.float32))
    mask_x = np.asarray(inputs["mask_x"])
    Wi = np.asarray(inputs["Wi"], np.float32)
    bi = np.asarray(inputs["bi"], np.float32)
    I = np.asarray(inputs["I"], np.float32)
    Wq = np.asarray(inputs["Wq"], np.float32)
    bq = np.asarray(inputs["bq"], np.float32)
    Wk = np.asarray(inputs["Wk"], np.float32)
    bk = np.asarray(inputs["bk"], np.float32)
    Wv = np.asarray(inputs["Wv"], np.float32)
    bv = np.asarray(inputs["bv"], np.float32)
    Wo = np.asarray(inputs["Wo"], np.float32)
    bo = np.asarray(inputs["bo"], np.float32)

    mflat = mask_x.reshape(B, L)
    order = np.argsort(1 - mflat, axis=1, kind="stable")
    nvalid = int(mflat.sum(axis=1).max())
    NT = max(1, min(CMAX, -(-nvalid // 128)))
    NTP = NT * 128

    ordp = order[:, :NTP]
    mkc = np.take_along_axis(mflat, ordp, axis=1).astype(np.float32)
    tfull = np.broadcast_to(time_x[:, :, None], (B, S, D)).reshape(B, L)
    tmk = np.take_along_axis(tfull, ordp, axis=1) * mkc
    umk = np.take_along_axis(value_x.reshape(B, L), ordp, axis=1) * mkc
    cidx = np.where(mkc > 0, (ordp % D).astype(np.float32), 63.0).astype(np.float32)

    wtab = np.concatenate([Wi[:D] + bi[None, :], Wi[D:D + 2]], 0)  # [43,128]

    g0 = np.zeros((NLAYERS, LATENT, 512), np.float32)
    qh0 = np.zeros((NLAYERS, NREF, LATENT), np.float32)
    r0 = np.zeros((NLAYERS, 512), np.float32)
    for l in range(NLAYERS):
        Q = I[l] @ Wq[l, 0] + bq[l, 0]
        qh0[l] = Q
        for h in range(NHEADS):
            hs = slice(h * DH, (h + 1) * DH)
            g0[l][:, h * NREF:(h + 1) * NREF] = Wk[l, 0][:, hs] @ Q[:, hs].T
            r0[l][h * NREF:(h + 1) * NREF] = bk[l, 0][hs] @ Q[:, hs].T

    w = dict(
        wv0=Wv[:, 0], wo0=Wo[:, 0], wq1=Wq[:, 1],
        wq1t=np.ascontiguousarray(Wq[:, 1].transpose(0, 2, 1)),
        wk1=Wk[:, 1], wv1=Wv[:, 1], wo1=Wo[:, 1],
    )
    bvec = dict(r0=r0, bv0=bv[:, 0], bo0=bo[:, 0], bq1=bq[:, 1],
                bk1=bk[:, 1], bv1=bv[:, 1], bo1=bo[:, 1])
    flags = {n: bool(np.any(v != 0)) for n, v in bvec.items()}
    return dict(NT=NT, NTP=NTP, tmk=tmk, umk=umk, mkc=mkc, cidx=cidx,
                wtab=wtab, g0=g0, qh0=qh0, w=w, bvec=bvec, flags=flags)


def kernel(**inputs):
    global LAST_RESULT
    p = _prep(inputs)

    key = (p["NT"], tuple(sorted(p["flags"].items())))
    if key not in _PROG_CACHE:
        _PROG_CACHE[key] = _build_program(p["NT"], p["flags"])
    nc = _PROG_CACHE[key]

    import ml_dtypes
    bf16 = ml_dtypes.bfloat16
    shared = dict(wtab=np.ascontiguousarray(p["wtab"]),
                  g0=np.ascontiguousarray(p["g0"].astype(bf16)), qh0=p["qh0"])
    for n, v in p["w"].items():
        shared[n] = np.ascontiguousarray(v.astype(bf16))
    for n, v in p["bvec"].items():
        shared[n] = np.ascontiguousarray(v)

    in_maps = []
    for m in range(NCORES):
        sl = slice(m * BPC, (m + 1) * BPC)
        im = dict(shared)
        im["cidx"] = np.ascontiguousarray(p["cidx"][sl])
        im["tmk"] = np.ascontiguousarray(p["tmk"][sl])
        im["umk"] = np.ascontiguousarray(p["umk"][sl])
        im["mk"] = np.ascontiguousarray(p["mkc"][sl])
        in_maps.append(im)

    res = run_bass_kernel_spmd(nc, in_maps, core_ids=list(range(NCORES)),
                               trace=TRACE)
    LAST_RESULT = res

    Z = np.concatenate([r["zout"] for r in res.results], axis=0)
    mk = np.concatenate([r["mkout"] for r in res.results], axis=0)[..., None]
    return Z.reshape(B, L, LATENT), mk.reshape(B, L, 1)


# revision 28
# speedup vs baseline: 2.8174x; 1.1698x over previous
"""Trainium2 Bass kernel for the masked set-transformer encoder (ISAB stack).

Strategy (pure data parallel, B=16 over 8 cores, 2 batch elements/core):
  * The compaction permutation commutes with the whole network: softmax over
    keys is permutation invariant, everything else is row-wise, and masked
    rows are exactly zero throughout.  So the host only computes the stable
    argsort *indices*; the device processes tokens in compacted order and the
    output is already compacted (zero tail appended on device).
  * Only NT = ceil(max_b nvalid_b / 128) tiles of 128 tokens are processed
    (~42 instead of 82 for random masks).  NT is a compile-time constant
    derived from the actual mask; the program is recompiled if it changes.
  * The one-hot input FF collapses to a [43,128] table matmul against a
    device-built X^T = [one_hot(c); t*mk; u*mk] (no gathers: one-hot rows are
    built with an is_equal against an iota column; invalid tokens get an
    out-of-range channel id so their X^T column is exactly zero).
  * Attention layouts keep softmax reductions on natural axes:
      MAB0 scores  S^T[tok,(h,q)] = Z @ G0,  G0 = fold(Wk, I@Wq+bq)  (host)
      MAB0 key masking is folded into the exp bias: exp(s*scale + (mk-1)*30)
      MAB0 num/den via lhsT=E^T_h, rhs=[Vh_h | 1], DVE-accumulated over chunks
      MAB1 scores  S1^T[k, tok] per head via lhsT=G1_h, rhs=Z^T (4-chunk tiles)
      MAB1 num/den via lhsT=E1^T_h, rhs=[Vh1_h | 1]
  * Z^T lives in SBUF as [128, 512] group tiles so MAB1 score matmuls stream
    512 tokens per instruction.
  * ACT does exp only; biases are all zero in practice (trace-time fallbacks
    emit extra ones-row matmuls / bias adds when they are not).
"""

import math

import numpy as np

import concourse.bacc as bacc
import concourse.bass as bass
import concourse.mybir as mybir
import concourse.tile as tile
from concourse.bass_utils import run_bass_kernel_spmd
from concourse.masks import make_identity

F32 = mybir.dt.float32
BF16 = mybir.dt.bfloat16
AF = mybir.ActivationFunctionType
OP = mybir.AluOpType

B, S, D = 16, 256, 41
L = S * D                      # 10496
LATENT, NREF, NLAYERS, NHEADS = 128, 128, 3, 4
DH = LATENT // NHEADS          # 32
SCALE = 1.0 / math.sqrt(LATENT)
NCORES = 8
BPC = B // NCORES              # 2
CMAX = L // 128                # 82
NEGBIG = -30.0                 # exp(-30) ~ 1e-13: masked-key contribution
GW = 4                         # chunks per Z^T group tile

# set by test harness to capture profiling info
TRACE = False
LAST_RESULT = None

_PROG_CACHE: dict = {}


def _build_program(NT: int, flags: dict, nlayers: int = NLAYERS):
    NTP = NT * 128
    NG = -(-NT // GW)          # number of Z^T group tiles
    nc = bacc.Bacc("TRN2")

    def gslice(c):
        """(group index, column slice within the group tile) for chunk c."""
        return c // GW, slice((c % GW) * 128, (c % GW) * 128 + 128)

    # ---------------- DRAM I/O ----------------
    d_cidx = nc.dram_tensor("cidx", [BPC, NTP], F32, kind="ExternalInput")
    d_tmk = nc.dram_tensor("tmk", [BPC, NTP], F32, kind="ExternalInput")
    d_umk = nc.dram_tensor("umk", [BPC, NTP], F32, kind="ExternalInput")
    d_mk = nc.dram_tensor("mk", [BPC, NTP], F32, kind="ExternalInput")
    d_wtab = nc.dram_tensor("wtab", [43, LATENT], F32, kind="ExternalInput")
    d_g0 = nc.dram_tensor("g0", [NLAYERS, LATENT, 512], BF16, kind="ExternalInput")
    d_qh0 = nc.dram_tensor("qh0", [NLAYERS, NREF, LATENT], F32, kind="ExternalInput")
    WNAMES = ["wv0", "wo0", "wq1", "wq1t", "wk1", "wv1", "wo1"]
    d_w = {
        n: nc.dram_tensor(n, [NLAYERS, LATENT, LATENT], BF16, kind="ExternalInput")
        for n in WNAMES
    }
    BNAMES = ["r0", "bv0", "bo0", "bq1", "bk1", "bv1", "bo1"]
    d_b = {
        n: nc.dram_tensor(n, [NLAYERS, 512 if n == "r0" else LATENT], F32,
                          kind="ExternalInput")
        for n in BNAMES
    }
    d_zout = nc.dram_tensor("zout", [BPC, L, LATENT], F32, kind="ExternalOutput")
    d_mkout = nc.dram_tensor("mkout", [BPC, L], F32, kind="ExternalOutput")

    with tile.TileContext(nc) as tc:
        with (
            tc.tile_pool(name="persist", bufs=1) as pp,
            tc.tile_pool(name="work", bufs=3) as wp,
            tc.tile_pool(name="stream", bufs=8) as sp,
            tc.tile_pool(name="ps_sc", bufs=2, space="PSUM") as ps_sc,
            tc.tile_pool(name="ps_n1", bufs=2, space="PSUM") as ps_n1,
            tc.tile_pool(name="ps_sm", bufs=4, space="PSUM") as ps_sm,
        ):
            # ---------------- constants & weights ----------------
            ident = pp.tile([128, 128], F32, name="ident")
            make_identity(nc, ident[:])

            iota_i = pp.tile([41, 1], mybir.dt.int32, name="iota_i")
            nc.gpsimd.iota(iota_i[:], [[1, 1]], channel_multiplier=1)
            iota_f = pp.tile([41, 1], F32, name="iota_f")
            nc.vector.tensor_copy(iota_f[:], iota_i[:])

            ones_row = pp.tile([1, 128], F32, name="ones_row")
            nc.vector.memset(ones_row[:], 1.0)
            zerot = pp.tile([128, 512], F32, name="zerot")
            nc.vector.memset(zerot[:], 0.0)

            wtab_s = pp.tile([43, LATENT], F32, name="wtab_s")
            nc.sync.dma_start(wtab_s[:], d_wtab[:, :])

            g0_s, qh0_s = [], []
            w_s = {n: [] for n in WNAMES}
            b_s = {n: [] for n in BNAMES}
            for l in range(NLAYERS):
                g = pp.tile([LATENT, 512], BF16, name=f"g0s{l}", tag=f"g0s{l}")
                nc.sync.dma_start(g[:], d_g0[l, :, :])
                g0_s.append(g)
                q = pp.tile([NREF, LATENT], F32, name=f"qh0s{l}", tag=f"qh0s{l}")
                nc.sync.dma_start(q[:], d_qh0[l, :, :])
                qh0_s.append(q)
                for n in WNAMES:
                    if n == "wq1t":
                        # per-head [32,128] tiles (PE weights must start at
                        # partition 0/32/64, so a [96:128] slice is illegal)
                        hh_tiles = []
                        for h in range(NHEADS):
                            t = pp.tile([DH, LATENT], BF16, name=f"wq1t{l}h{h}",
                                        tag=f"wq1t{l}h{h}")
                            nc.sync.dma_start(
                                t[:], d_w[n][l, h * DH:(h + 1) * DH, :])
                            hh_tiles.append(t)
                        w_s[n].append(hh_tiles)
                        continue
                    t = pp.tile([LATENT, LATENT], BF16, name=f"{n}s{l}", tag=f"{n}s{l}")
                    nc.sync.dma_start(t[:], d_w[n][l, :, :])
                    w_s[n].append(t)
                for n in BNAMES:
                    if not flags[n]:
                        b_s[n].append(None)
                        continue
                    if n in ("bk1",):        # needed as a [128,1] column
                        t = pp.tile([LATENT, 1], F32, name=f"{n}s{l}", tag=f"{n}s{l}")
                        nc.sync.dma_start(
                            t[:], bass.AP(d_b[n], l * LATENT, [[1, LATENT], [1, 1]]))
                    elif n == "bq1":         # per-head column tiles [32,1]
                        t = []
                        for h in range(NHEADS):
                            th = pp.tile([DH, 1], F32, name=f"{n}c{l}h{h}",
                                         tag=f"{n}c{l}h{h}")
                            nc.sync.dma_start(
                                th[:], bass.AP(d_b[n], l * LATENT + h * DH,
                                               [[1, DH], [1, 1]]))
                            t.append(th)
                    else:
                        w = 512 if n == "r0" else LATENT
                        t = pp.tile([1, w], F32, name=f"{n}s{l}", tag=f"{n}s{l}")
                        nc.sync.dma_start(t[:], d_b[n][l:l + 1, :])
                    b_s[n].append(t)
                if flags["bq1"]:  # row form for the ones-matmul into Qh1
                    t = pp.tile([1, LATENT], F32, name=f"bq1rs{l}", tag=f"bq1rs{l}")
                    nc.sync.dma_start(t[:], d_b["bq1"][l:l + 1, :])
                    b_s.setdefault("bq1r", []).append(t)

            # ---------------- per-batch setup + Z0 ----------------
            mkp_s, mkneg_s, mkt_s, ZT, ZTB = [], [], [], [], []
            mkpB = pp.tile([128, 2 * NT], F32, name="mkpB")
            for b in range(BPC):
                mkt = pp.tile([NT, 128], F32, name=f"mkt{b}", tag=f"mkt{b}")
                nc.sync.dma_start(mkt[:], bass.AP(d_mk, b * NTP, [[128, NT], [1, 128]]))
                mkt_s.append(mkt)
                mkpp = ps_sm.tile([128, NT], F32, name="mkpp", tag="sm")
                nc.tensor.transpose(mkpp[:], mkt[:], ident[0:NT, 0:NT])
                mkp = mkpB[:, b * NT:(b + 1) * NT]
                nc.vector.tensor_copy(mkp, mkpp[:])
                mkp_s.append(mkp)
                mkneg = pp.tile([128, NT], F32, name=f"mkneg{b}", tag=f"mkneg{b}")
                nc.vector.tensor_scalar(
                    mkneg[:], mkp, -1.0, -NEGBIG, op0=OP.add, op1=OP.mult)
                mkneg_s.append(mkneg)

                xt = pp.tile([43, NTP], F32, name=f"xt{b}", tag="xt")
                crow = pp.tile([1, NTP], F32, name=f"crow{b}", tag="crow")
                nc.sync.dma_start(crow[:], d_cidx[b:b + 1, :])
                # replicate cidx row across 41 partitions via a K=1 matmul,
                # then one-hot it against the iota column
                for j in range(0, NTP, 512):
                    w = min(512, NTP - j)
                    cb = ps_sm.tile([41, 512], F32, name="cb", tag="sm")
                    nc.tensor.matmul(cb[:, :w], lhsT=ones_row[:, 0:41],
                                     rhs=crow[:, j:j + w], start=True, stop=True)
                    nc.vector.tensor_scalar(
                        xt[0:41, j:j + w], cb[:, :w], iota_f[:], None,
                        op0=OP.is_equal)
                nc.sync.dma_start(xt[41:42, :], d_tmk[b:b + 1, :])
                nc.sync.dma_start(xt[42:43, :], d_umk[b:b + 1, :])

                ztg, ztbg = [], []
                for g in range(NG):
                    zt = pp.tile([128, GW * 128], F32, name=f"zt{b}_{g}",
                                 tag=f"zt{b}_{g}")
                    ztg.append(zt)
                    ztb = pp.tile([128, GW * 128], BF16, name=f"ztb{b}_{g}",
                                  tag=f"ztb{b}_{g}")
                    ztbg.append(ztb)
                for c in range(NT):
                    g, js = gslice(c)
                    z0p = ps_sm.tile([128, 128], F32, name="z0p", tag="sm")
                    nc.tensor.matmul(
                        z0p[:], lhsT=wtab_s[:], rhs=xt[:, c * 128:(c + 1) * 128],
                        start=True, stop=True)
                    nc.vector.tensor_scalar_max(ztg[g][:, js], z0p[:], 0.0)
                    nc.gpsimd.tensor_copy(ztbg[g][:, js], ztg[g][:, js])
                ZT.append(ztg)
                ZTB.append(ztbg)

            # ---------------- layers ----------------
            for l in range(nlayers):
                g1_b, vo1_b, r1b_b = {}, {}, {}
                num0p = wp.tile([128, 264], F32, name="num0p", tag="num0p")
                nc.vector.memset(num0p[:], 0.0)
                # ===== MAB0: induced points attend to data =====
                # batches paired: one [128,264] VO build + one accumulator
                # add per chunk instead of two of each (DVE-bound phase)
                for c in range(NT):
                    ets = {}
                    vhp = ps_sm.tile([128, 256], F32, name="vhp", tag="sm")
                    for b in range(BPC):
                        ztbg = ZTB[b]
                        g, js = gslice(c)
                        s0 = ps_sc.tile([128, 512], F32, name="s0", tag="sc")
                        if flags["r0"]:
                            nc.tensor.matmul(s0[:], lhsT=ones_row[:],
                                             rhs=b_s["r0"][l][:],
                                             start=True, stop=False)
                        nc.tensor.matmul(s0[:], lhsT=ztbg[g][:, js], rhs=g0_s[l][:],
                                         start=not flags["r0"], stop=True)
                        et = sp.tile([128, 512], BF16, name="et", tag="et")
                        nc.scalar.activation(et[:], s0[:], AF.Exp,
                                             bias=mkneg_s[b][:, c:c + 1],
                                             scale=SCALE)
                        ets[b] = et
                        if flags["bv0"]:
                            nc.tensor.matmul(vhp[:, b * 128:(b + 1) * 128],
                                             lhsT=ones_row[:],
                                             rhs=b_s["bv0"][l][:],
                                             start=True, stop=False)
                        nc.tensor.matmul(vhp[:, b * 128:(b + 1) * 128],
                                         lhsT=ztbg[g][:, js],
                                         rhs=w_s["wv0"][l][:],
                                         start=not flags["bv0"], stop=True)
                    vo = sp.tile([128, 264], BF16, name="vo", tag="vo")
                    nc.gpsimd.memset(vo[:], 1.0)
                    nc.scalar.copy(
                        vo[:].rearrange("p (x h v) -> p x h v", x=2, v=33)[:, :, :, 0:32],
                        vhp[:].rearrange("p (x h v) -> p x h v", x=2, v=32))
                    n0c = ps_n1.tile([128, 264], F32, name="n0c", tag="nacc")
                    for b in range(BPC):
                        for h in range(NHEADS):
                            o = b * 132 + h * 33
                            nc.tensor.matmul(
                                n0c[:, o:o + 33],
                                lhsT=ets[b][:, h * 128:(h + 1) * 128],
                                rhs=vo[:, o:o + 33],
                                start=True, stop=True)
                    nc.vector.tensor_add(num0p[:], num0p[:], n0c[:])
                # ===== MAB0 tail + MAB1 prep (per batch) =====
                for b in range(BPC):
                    ztg = ZT[b]
                    num0 = num0p[:, b * 132:(b + 1) * 132]
                    rd0 = sp.tile([128, 4], F32, name="rd0", tag="rd0")
                    nc.vector.reciprocal(
                        rd0[:].rearrange("p (h x) -> p h x", x=1),
                        num0.rearrange("p (h x) -> p h x", x=33)[:, :, 32:33])
                    o0 = wp.tile([128, 128], F32, name="o0", tag=f"o0{b}")
                    nc.vector.tensor_tensor(
                        o0[:].rearrange("p (h x) -> p h x", x=32),
                        num0.rearrange("p (h x) -> p h x", x=33)[:, :, 0:32],
                        rd0[:].rearrange("p (h x) -> p h x", x=1).to_broadcast(
                            [128, 4, 32]),
                        op=OP.mult)
                    nc.vector.tensor_add(o0[:], o0[:], qh0_s[l][:])
                    o0tp = ps_sm.tile([128, 128], F32, name="o0tp", tag="sm")
                    nc.tensor.transpose(o0tp[:], o0[:], ident[:])
                    o0t = wp.tile([128, 128], BF16, name="o0t", tag=f"o0t{b}")
                    nc.vector.tensor_copy(o0t[:], o0tp[:])
                    fc0 = ps_sm.tile([128, 128], F32, name="fc0", tag="sm")
                    if flags["bo0"]:
                        nc.tensor.matmul(fc0[:], lhsT=ones_row[:],
                                         rhs=b_s["bo0"][l][:], start=True, stop=False)
                    nc.tensor.matmul(fc0[:], lhsT=o0t[:], rhs=w_s["wo0"][l][:],
                                     start=not flags["bo0"], stop=True)
                    hh = wp.tile([128, 128], F32, name="hh", tag=f"hh{b}")
                    nc.vector.scalar_tensor_tensor(
                        hh[:], in0=fc0[:], scalar=0.0, in1=o0[:],
                        op0=OP.max, op1=OP.add)
                    htp = ps_sm.tile([128, 128], F32, name="htp", tag="sm")
                    nc.tensor.transpose(htp[:], hh[:], ident[:])
                    ht = wp.tile([128, 128], BF16, name="ht", tag=f"ht{b}")
                    nc.vector.tensor_copy(ht[:], htp[:])

                    # ===== MAB1 prep =====
                    kh1p = ps_sm.tile([128, 128], F32, name="kh1p", tag="sm")
                    nc.tensor.matmul(kh1p[:], lhsT=w_s["wk1"][l][:], rhs=ht[:],
                                     start=True, stop=True)
                    kh1 = wp.tile([128, 128], BF16, name="kh1", tag=f"kh1{b}")
                    if flags["bk1"]:
                        nc.vector.tensor_scalar_add(kh1[:], kh1p[:], b_s["bk1"][l][:])
                    else:
                        nc.vector.tensor_copy(kh1[:], kh1p[:])
                    kh1h = []
                    for h in range(NHEADS):
                        t = wp.tile([DH, LATENT], BF16, name=f"kh1h{h}", tag=f"kh1h{h}")
                        nc.sync.dma_start(t[:], kh1[h * DH:(h + 1) * DH, :])
                        kh1h.append(t)
                    g1p = ps_sc.tile([128, 512], F32, name="g1p", tag="sc")
                    for h in range(NHEADS):
                        nc.tensor.matmul(g1p[:, h * 128:(h + 1) * 128],
                                         lhsT=w_s["wq1t"][l][h][:], rhs=kh1h[h][:],
                                         start=True, stop=True)
                    g1 = wp.tile([128, 512], BF16, name="g1", tag=f"g1{b}")
                    nc.vector.tensor_copy(g1[:], g1p[:])
                    vh1p = ps_sm.tile([128, 128], F32, name="vh1p", tag="sm")
                    if flags["bv1"]:
                        nc.tensor.matmul(vh1p[:], lhsT=ones_row[:],
                                         rhs=b_s["bv1"][l][:], start=True, stop=False)
                    nc.tensor.matmul(vh1p[:], lhsT=ht[:], rhs=w_s["wv1"][l][:],
                                     start=not flags["bv1"], stop=True)
                    vo1 = wp.tile([128, 132], BF16, name="vo1", tag=f"vo1{b}")
                    nc.vector.memset(vo1[:], 1.0)
                    nc.vector.tensor_copy(
                        vo1[:].rearrange("p (h x) -> p h x", x=33)[:, :, 0:32],
                        vh1p[:].rearrange("p (h x) -> p h x", x=32))
                    r1b = None
                    if flags["bq1"]:
                        r1bp = ps_sm.tile([128, 4], F32, name="r1bp", tag="sm")
                        for h in range(NHEADS):
                            nc.tensor.matmul(r1bp[:, h:h + 1], lhsT=kh1h[h][:],
                                             rhs=b_s["bq1"][l][h][:],
                                             start=True, stop=True)
                        r1b = wp.tile([128, 4], F32, name="r1b", tag=f"r1b{b}")
                        nc.vector.tensor_scalar_mul(r1b[:], r1bp[:], SCALE)
                    g1_b[b], vo1_b[b], r1b_b[b] = g1, vo1, r1b

                # ===== MAB1 chunks: data attends to induced (b-interleaved) ==
                for g in range(NG):
                    gw = min(GW * 128, NTP - g * GW * 128)
                    e1h_b = {}
                    for b in range(BPC):
                        ztbg = ZTB[b]
                        g1, r1b = g1_b[b], r1b_b[b]
                        e1h = []
                        for h in range(NHEADS):
                            s1 = ps_sc.tile([128, GW * 128], F32, name="s1", tag="sc")
                            nc.tensor.matmul(s1[:, :gw],
                                             lhsT=g1[:, h * 128:(h + 1) * 128],
                                             rhs=ztbg[g][:, :gw],
                                             start=True, stop=True)
                            e1 = sp.tile([128, GW * 128], BF16, name="e1", tag="e1")
                            if flags["bq1"]:
                                nc.scalar.activation(e1[:, :gw], s1[:, :gw], AF.Exp,
                                                     bias=r1b[:, h:h + 1], scale=SCALE)
                            else:
                                nc.scalar.activation(e1[:, :gw], s1[:, :gw], AF.Exp,
                                                     scale=SCALE)
                            e1h.append(e1)
                        e1h_b[b] = e1h
                    if not flags["bq1"] and not flags["bo1"]:
                        # both batch elements' chunk pipelines fused into
                        # shared wide tiles: one DVE op per stage
                        for j in range(gw // 128):
                            c = g * GW + j
                            js = slice(j * 128, (j + 1) * 128)
                            num1p = ps_n1.tile([128, 264], F32, name="num1p",
                                               tag="nacc")
                            for b in range(BPC):
                                for h in range(NHEADS):
                                    o = b * 132 + h * 33
                                    nc.tensor.matmul(
                                        num1p[:, o:o + 33],
                                        lhsT=e1h_b[b][h][:, js],
                                        rhs=vo1_b[b][:, h * 33:(h + 1) * 33],
                                        start=True, stop=True)
                            qh1p = ps_sm.tile([128, 256], F32, name="qh1p", tag="sm")
                            for b in range(BPC):
                                nc.tensor.matmul(qh1p[:, b * 128:(b + 1) * 128],
                                                 lhsT=ZTB[b][g][:, js],
                                                 rhs=w_s["wq1"][l][:],
                                                 start=True, stop=True)
                            rd1 = sp.tile([128, 8], F32, name="rd1", tag="rd1")
                            nc.vector.reciprocal(
                                rd1[:].rearrange("p (x h v) -> p x h v", x=2, v=1),
                                num1p[:].rearrange("p (x h v) -> p x h v",
                                                   x=2, v=33)[:, :, :, 32:33])
                            rdm = sp.tile([128, 8], F32, name="rdm", tag="rdm")
                            nc.vector.tensor_tensor(
                                rdm[:].rearrange("p (x h) -> p x h", x=2),
                                rd1[:].rearrange("p (x h) -> p x h", x=2),
                                mkpB[:].rearrange("p (x c) -> p x c",
                                                  x=2)[:, :, c:c + 1].to_broadcast(
                                    [128, 2, NHEADS]),
                                op=OP.mult)
                            o1p = sp.tile([128, 256], F32, name="o1p", tag="o1")
                            nc.vector.tensor_tensor(
                                o1p[:].rearrange("p (x h v) -> p x h v", x=2, v=32),
                                num1p[:].rearrange("p (x h v) -> p x h v",
                                                   x=2, v=33)[:, :, :, 0:32],
                                rdm[:].rearrange("p (x h v) -> p x h v",
                                                 x=2, v=1).to_broadcast(
                                    [128, 2, NHEADS, 32]),
                                op=OP.mult)
                            nc.vector.tensor_add(o1p[:], o1p[:], qh1p[:])
                            o1tp = ps_sm.tile([128, 256], F32, name="o1tp", tag="sm")
                            for b in range(BPC):
                                nc.tensor.transpose(
                                    o1tp[:, b * 128:(b + 1) * 128],
                                    o1p[:, b * 128:(b + 1) * 128], ident[:])
                            o1t = sp.tile([128, 256], BF16, name="o1t", tag="o1t")
                            nc.scalar.copy(o1t[:], o1tp[:])
                            fc1p = ps_sm.tile([128, 256], F32, name="fc1p", tag="sm")
                            for b in range(BPC):
                                nc.tensor.matmul(fc1p[:, b * 128:(b + 1) * 128],
                                                 lhsT=w_s["wo1"][l][:],
                                                 rhs=o1t[:, b * 128:(b + 1) * 128],
                                                 start=True, stop=True)
                            for b in range(BPC):
                                bs = slice(b * 128, (b + 1) * 128)
                                nc.vector.tensor_add(ZT[b][g][:, js], ZT[b][g][:, js],
                                                     o1tp[:, bs])
                                nc.vector.scalar_tensor_tensor(
                                    ZT[b][g][:, js], in0=fc1p[:, bs], scalar=0.0,
                                    in1=ZT[b][g][:, js], op0=OP.max, op1=OP.add)
                                nc.gpsimd.tensor_copy(ZTB[b][g][:, js],
                                                      ZT[b][g][:, js])
                        continue
                    for b in range(BPC):
                        ztg, ztbg = ZT[b], ZTB[b]
                        g1, vo1, r1b = g1_b[b], vo1_b[b], r1b_b[b]
                        e1h = e1h_b[b]
                        for j in range(gw // 128):
                            c = g * GW + j
                            js = slice(j * 128, (j + 1) * 128)
                            num1 = ps_n1.tile([128, 132], F32, name="num1", tag="nacc")
                            for h in range(NHEADS):
                                nc.tensor.matmul(num1[:, h * 33:(h + 1) * 33],
                                                 lhsT=e1h[h][:, js],
                                                 rhs=vo1[:, h * 33:(h + 1) * 33],
                                                 start=True, stop=True)
                            qh1 = ps_sm.tile([128, 128], F32, name="qh1", tag="sm")
                            if flags["bq1"]:
                                nc.tensor.matmul(qh1[:], lhsT=ones_row[:],
                                                 rhs=b_s["bq1r"][l][:],
                                                 start=True, stop=False)
                            nc.tensor.matmul(qh1[:], lhsT=ztbg[g][:, js],
                                             rhs=w_s["wq1"][l][:],
                                             start=not flags["bq1"], stop=True)
                            rd1 = sp.tile([128, 4], F32, name="rd1", tag="rd1")
                            nc.vector.reciprocal(
                                rd1[:].rearrange("p (h x) -> p h x", x=1),
                                num1[:].rearrange("p (h x) -> p h x", x=33)[:, :, 32:33])
                            o1 = sp.tile([128, 128], F32, name="o1", tag="o1")
                            if not flags["bo1"]:
                                # masked-O1 path: invalid rows are exactly
                                # zeroed through fc_o since bo1 == 0
                                nc.vector.tensor_scalar_mul(
                                    rd1[:], rd1[:], mkp_s[b][:, c:c + 1])
                            nc.vector.tensor_tensor(
                                o1[:].rearrange("p (h x) -> p h x", x=32),
                                num1[:].rearrange("p (h x) -> p h x", x=33)[:, :, 0:32],
                                rd1[:].rearrange("p (h x) -> p h x", x=1).to_broadcast(
                                    [128, 4, 32]),
                                op=OP.mult)
                            if flags["bo1"]:
                                nc.vector.tensor_add(o1[:], o1[:], qh1[:])
                            else:
                                nc.vector.scalar_tensor_tensor(
                                    o1[:], in0=qh1[:], scalar=mkp_s[b][:, c:c + 1],
                                    in1=o1[:], op0=OP.mult, op1=OP.add)
                            o1tp = ps_sm.tile([128, 128], F32, name="o1tp", tag="sm")
                            nc.tensor.transpose(o1tp[:], o1[:], ident[:])
                            o1t = sp.tile([128, 128], BF16, name="o1t", tag="o1t")
                            nc.scalar.copy(o1t[:], o1tp[:])
                            fc1 = ps_sm.tile([128, 128], F32, name="fc1", tag="sm")
                            if flags["bo1"]:
                                # token-major fallback path with explicit mask
                                nc.tensor.matmul(fc1[:], lhsT=ones_row[:],
                                                 rhs=b_s["bo1"][l][:],
                                                 start=True, stop=False)
                                nc.tensor.matmul(fc1[:], lhsT=o1t[:],
                                                 rhs=w_s["wo1"][l][:],
                                                 start=False, stop=True)
                                u = sp.tile([128, 128], F32, name="u", tag="u")
                                nc.vector.scalar_tensor_tensor(
                                    u[:], in0=fc1[:], scalar=0.0, in1=o1[:],
                                    op0=OP.max, op1=OP.add)
                                nc.vector.tensor_scalar_mul(u[:], u[:],
                                                            mkp_s[b][:, c:c + 1])
                                utp = ps_sm.tile([128, 128], F32, name="utp", tag="sm")
                                nc.tensor.transpose(utp[:], u[:], ident[:])
                                nc.vector.tensor_add(ztg[g][:, js], ztg[g][:, js],
                                                     utp[:])
                            else:
                                # fc computed lat-major (lhsT = Wo1 fixed);
                                # O1m is pre-masked so U = O1m + relu(fc)
                                # needs no further masking: fold the residual
                                # as two in-place adds on Z^T (no U transpose)
                                nc.tensor.matmul(fc1[:], lhsT=w_s["wo1"][l][:],
                                                 rhs=o1t[:], start=True, stop=True)
                                nc.vector.tensor_add(ztg[g][:, js], ztg[g][:, js],
                                                     o1tp[:])
                                nc.vector.scalar_tensor_tensor(
                                    ztg[g][:, js], in0=fc1[:], scalar=0.0,
                                    in1=ztg[g][:, js], op0=OP.max, op1=OP.add)
                            nc.gpsimd.tensor_copy(ztbg[g][:, js], ztg[g][:, js])

            # ---------------- output ----------------
            for c in range(NT):
                for b in range(BPC):
                    g, js = gslice(c)
                    zp = ps_sm.tile([128, 128], F32, name="zp", tag="sm")
                    nc.tensor.transpose(zp[:], ZT[b][g][:, js], ident[:])
                    zo = sp.tile([128, 128], F32, name="zo", tag="zo")
                    nc.vector.tensor_copy(zo[:], zp[:])
                    nc.sync.dma_start(
                        bass.AP(d_zout, (b * L + c * 128) * LATENT,
                                [[LATENT, 128], [1, LATENT]]),
                        zo[:])
            for b in range(BPC):
                # zero tail rows [NTP, L): 2KB-contiguous descriptor runs
                r = NTP
                while r < L:
                    n = min(512, L - r)   # rows; n*128 elems; dst stays contiguous
                    nelem = n * LATENT
                    inner = nelem // 128
                    nc.sync.dma_start(
                        bass.AP(d_zout, (b * L + r) * LATENT,
                                [[inner, 128], [1, inner]]),
                        zerot[:, :inner])
                    r += n
                # mask output (contiguous 512B rows from the token-major tile)
                nc.sync.dma_start(
                    bass.AP(d_mkout, b * L, [[128, NT], [1, 128]]), mkt_s[b][:])
                if CMAX > NT:
                    nc.sync.dma_start(
                        bass.AP(d_mkout, b * L + NTP, [[128, CMAX - NT], [1, 128]]),
                        zerot[0:CMAX - NT, 0:128])
    nc.compile()
    return nc


def _prep(inputs):
    """Host-side prep: compaction indices + weight folding (all O(small))."""
    time_x = np.ascontiguousarray(np.asarray(inputs["time_x"], np.float32))
    value_x = np.ascontiguousarray(np.asarray(inputs["value_x"], np.float32))
    mask_x = np.asarray(inputs["mask_x"])
    Wi = np.asarray(inputs["Wi"], np.float32)
    bi = np.asarray(inputs["bi"], np.float32)
    I = np.asarray(inputs["I"], np.float32)
    Wq = np.asarray(inputs["Wq"], np.float32)
    bq = np.asarray(inputs["bq"], np.float32)
    Wk = np.asarray(inputs["Wk"], np.float32)
    bk = np.asarray(inputs["bk"], np.float32)
    Wv = np.asarray(inputs["Wv"], np.float32)
    bv = np.asarray(inputs["bv"], np.float32)
    Wo = np.asarray(inputs["Wo"], np.float32)
    bo = np.asarray(inputs["bo"], np.float32)

    mflat = mask_x.reshape(B, L)
    order = np.argsort(1 - mflat, axis=1, kind="stable")
    nvalid = int(mflat.sum(axis=1).max())
    NT = max(1, min(CMAX, -(-nvalid // 128)))
    NTP = NT * 128

    ordp = order[:, :NTP]
    mkc = np.take_along_axis(mflat, ordp, axis=1).astype(np.float32)
    tfull = np.broadcast_to(time_x[:, :, None], (B, S, D)).reshape(B, L)
    tmk = np.take_along_axis(tfull, ordp, axis=1) * mkc
    umk = np.take_along_axis(value_x.reshape(B, L), ordp, axis=1) * mkc
    cidx = np.where(mkc > 0, (ordp % D).astype(np.float32), 63.0).astype(np.float32)

    wtab = np.concatenate([Wi[:D] + bi[None, :], Wi[D:D + 2]], 0)  # [43,128]

    g0 = np.zeros((NLAYERS, LATENT, 512), np.float32)
    qh0 = np.zeros((NLAYERS, NREF, LATENT), np.float32)
    r0 = np.zeros((NLAYERS, 512), np.float32)
    for l in range(NLAYERS):
        Q = I[l] @ Wq[l, 0] + bq[l, 0]
        qh0[l] = Q
        for h in range(NHEADS):
            hs = slice(h * DH, (h + 1) * DH)
            g0[l][:, h * NREF:(h + 1) * NREF] = Wk[l, 0][:, hs] @ Q[:, hs].T
            r0[l][h * NREF:(h + 1) * NREF] = bk[l, 0][hs] @ Q[:, hs].T

    w = dict(
        wv0=Wv[:, 0], wo0=Wo[:, 0], wq1=Wq[:, 1],
        wq1t=np.ascontiguousarray(Wq[:, 1].transpose(0, 2, 1)),
        wk1=Wk[:, 1], wv1=Wv[:, 1], wo1=Wo[:, 1],
    )
    bvec = dict(r0=r0, bv0=bv[:, 0], bo0=bo[:, 0], bq1=bq[:, 1],
                bk1=bk[:, 1], bv1=bv[:, 1], bo1=bo[:, 1])
    flags = {n: bool(np.any(v != 0)) for n, v in bvec.items()}
    return dict(NT=NT, NTP=NTP, tmk=tmk, umk=umk, mkc=mkc, cidx=cidx,
                wtab=wtab, g0=g0, qh0=qh0, w=w, bvec=bvec, flags=flags)


def kernel(**inputs):
    global LAST_RESULT
    p = _prep(inputs)

    key = (p["NT"], tuple(sorted(p["flags"].items())))
    if key not in _PROG_CACHE:
        _PROG_CACHE[key] = _build_program(p["NT"], p["flags"])
    nc = _PROG_CACHE[key]

    import ml_dtypes
    bf16 = ml_dtypes.bfloat16
    shared = dict(wtab=np.ascontiguousarray(p["wtab"]),
                  g0=np.ascontiguousarray(p["g0"].astype(bf16)), qh0=p["qh0"])
    for n, v in p["w"].items():
        shared[n] = np.ascontiguousarray(v.astype(bf16))
    for n, v in p["bvec"].items():
        shared[n] = np.ascontiguousarray(v)

    in_maps = []
    for m in range(NCORES):
        sl = slice(m * BPC, (m + 1) * BPC)
        im = dict(shared)
        im["cidx"] = np.ascontiguousarray(p["cidx"][sl])
        im["tmk"] = np.ascontiguousarray(p["tmk"][sl])
        im["umk"] = np.ascontiguousarray(p["umk"][sl])
        im["mk"] = np.ascontiguousarray(p["mkc"][sl])
        in_maps.append(im)

    res = run_bass_kernel_spmd(nc, in_maps, core_ids=list(range(NCORES)),
                               trace=TRACE)
    LAST_RESULT = res

    Z = np.concatenate([r["zout"] for r in res.results], axis=0)
    mk = np.concatenate([r["mkout"] for r in res.results], axis=0)[..., None]
    return Z.reshape(B, L, LATENT), mk.reshape(B, L, 1)
